# revision 1
# baseline (speedup 1.0000x reference)
"""Bass/Tile TRN2 kernel for EnhancedIPA3 (invariant-point-attention variant).

Sharding: 8 cores = batch(2) x query-block(4).  Each core computes Q/K/V
features for its own 256 rows; K-side features are all-gathered (bf16)
within the 4-core batch group; attention runs sequence-parallel over
query blocks.  All matmuls run fp32r (single-pass) or bf16.

Self-contained: hardcodes all shapes; only depends on numpy + concourse.
"""

import numpy as np
from contextlib import ExitStack

import concourse.bass as bass
import concourse.bacc as bacc
import concourse.mybir as mybir
import concourse.tile as tile
from concourse.bass_utils import run_bass_kernel_spmd
from concourse.masks import make_identity

F32 = mybir.dt.float32
F32R = mybir.dt.float32r
BF16 = mybir.dt.bfloat16
AF = mybir.ActivationFunctionType
OP = mybir.AluOpType

B, N, CS, H, C, P, V = 2, 1024, 384, 12, 16, 4, 8
EPS = 1e-8
NB = N // 4            # 256 rows per core
NT = NB // 128         # 2 row-tiles per core
NPTS = P + P + V       # 16 unified points per head (qp:0-3, kp:4-7, vp:8-15)
PTS_COLS = H * NPTS * 6          # 1152
Q_OFF, K_OFF, V_OFF, G_OFF, PTS_OFF = 0, 192, 384, 576, 624
WALL_COLS = PTS_OFF + PTS_COLS   # 1776
FEAT = 64              # padded per-head attention feature stride
FS = 42                # used attention features per head
OCH = 66               # v chans + ones col + pad (even for fp32r)
FOUT = H * (C + 7 * V)           # 864 output-proj input channels
KCH = 7                # contraction chunks for output proj (last = 98 rows)
GROUPS = [[0, 1, 2, 3], [4, 5, 6, 7]]
NKB = 8                # gathered key blocks of 128
KFSZ = 6 * 128 * NB    # kf gather elements
VASZ = NB * H * OCH    # va gather elements


def _host_prep(inputs):
    """Build the combined/permuted weight matrices and scale tables."""
    wq, bq = inputs["wq"], inputs["bq"]
    wkv, bkv = inputs["wkv"], inputs["bkv"]
    wqp, bqp = inputs["wqp"], inputs["bqp"]
    wkvp, bkvp = inputs["wkvp"], inputs["bkvp"]
    wg, bg = inputs["wg"], inputs["bg"]
    gw = np.asarray(inputs["geom_weight"], np.float32)
    hw = np.asarray(inputs["head_weights"], np.float32)
    sh = 1.0 / (1.0 + np.exp(-hw))           # sigmoid(head_weights) [H]

    wall = np.zeros((CS + 2, WALL_COLS), np.float32)
    wall[:CS, Q_OFF:Q_OFF + 192] = wq
    wall[CS, Q_OFF:Q_OFF + 192] = bq
    wall[:CS, K_OFF:K_OFF + 192] = wkv[:, :192]
    wall[CS, K_OFF:K_OFF + 192] = bkv[:192]
    wall[:CS, V_OFF:V_OFF + 192] = wkv[:, 192:]
    wall[CS, V_OFF:V_OFF + 192] = bkv[192:]
    wall[:CS, G_OFF:G_OFF + 48] = wg
    wall[CS, G_OFF:G_OFF + 48] = bg
    # unified point layout: col = PTS_OFF + h*96 + p*6 + cc
    for h in range(H):
        for p in range(NPTS):
            d0 = PTS_OFF + h * 96 + p * 6
            if p < P:
                s0 = h * (P * 6) + p * 6
                wall[:CS, d0:d0 + 6] = wqp[:, s0:s0 + 6]
                wall[CS, d0:d0 + 6] = bqp[s0:s0 + 6]
            else:
                s0 = h * ((P + V) * 6) + (p - P) * 6
                wall[:CS, d0:d0 + 6] = wkvp[:, s0:s0 + 6]
                wall[CS, d0:d0 + 6] = bkvp[s0:s0 + 6]
    # two half-bias rows (keeps fp32r contraction dims even)
    wall[CS + 1] = wall[CS] * 0.5
    wall[CS] = wall[CS + 1]

    bout_half = np.asarray(inputs["bout"], np.float32)[None, :] * 0.5
    wout_b = np.concatenate(
        [np.asarray(inputs["wout"], np.float32), bout_half, bout_half],
        axis=0)  # [866, 384]

    # per-column scale for the assembled Qfeat [n, H*FEAT]
    qs = np.zeros((FEAT * H,), np.float32)
    for h in range(H):
        o = h * FEAT
        qs[o:o + 16] = sh[h] / np.sqrt(C)        # scalar q . k
        qs[o + 16:o + 28] = sh[h] * gw[0] * 0.5  # 2*gw0/P * (qc.kc), P=4
        qs[o + 28:o + 40] = sh[h] * gw[1]        # gw1 * (qd.kd)
        qs[o + 40] = sh[h]                       # * (-gw0/P * k2sum)
        qs[o + 41] = sh[h]                       # combo col * 1
    qscale = np.broadcast_to(qs, (128, FEAT * H)).copy()

    rot9 = np.ascontiguousarray(
        np.asarray(inputs["rot"], np.float32).reshape(B, N, 9))
    trans = np.asarray(inputs["trans"], np.float32)
    s = np.asarray(inputs["s"], np.float32)
    return s, rot9, trans, wall, wout_b, qscale, gw


_PROGRAM_CACHE = {}


def _build_program(gw0, gw1):
    key = (float(gw0), float(gw1))
    if key in _PROGRAM_CACHE:
        return _PROGRAM_CACHE[key]

    nc = bacc.Bacc("TRN2", target_bir_lowering=False, debug=False, num_devices=8)

    s_loc = nc.dram_tensor("s_loc", [NB, CS], F32, kind="ExternalInput")
    rot_loc = nc.dram_tensor("rot_loc", [NB, 9], F32, kind="ExternalInput")
    trans_loc = nc.dram_tensor("trans_loc", [NB, 3], F32, kind="ExternalInput")
    wall_d = nc.dram_tensor("wall", [CS + 2, WALL_COLS], F32, kind="ExternalInput")
    wout_d = nc.dram_tensor("wout_b", [FOUT + 2, CS], F32, kind="ExternalInput")
    qscale_d = nc.dram_tensor("qscale", [128, FEAT * H], F32, kind="ExternalInput")
    out_loc = nc.dram_tensor("out_loc", [NB, CS], F32, kind="ExternalOutput")

    kf_loc = nc.dram_tensor("kf_loc", [KFSZ], BF16)
    kf_gath = nc.dram_tensor("kf_gath", [4, KFSZ], BF16)
    va_loc = nc.dram_tensor("va_loc", [VASZ], BF16)
    va_gath = nc.dram_tensor("va_gath", [4, VASZ], BF16)

    with tile.TileContext(nc) as tc:
        with ExitStack() as ctx:
            _emit(ctx, tc, nc, s_loc, rot_loc, trans_loc, wall_d, wout_d,
                  qscale_d, out_loc, kf_loc, kf_gath, va_loc, va_gath, gw0, gw1)

    nc.compile()
    _PROGRAM_CACHE[key] = nc
    return nc


def _emit(ctx, tc, nc, s_loc, rot_loc, trans_loc, wall_d, wout_d, qscale_d,
          out_loc, kf_loc, kf_gath, va_loc, va_gath, gw0, gw1):
    PS = bass.MemorySpace.PSUM

    const = ctx.enter_context(tc.tile_pool(name="const", bufs=1))
    work = ctx.enter_context(tc.tile_pool(name="work", bufs=1))
    tmp_pool = ctx.enter_context(tc.tile_pool(name="tmp", bufs=2))
    pA_ctx = ExitStack()
    pA = pA_ctx.enter_context(tc.tile_pool(name="pA", bufs=1))
    pre_ctx = ExitStack()
    tpsum = pre_ctx.enter_context(tc.tile_pool(name="tpsum", bufs=3, space=PS))

    # ---- constants -------------------------------------------------------
    ident = const.tile([128, 128], F32)
    make_identity(nc, ident[:])
    ident_r = const.tile([128, 128], F32R)
    nc.vector.tensor_copy(ident_r[:], ident[:])
    ones2_f32 = const.tile([2, NB], F32)
    nc.gpsimd.memset(ones2_f32[:], 1.0)
    ones_row = const.tile([2, NB], F32R)
    nc.vector.tensor_copy(ones_row[:], ones2_f32[:])

    wall_sb = []
    for kc in range(3):
        t = pA.tile([128, WALL_COLS], F32R, name=f"wall{kc}")
        nc.sync.dma_start(t[:], wall_d[kc * 128:(kc + 1) * 128, :].bitcast(F32R))
        wall_sb.append(t)
    wall_bias = pA.tile([2, WALL_COLS], F32R)
    nc.sync.dma_start(wall_bias[:], wall_d[CS:CS + 2, :].bitcast(F32R))

    wout_sb = []
    for kc in range(KCH):
        r0 = kc * 128
        r1 = min(FOUT + 2, r0 + 128)
        t = const.tile([r1 - r0, CS], F32R, name=f"wout{kc}")
        nc.sync.dma_start(t[:], wout_d[r0:r1, :].bitcast(F32R))
        wout_sb.append(t)

    qscale_sb = const.tile([128, FEAT * H], F32)
    nc.sync.dma_start(qscale_sb[:], qscale_d[:, :])

    # ---- inputs ----------------------------------------------------------
    s_sb, rot_sb, trans_sb = [], [], []
    for nt in range(NT):
        r = slice(nt * 128, (nt + 1) * 128)
        t = pA.tile([128, CS], F32, name=f"s{nt}")
        nc.sync.dma_start(t[:], s_loc[r, :])
        s_sb.append(t)
        t = const.tile([128, 9], F32, name=f"rot{nt}")
        nc.sync.dma_start(t[:], rot_loc[r, :])
        rot_sb.append(t)
        t = const.tile([128, 3], F32, name=f"trans{nt}")
        nc.sync.dma_start(t[:], trans_loc[r, :])
        trans_sb.append(t)

    # ---- sT (transpose s) ------------------------------------------------
    sT = [pA.tile([128, NB], F32R, name=f"sT{kc}") for kc in range(3)]
    for nt in range(NT):
        for kc in range(3):
            ps = tpsum.tile([128, 128], F32, tag="tps")
            nc.tensor.transpose(ps[:], s_sb[nt][:, kc * 128:(kc + 1) * 128], ident[:])
            nc.scalar.copy(sT[kc][:, nt * 128:(nt + 1) * 128], ps[:])

    # ---- projections (pts-bearing col blocks first: K-side critical) -----
    q_sb = [work.tile([128, 192], F32, name=f"q{nt}") for nt in range(NT)]
    k_sb = [work.tile([128, 192], F32, name=f"k{nt}") for nt in range(NT)]
    v_sb = [work.tile([128, 192], F32, name=f"v{nt}") for nt in range(NT)]
    g_sb = [work.tile([128, 48], F32, name=f"g{nt}") for nt in range(NT)]
    pts_sb = [work.tile([128, PTS_COLS], F32, name=f"pts{nt}") for nt in range(NT)]

    CB = [(512, 1024), (1024, 1536), (1536, WALL_COLS), (0, 512)]
    regions = [(Q_OFF, 192, q_sb, "copy"), (K_OFF, 192, k_sb, "copy"),
               (V_OFF, 192, v_sb, "copy"), (G_OFF, 48, g_sb, "sigmoid"),
               (PTS_OFF, PTS_COLS, pts_sb, "relu")]
    ppsum = pre_ctx.enter_context(tc.tile_pool(name="ppsum", bufs=3, space=PS))
    for (c0, c1), nt in [(cb, nt) for cb in CB for nt in range(NT)]:
        if True:
            nsl = slice(nt * 128, (nt + 1) * 128)
            ps = ppsum.tile([128, c1 - c0], F32, tag="proj", name="ps")
            for kc in range(3):
                nc.tensor.matmul(ps[:], sT[kc][:, nsl], wall_sb[kc][:, c0:c1],
                                 start=(kc == 0), stop=False)
            nc.tensor.matmul(ps[:], ones_row[:, nsl], wall_bias[:, c0:c1],
                             start=False, stop=True)
            for (r0, rw, dst, kind) in regions:
                lo, hi = max(r0, c0), min(r0 + rw, c1)
                if lo >= hi:
                    continue
                src = ps[:, lo - c0:hi - c0]
                dv = dst[nt][:, lo - r0:hi - r0]
                if kind == "copy":
                    nc.vector.tensor_copy(dv, src)
                elif kind == "sigmoid":
                    nc.scalar.activation(dv, src, AF.Sigmoid)
                else:
                    nc.scalar.activation(dv, src, AF.Relu)

    # ---- rigid transform (coords + dirs), all 16 points ------------------
    # pco layout per nt: [co0|co1|co2|di0|di1|di2], each block [128, H*NPTS=192]
    pco = [work.tile([128, 6 * 192], F32, name=f"pco{nt}") for nt in range(NT)]
    for nt in range(NT):
        pview = pts_sb[nt][:].rearrange("p (m c) -> p m c", c=6)
        rt = rot_sb[nt]
        tr = trans_sb[nt]
        for i in range(3):
            dco = pco[nt][:, i * 192:(i + 1) * 192]
            nc.vector.tensor_scalar(dco, pview[:, :, 0], rt[:, 3 * i:3 * i + 1],
                                    tr[:, i:i + 1], OP.mult, OP.add)
            nc.vector.scalar_tensor_tensor(dco, pview[:, :, 1],
                                           rt[:, 3 * i + 1:3 * i + 2], dco,
                                           OP.mult, OP.add)
            nc.vector.scalar_tensor_tensor(dco, pview[:, :, 2],
                                           rt[:, 3 * i + 2:3 * i + 3], dco,
                                           OP.mult, OP.add)
            ddi = pco[nt][:, (3 + i) * 192:(4 + i) * 192]
            nc.vector.tensor_scalar_mul(ddi, pview[:, :, 3], rt[:, 3 * i:3 * i + 1])
            nc.vector.scalar_tensor_tensor(ddi, pview[:, :, 4],
                                           rt[:, 3 * i + 1:3 * i + 2], ddi,
                                           OP.mult, OP.add)
            nc.vector.scalar_tensor_tensor(ddi, pview[:, :, 5],
                                           rt[:, 3 * i + 2:3 * i + 3], ddi,
                                           OP.mult, OP.add)

    def comp(nt, i, p0, pn):
        """[128, H, pn] view of component block i, points p0..p0+pn."""
        blk = pco[nt][:, i * 192:(i + 1) * 192]
        return blk.rearrange("p (h x) -> p h x", x=NPTS)[:, :, p0:p0 + pn]

    # ---- K side first: k2, Kfeat, transpose, DMA, kf collective ----------
    k2c = [work.tile([128, H], F32, name=f"k2c{nt}") for nt in range(NT)]

    def sumsq(eng, nt, dst, t2, p0, cset=(0, 1, 2)):
        eng.tensor_tensor(dst[:], comp(nt, cset[0], p0, P),
                          comp(nt, cset[0], p0, P), OP.mult)
        for cc in cset[1:]:
            eng.tensor_tensor(t2[:], comp(nt, cc, p0, P),
                              comp(nt, cc, p0, P), OP.mult)
            eng.tensor_tensor(dst[:], dst[:], t2[:], OP.add)

    def psum4(eng, dst, srct):  # [128,48]=(H,4) -> [128,H]
        sv = srct[:].rearrange("p (h x) -> p h x", x=P)
        eng.tensor_tensor(dst, sv[:, :, 0], sv[:, :, 1], OP.add)
        eng.tensor_tensor(dst, dst, sv[:, :, 2], OP.add)
        eng.tensor_tensor(dst, dst, sv[:, :, 3], OP.add)

    kf = [work.tile([128, FEAT * H], F32, name=f"kf{nt}") for nt in range(NT)]
    va = [work.tile([128, OCH * H], BF16, name=f"va{nt}") for nt in range(NT)]
    for nt in range(NT):
        sq = tmp_pool.tile([128, 48], F32, tag="sq", name="sq")
        t2 = tmp_pool.tile([128, 48], F32, tag="t2", name="t2")
        sumsq(nc.gpsimd, nt, sq, t2, P)
        psum4(nc.gpsimd, k2c[nt][:], sq)
        nc.vector.tensor_scalar_mul(k2c[nt][:], k2c[nt][:], -gw0 / P)

        kfv = kf[nt][:].rearrange("p (h f) -> p h f", f=FEAT)
        nc.vector.tensor_copy(kfv[:, :, 0:16],
                              k_sb[nt][:].rearrange("p (h c) -> p h c", c=16))
        for i in range(3):
            dst = kfv[:, :, 16:28].rearrange("p h (x c) -> p h x c", c=3)[:, :, :, i]
            nc.vector.tensor_copy(dst, comp(nt, i, P, P))
            dst = kfv[:, :, 28:40].rearrange("p h (x c) -> p h x c", c=3)[:, :, :, i]
            nc.vector.tensor_copy(dst, comp(nt, 3 + i, P, P))
        nc.vector.tensor_copy(kfv[:, :, 40], k2c[nt][:])
        nc.gpsimd.memset(kfv[:, :, 41], 1.0)

    for t in range(6):
        for nt in range(NT):
            ps = tpsum.tile([128, 128], F32, tag="tps")
            nc.tensor.transpose(ps[:], kf[nt][:, t * 128:(t + 1) * 128], ident[:])
            stg = tmp_pool.tile([128, 128], BF16, tag="kstg", name="kstg", bufs=3)
            if t % 2:
                nc.scalar.copy(stg[:], ps[:])
            else:
                nc.vector.tensor_copy(stg[:], ps[:])
            dst = kf_loc[t * 128 * NB:(t + 1) * 128 * NB].rearrange(
                "(p m) -> p m", m=NB)[:, nt * 128:(nt + 1) * 128]
            nc.sync.dma_start(dst, stg[:])
    nc.gpsimd.collective_compute(
        "AllGather", OP.bypass, replica_groups=GROUPS,
        ins=[kf_loc[:]], outs=[kf_gath[:]])

    # ---- Vall assembly (bf16) + DMA + va collective ----------------------
    for nt in range(NT):
        vav = va[nt][:].rearrange("p (h f) -> p h f", f=OCH)
        nc.vector.tensor_copy(vav[:, :, 0:16],
                              v_sb[nt][:].rearrange("p (h c) -> p h c", c=16))
        for j in range(6):
            dst = vav[:, :, 16:64].rearrange("p h (x c) -> p h x c", c=6)[:, :, :, j]
            nc.vector.tensor_copy(dst, comp(nt, j, 2 * P, V))
        nc.gpsimd.memset(vav[:, :, 64], 1.0)
        nc.gpsimd.memset(vav[:, :, 65], 0.0)
        dstv = va_loc[nt * 128 * H * OCH:(nt + 1) * 128 * H * OCH]
        nc.sync.dma_start(dstv.rearrange("(p m) -> p m", m=H * OCH), va[nt][:])
    nc.gpsimd.collective_compute(
        "AllGather", OP.bypass, replica_groups=GROUPS,
        ins=[va_loc[:]], outs=[va_gath[:]])

    # ---- Q side (overlaps collectives) -----------------------------------
    for nt in range(NT):
        gv = g_sb[nt][:].rearrange("p (h x) -> p h x", x=P)
        for i in range(6):
            nc.vector.tensor_tensor(comp(nt, i, 0, P), comp(nt, i, 0, P), gv, OP.mult)

    q2c = [work.tile([128, H], F32, name=f"q2c{nt}") for nt in range(NT)]
    for nt in range(NT):
        sq = tmp_pool.tile([128, 48], F32, tag="sq", name="sq")
        t2 = tmp_pool.tile([128, 48], F32, tag="t2", name="t2")
        sumsq(nc.gpsimd, nt, sq, t2, 0)
        q2s = tmp_pool.tile([128, H], F32, tag="q2s", name="q2s")
        psum4(nc.gpsimd, q2s[:], sq)

        cr = tmp_pool.tile([128, 48], F32, tag="cr", name="cr")
        cs_ = tmp_pool.tile([128, 48], F32, tag="cs_", name="cs_")
        t3 = tmp_pool.tile([128, 48], F32, tag="t3", name="t3")
        first = True
        for (a, b_) in ((1, 2), (2, 0), (0, 1)):
            nc.gpsimd.tensor_tensor(cr[:], comp(nt, 3 + a, 0, P),
                                    comp(nt, 3 + b_, P, P), OP.mult)
            nc.gpsimd.tensor_tensor(t3[:], comp(nt, 3 + b_, 0, P),
                                    comp(nt, 3 + a, P, P), OP.mult)
            nc.gpsimd.tensor_tensor(cr[:], cr[:], t3[:], OP.subtract)
            nc.gpsimd.tensor_tensor(cr[:], cr[:], cr[:], OP.mult)
            if first:
                nc.gpsimd.tensor_copy(cs_[:], cr[:])
                first = False
            else:
                nc.gpsimd.tensor_tensor(cs_[:], cs_[:], cr[:], OP.add)
        nq2 = tmp_pool.tile([128, 48], F32, tag="nq2", name="nq2")
        nk2 = tmp_pool.tile([128, 48], F32, tag="nk2", name="nk2")
        sumsq(nc.gpsimd, nt, nq2, t2, 0, (3, 4, 5))
        sumsq(nc.gpsimd, nt, nk2, t2, P, (3, 4, 5))
        nc.gpsimd.tensor_tensor(nq2[:], nq2[:], nk2[:], OP.mult)
        nc.scalar.activation(nq2[:], nq2[:], AF.Sqrt)      # |qd||kd|
        nc.vector.tensor_scalar_add(nq2[:], nq2[:], EPS)
        nc.vector.reciprocal(nq2[:], nq2[:])
        nc.scalar.activation(cs_[:], cs_[:], AF.Sqrt)      # |cross|
        nc.gpsimd.tensor_tensor(cs_[:], cs_[:], nq2[:], OP.mult)
        curv = tmp_pool.tile([128, H], F32, tag="curv", name="curv")
        psum4(nc.gpsimd, curv[:], cs_)
        nc.vector.tensor_scalar_mul(q2c[nt][:], q2s[:], -gw0 / P)
        nc.vector.scalar_tensor_tensor(q2c[nt][:], curv[:], -gw1 / P, q2c[nt][:],
                                       OP.mult, OP.add)

    qf = [work.tile([128, FEAT * H], F32, name=f"qf{nt}") for nt in range(NT)]
    for nt in range(NT):
        qfv = qf[nt][:].rearrange("p (h f) -> p h f", f=FEAT)
        nc.vector.tensor_copy(qfv[:, :, 0:16],
                              q_sb[nt][:].rearrange("p (h c) -> p h c", c=16))
        for i in range(3):
            dst = qfv[:, :, 16:28].rearrange("p h (x c) -> p h x c", c=3)[:, :, :, i]
            nc.vector.tensor_copy(dst, comp(nt, i, 0, P))
            dst = qfv[:, :, 28:40].rearrange("p h (x c) -> p h x c", c=3)[:, :, :, i]
            nc.vector.tensor_copy(dst, comp(nt, 3 + i, 0, P))
        nc.gpsimd.memset(qfv[:, :, 40], 1.0)
        nc.vector.tensor_copy(qfv[:, :, 41], q2c[nt][:])
        nc.vector.tensor_tensor(qf[nt][:], qf[nt][:], qscale_sb[:], OP.mult)

    qfT = [work.tile([128, NB], BF16, name=f"qfT{t}") for t in range(6)]
    for t in range(6):
        for nt in range(NT):
            ps = tpsum.tile([128, 128], F32, tag="tps")
            nc.tensor.transpose(ps[:], qf[nt][:, t * 128:(t + 1) * 128], ident[:])
            if t % 2:
                nc.scalar.copy(qfT[t][:, nt * 128:(nt + 1) * 128], ps[:])
            else:
                nc.vector.tensor_copy(qfT[t][:, nt * 128:(nt + 1) * 128], ps[:])

    # ---- gather-in -------------------------------------------------------
    pA_ctx.close()
    attA = ctx.enter_context(tc.tile_pool(name="attA", bufs=1))
    kfG = [attA.tile([128, N], BF16, name=f"kfG{t}") for t in range(6)]
    for t in range(6):
        for kb in range(4):
            srcv = kf_gath[kb, t * 128 * NB:(t + 1) * 128 * NB]
            nc.sync.dma_start(kfG[t][:, kb * NB:(kb + 1) * NB],
                              srcv.rearrange("(p m) -> p m", m=NB))
    vaGb = [attA.tile([128, OCH * H], BF16, name=f"vaGb{kb}") for kb in range(NKB)]
    vaG = [attA.tile([128, OCH * H], F32R, name=f"vaG{kb}") for kb in range(NKB)]
    for kb in range(NKB):
        o0 = (kb % 2) * 128 * H * OCH
        srcv = va_gath[kb // 2, o0:o0 + 128 * H * OCH]
        nc.sync.dma_start(vaGb[kb][:], srcv.rearrange("(p m) -> p m", m=H * OCH))
        nc.vector.tensor_copy(vaG[kb][:], vaGb[kb][:])

    # ---- inverse transform helper (per qt, per 6-head half) --------------
    feats = [work.tile([128, FOUT], F32, name=f"feats{qt}") for qt in range(NT)]

    def emit_inverse(qt, hh):
        hs = slice(hh * 6, hh * 6 + 6)
        ovv = o_all[qt][:].rearrange("p (h f) -> p h f", f=FEAT)[:, hs]

        def og(j):  # [128, 6, V] component j of attention-weighted points
            return ovv[:, :, 16:64].rearrange("p h (x c) -> p h x c", c=6)[:, :, :, j]

        nc.vector.tensor_copy(
            feats[qt][:, hh * 96:hh * 96 + 96].rearrange("p (h c) -> p h c", c=16),
            ovv[:, :, 0:16])
        gview = feats[qt][:, 192:FOUT].rearrange(
            "p (h x c) -> p h x c", h=H, c=7)[:, hs]
        rt, tr = rot_sb[qt], trans_sb[qt]

        ogs = [tmp_pool.tile([128, 48], F32, tag=f"ogs{j}", name=f"ogs{j}")
               for j in range(3)]
        for j in range(3):
            nc.vector.tensor_scalar(
                ogs[j][:].rearrange("p (h x) -> p h x", x=V), og(j),
                tr[:, j:j + 1], None, OP.subtract)
        lc = [tmp_pool.tile([128, 48], F32, tag=f"lc{i}", name=f"lc{i}")
              for i in range(3)]
        ld = [tmp_pool.tile([128, 48], F32, tag=f"ld{i}", name=f"ld{i}")
              for i in range(3)]
        for i in range(3):
            nc.vector.tensor_scalar_mul(lc[i][:], ogs[0][:], rt[:, i:i + 1])
            nc.vector.scalar_tensor_tensor(lc[i][:], ogs[1][:], rt[:, 3 + i:4 + i],
                                           lc[i][:], OP.mult, OP.add)
            nc.vector.scalar_tensor_tensor(lc[i][:], ogs[2][:], rt[:, 6 + i:7 + i],
                                           lc[i][:], OP.mult, OP.add)
            ldv = ld[i][:].rearrange("p (h x) -> p h x", x=V)
            nc.vector.tensor_scalar_mul(ldv, og(3), rt[:, i:i + 1])
            nc.vector.scalar_tensor_tensor(ldv, og(4), rt[:, 3 + i:4 + i],
                                           ldv, OP.mult, OP.add)
            nc.vector.scalar_tensor_tensor(ldv, og(5), rt[:, 6 + i:7 + i],
                                           ldv, OP.mult, OP.add)
        n2 = tmp_pool.tile([128, 48], F32, tag="n2", name="n2")
        t2b = tmp_pool.tile([128, 48], F32, tag="t2b", name="t2b")
        nc.vector.tensor_tensor(n2[:], lc[0][:], lc[0][:], OP.mult)
        for i in (1, 2):
            nc.vector.tensor_tensor(t2b[:], lc[i][:], lc[i][:], OP.mult)
            nc.vector.tensor_tensor(n2[:], n2[:], t2b[:], OP.add)
        nc.scalar.activation(
            gview[:, :, :, 6].rearrange("p h x -> p (h x)"), n2[:], AF.Sqrt)
        for i in range(3):
            nc.vector.tensor_copy(
                gview[:, :, :, i].rearrange("p h x -> p (h x)"), lc[i][:])
        nc.vector.tensor_tensor(n2[:], ld[0][:], ld[0][:], OP.mult)
        for i in (1, 2):
            nc.vector.tensor_tensor(t2b[:], ld[i][:], ld[i][:], OP.mult)
            nc.vector.tensor_tensor(n2[:], n2[:], t2b[:], OP.add)
        nc.scalar.activation(n2[:], n2[:], AF.Sqrt)
        nc.vector.tensor_scalar_max(n2[:], n2[:], EPS)
        nc.vector.reciprocal(n2[:], n2[:])
        for i in range(3):
            nc.vector.tensor_tensor(
                gview[:, :, :, 3 + i].rearrange("p h x -> p (h x)"),
                ld[i][:], n2[:], OP.mult)

    # ---- attention -------------------------------------------------------
    pre_ctx.close()
    att_ctx = ExitStack()
    apsum = att_ctx.enter_context(tc.tile_pool(name="apsum", bufs=2, space=PS))
    opsum = att_ctx.enter_context(tc.tile_pool(name="opsum", bufs=2, space=PS))
    expT_tiles = [work.tile([128, 2048], F32R, name=f"expT{i}") for i in range(5)]
    o_all = [work.tile([128, FEAT * H], F32, name=f"oall{qt}") for qt in range(NT)]
    RUN = 4

    def emit_qk_exp(h):
        t, base = h // 2, (h % 2) * FEAT
        expT = expT_tiles[h % 5]
        for half in range(2):
            aps = apsum.tile([128, 1024], F32, tag="attT", name="aps")
            for kb4 in range(4):
                kb = half * 4 + kb4
                nc.tensor.matmul(
                    aps[:, kb4 * NB:(kb4 + 1) * NB],
                    kfG[t][base:base + FS, kb * 128:(kb + 1) * 128],
                    qfT[t][base:base + FS, :],
                    start=True, stop=True)
            nc.scalar.activation(expT[:, half * 1024:(half + 1) * 1024], aps[:],
                                 AF.Exp)

    def emit_av(h):
        expT = expT_tiles[h % 5]
        ot_ps = opsum.tile([OCH, NB], F32, tag="otacc", name="ot_ps")
        for kb in range(NKB):
            nc.tensor.matmul(
                ot_ps[:],
                vaG[kb][:, h * OCH:(h + 1) * OCH],
                expT[:, kb * NB:(kb + 1) * NB],
                start=(kb == 0), stop=(kb == NKB - 1))
        ot_sb = tmp_pool.tile([OCH, NB], F32R, tag="otsb", name="otsb", bufs=2)
        nc.scalar.copy(ot_sb[:], ot_ps[:])
        for qt in range(NT):
            tp = opsum.tile([128, OCH], F32R, tag="otp", name="tp")
            nc.tensor.transpose(tp[:], ot_sb[:, qt * 128:(qt + 1) * 128],
                                ident_r[0:OCH, 0:OCH])
            rec = tmp_pool.tile([128, 1], F32, tag="rec", name="rec")
            nc.vector.reciprocal(rec[:], tp[:, 64:65].bitcast(F32))
            nc.vector.tensor_scalar_mul(
                o_all[qt][:, h * FEAT:h * FEAT + 64], tp[:, 0:64].bitcast(F32),
                rec[:])

    for h in range(H + RUN):
        if h < H:
            emit_qk_exp(h)
        if h >= RUN:
            emit_av(h - RUN)
            if h - RUN == 5:
                for qt in range(NT):
                    emit_inverse(qt, 0)
    for qt in range(NT):
        emit_inverse(qt, 1)

    # ---- inverse transform + feats (emitted per head-half above) ---------
    # ---- output projection ----------------------------------------------
    att_ctx.close()
    tpsum2 = ctx.enter_context(tc.tile_pool(name="tpsum2", bufs=2, space=PS))
    opsum2 = ctx.enter_context(tc.tile_pool(name="opsum2", bufs=2, space=PS))
    fT = []
    for kc in range(KCH):
        r0 = kc * 128
        rw = min(FOUT, r0 + 128) - r0          # 128 or 96
        pw = rw + 2 if kc == KCH - 1 else rw   # +2 ones rows on last chunk
        t = work.tile([pw, NB], F32R, name=f"fT{kc}")
        fT.append(t)
    lastr = FOUT - (KCH - 1) * 128
    nc.vector.tensor_copy(fT[KCH - 1][lastr:lastr + 2, :], ones2_f32[:])
    for kc in range(KCH):
        r0 = kc * 128
        rw = min(FOUT, r0 + 128) - r0
        for qt in range(NT):
            ps = tpsum2.tile([128, 128], F32, tag="tps2")
            nc.tensor.transpose(ps[:rw, :], feats[qt][:, r0:r0 + rw], ident[:])
            if kc % 2:
                nc.scalar.copy(fT[kc][:rw, qt * 128:(qt + 1) * 128], ps[:rw, :])
            else:
                nc.vector.tensor_copy(fT[kc][:rw, qt * 128:(qt + 1) * 128],
                                      ps[:rw, :])

    for qt in range(NT):
        ps = opsum2.tile([128, CS], F32, tag="oproj")
        for kc in range(KCH):
            nc.tensor.matmul(ps[:], fT[kc][:, qt * 128:(qt + 1) * 128],
                             wout_sb[kc][:], start=(kc == 0), stop=(kc == KCH - 1))
        osb = tmp_pool.tile([128, CS], F32, tag="osb", name="osb")
        nc.scalar.copy(osb[:], ps[:])
        nc.sync.dma_start(out_loc[qt * 128:(qt + 1) * 128, :], osb[:])


def _run(inputs, trace=False):
    s, rot9, trans, wall, wout_b, qscale, gw = _host_prep(inputs)
    nc = _build_program(float(gw[0]), float(gw[1]))
    in_maps = []
    for c in range(8):
        b, qb = c // 4, c % 4
        r = slice(qb * NB, (qb + 1) * NB)
        in_maps.append({
            "s_loc": np.ascontiguousarray(s[b, r]),
            "rot_loc": np.ascontiguousarray(rot9[b, r]),
            "trans_loc": np.ascontiguousarray(trans[b, r]),
            "wall": wall, "wout_b": wout_b, "qscale": qscale,
        })
    res = run_bass_kernel_spmd(nc, in_maps, list(range(8)), trace=trace)
    out = np.empty((B, N, CS), np.float32)
    for c in range(8):
        b, qb = c // 4, c % 4
        out[b, qb * NB:(qb + 1) * NB] = res.results[c]["out_loc"]
    return out, res


def kernel(**inputs):
    out, _ = _run(inputs, trace=False)
    return out


def kernel_traced(**inputs):
    return _run(inputs, trace=True)



# revision 12
# speedup vs baseline: 1.9607x; 1.9607x over previous
"""Bass/Tile TRN2 kernel for EnhancedIPA3 — collective-free redesign.

8 cores = batch(2) x query-block(4).  Each core redundantly computes the
K/V-side features for ALL 1024 keys of its batch (projections + rigid
frame transforms), then runs attention for its own 256 queries only.  No
inter-core communication: the collective-bootstrap barrier and the two
serialized AllGathers of the previous design are gone, and the cores are
fully independent, so cross-core launch skew no longer costs anything.

Key rows are rotated per core so the core's own query rows are always
tiles 0..1 (softmax over keys is permutation invariant) — one SPMD
program serves all 8 cores.

Self-contained: hardcodes all shapes; only depends on numpy + concourse.
"""

import numpy as np
from contextlib import ExitStack

import concourse.bass as bass
import concourse.bacc as bacc
import concourse.mybir as mybir
import concourse.tile as tile
from concourse.bass_utils import run_bass_kernel_spmd
from concourse.masks import make_identity

F32 = mybir.dt.float32
F32R = mybir.dt.float32r
BF16 = mybir.dt.bfloat16
AF = mybir.ActivationFunctionType
OP = mybir.AluOpType
AX = mybir.AxisListType

B, N, CS, H, C, P, V = 2, 1024, 384, 12, 16, 4, 8
EPS = 1e-8
NB = 256               # query rows per core (2 tiles)
NKT = 8                # key tiles of 128
# wall column map
K_OFF, V_OFF, PTS_OFF, Q_OFF, G_OFF, QPTS_OFF = 0, 192, 384, 1248, 1440, 1488
WALL_COLS = 1776
NPK = 12               # kv points per head (0:4 k_pts, 4:12 v_pts)
FEAT = 64              # per-head feature stride in kf/qf
FS = 42                # live features per head
OCH = 68               # va per-head stride: v16 | pts48 | ones | pad3
FOUT = H * (C + 7 * V)  # 864
KCH = 7                # output-proj contraction chunks


def _host_prep(inputs):
    wq = np.asarray(inputs["wq"], np.float32)
    bq = np.asarray(inputs["bq"], np.float32)
    wkv = np.asarray(inputs["wkv"], np.float32)
    bkv = np.asarray(inputs["bkv"], np.float32)
    wqp = np.asarray(inputs["wqp"], np.float32)
    bqp = np.asarray(inputs["bqp"], np.float32)
    wkvp = np.asarray(inputs["wkvp"], np.float32)
    bkvp = np.asarray(inputs["bkvp"], np.float32)
    wg = np.asarray(inputs["wg"], np.float32)
    bg = np.asarray(inputs["bg"], np.float32)
    gw = np.asarray(inputs["geom_weight"], np.float32)
    hw = np.asarray(inputs["head_weights"], np.float32)
    sh = 1.0 / (1.0 + np.exp(-hw))
    gw0, gw1 = float(gw[0]), float(gw[1])

    wall = np.zeros((CS + 2, WALL_COLS), np.float32)
    wall[:CS, K_OFF:K_OFF + 192] = wkv[:, :192]
    wall[CS, K_OFF:K_OFF + 192] = bkv[:192]
    wall[:CS, V_OFF:V_OFF + 192] = wkv[:, 192:]
    wall[CS, V_OFF:V_OFF + 192] = bkv[192:]
    # kv pts planar: dst col = PTS_OFF + cc*144 + h*12 + p  <-  src h*72 + p*6 + cc
    cc, h, p = np.meshgrid(np.arange(6), np.arange(H), np.arange(12),
                           indexing="ij")
    dst = (PTS_OFF + cc * 144 + h * 12 + p).ravel()
    src = (h * 72 + p * 6 + cc).ravel()
    wall[:CS, dst] = wkvp[:, src]
    wall[CS, dst] = bkvp[src]
    # q scaled by sh/sqrt(C)
    qs = np.repeat(sh / np.sqrt(C), 16)
    wall[:CS, Q_OFF:Q_OFF + 192] = wq * qs[None, :]
    wall[CS, Q_OFF:Q_OFF + 192] = bq * qs
    wall[:CS, G_OFF:G_OFF + 48] = wg
    wall[CS, G_OFF:G_OFF + 48] = bg
    # q pts planar: dst col = QPTS_OFF + cc*48 + h*4 + p  <-  src h*24 + p*6 + cc
    cc, h, p = np.meshgrid(np.arange(6), np.arange(H), np.arange(P),
                           indexing="ij")
    dst = (QPTS_OFF + cc * 48 + h * 4 + p).ravel()
    src = (h * 24 + p * 6 + cc).ravel()
    wall[:CS, dst] = wqp[:, src]
    wall[CS, dst] = bqp[src]
    has_bias = bool(np.any(wall[CS] != 0.0))
    wall[CS + 1] = wall[CS] * 0.5
    wall[CS] = wall[CS + 1]

    bout_half = np.asarray(inputs["bout"], np.float32)[None, :] * 0.5
    wout_b = np.concatenate(
        [np.asarray(inputs["wout"], np.float32), bout_half, bout_half], axis=0)

    # on-chip constants (broadcast to 128 partitions by a rank-1 matmul)
    qconst = np.zeros((1, 144), np.float32)
    SC = gw0 * 0.5 * sh            # coord feature scale (with gate)
    DC = gw1 * sh                  # dir feature scale (with gate)
    qconst[0, 0:48] = np.repeat(SC, P)
    qconst[0, 48:96] = np.repeat(DC, P)
    qconst[0, 96:108] = sh * gw0 / P                      # qf[40]
    c2 = np.where(np.abs(gw0 * sh) > 1e-12, -1.0 / (gw0 * sh + 1e-30), 0.0)
    qconst[0, 108:120] = c2                               # q2 coefficient
    qconst[0, 120:132] = -sh * gw1                        # curvature coeff

    rot9 = np.ascontiguousarray(
        np.asarray(inputs["rot"], np.float32).reshape(B, N, 9))
    trans = np.asarray(inputs["trans"], np.float32)
    s = np.asarray(inputs["s"], np.float32)
    return s, rot9, trans, wall, wout_b, qconst, has_bias


_PROGRAM_CACHE = {}


def _build_program(has_bias):
    key = (bool(has_bias),)
    if key in _PROGRAM_CACHE:
        return _PROGRAM_CACHE[key]
    nc = bacc.Bacc("TRN2", target_bir_lowering=False, debug=False,
                   num_devices=8)
    s_all = nc.dram_tensor("s_all", [N, CS], F32, kind="ExternalInput")
    rot_all = nc.dram_tensor("rot_all", [N, 9], F32, kind="ExternalInput")
    trans_all = nc.dram_tensor("trans_all", [N, 3], F32, kind="ExternalInput")
    wall_d = nc.dram_tensor("wall", [CS + 2, WALL_COLS], F32,
                            kind="ExternalInput")
    wout_d = nc.dram_tensor("wout_b", [FOUT + 2, CS], F32,
                            kind="ExternalInput")
    qconst_d = nc.dram_tensor("qconst", [1, 144], F32, kind="ExternalInput")
    out_loc = nc.dram_tensor("out_loc", [NB, CS], F32, kind="ExternalOutput")

    with tile.TileContext(nc) as tc:
        with ExitStack() as ctx:
            _emit(ctx, tc, nc, s_all, rot_all, trans_all, wall_d, wout_d,
                  qconst_d, out_loc, has_bias)
    nc.compile()
    _PROGRAM_CACHE[key] = nc
    return nc


def _emit(ctx, tc, nc, s_all, rot_all, trans_all, wall_d, wout_d, qconst_d,
          out_loc, has_bias):
    PS = bass.MemorySpace.PSUM

    const = ctx.enter_context(tc.tile_pool(name="const", bufs=1))
    work = ctx.enter_context(tc.tile_pool(name="work", bufs=1))
    tmp = ctx.enter_context(tc.tile_pool(name="tmp", bufs=2))
    pre_ctx = ExitStack()
    pA = pre_ctx.enter_context(tc.tile_pool(name="pA", bufs=1))
    kio = pre_ctx.enter_context(tc.tile_pool(name="kio", bufs=2))
    ppsum = pre_ctx.enter_context(tc.tile_pool(name="ppsum", bufs=2, space=PS))
    tpsum = pre_ctx.enter_context(tc.tile_pool(name="tpsum", bufs=2, space=PS))

    # ---- constants -------------------------------------------------------
    ident = const.tile([128, 128], F32)
    make_identity(nc, ident[:])
    ident_r = const.tile([128, 128], F32R)
    nc.vector.tensor_copy(ident_r[:], ident[:])
    ones2_f32 = const.tile([2, NB], F32)
    nc.gpsimd.memset(ones2_f32[:], 1.0)
    ones_row = const.tile([2, NB], F32R)
    nc.vector.tensor_copy(ones_row[:], ones2_f32[:])

    # ---- DMAs ------------------------------------------------------------
    wall_sb = []
    for kc in range(3):
        t = pA.tile([128, WALL_COLS], F32R, name=f"wall{kc}")
        nc.sync.dma_start(t[:], wall_d[kc * 128:(kc + 1) * 128, :].bitcast(F32R))
        wall_sb.append(t)
    wall_bias = pA.tile([2, WALL_COLS], F32R)
    if has_bias:
        nc.sync.dma_start(wall_bias[:], wall_d[CS:CS + 2, :].bitcast(F32R))

    wout_sb = []
    for kc in range(KCH):
        r0 = kc * 128
        r1 = min(FOUT + 2, r0 + 128)
        t = const.tile([r1 - r0, CS], F32R, name=f"wout{kc}")
        nc.sync.dma_start(t[:], wout_d[r0:r1, :].bitcast(F32R))
        wout_sb.append(t)

    qconst_sb = const.tile([1, 144], F32R)
    nc.sync.dma_start(qconst_sb[:], qconst_d[:, :].bitcast(F32R))

    rot_sb, trans_sb = [], []
    for kt in range(NKT):
        r = slice(kt * 128, (kt + 1) * 128)
        t = const.tile([128, 9], F32, name=f"rot{kt}")
        nc.sync.dma_start(t[:], rot_all[r, :])
        rot_sb.append(t)
        t = const.tile([128, 3], F32, name=f"trans{kt}")
        nc.sync.dma_start(t[:], trans_all[r, :])
        trans_sb.append(t)

    s_sb = []
    for kt in range(NKT):
        t = kio.tile([128, CS], F32, tag="s", name=f"s{kt}")
        nc.sync.dma_start(t[:], s_all[kt * 128:(kt + 1) * 128, :])
        s_sb.append(t)

    # ---- sT (transpose all of s) ----------------------------------------
    sT = pA.tile([128, 3 * N], F32R, name="sT")   # [:, kc*1024 + key]
    sT3 = sT[:].rearrange("p (c k) -> p c k", k=N)
    for kt in range(NKT):
        tps = tpsum.tile([128, 384], F32, tag="tps")
        for kc in range(3):
            nc.tensor.transpose(tps[:, kc * 128:(kc + 1) * 128],
                                s_sb[kt][:, kc * 128:(kc + 1) * 128], ident[:])
        dst = sT3[:, :, kt * 128:(kt + 1) * 128]
        src = tps[:].rearrange("p (c k) -> p c k", k=128)
        if kt % 2:
            nc.scalar.copy(dst, src)
        else:
            nc.vector.tensor_copy(dst, src)

    # ---- broadcast qconst row to 128 partitions --------------------------
    tps = tpsum.tile([128, 384], F32, tag="tps")
    nc.tensor.matmul(tps[:, 0:144], ones_row[0:1, 0:128], qconst_sb[:, :],
                     start=True, stop=True)
    qcst = const.tile([128, 144], F32)
    nc.vector.tensor_copy(qcst[:], tps[:, 0:144])
    # slices: SC48 0:48 | DC48 48:96 | A12 96:108 | c2 108:120 | c3 120:132

    # ---- K/V side: all 8 key tiles --------------------------------------
    kfT = work.tile([128, 6 * N], BF16, name="kfT")   # [:, t*1024 + key]
    kfT3 = kfT[:].rearrange("p (t k) -> p t k", k=N)
    nc.gpsimd.memset(kfT[:], 0.0)
    vaG = [work.tile([128, H * OCH], BF16, name=f"vaG{kb}")
           for kb in range(NKT)]
    kds = [work.tile([128, 144], F32, name=f"kds{qt}") for qt in range(2)]

    GROUPS_K = [(0, 384), (384, 896), (896, 1248)]

    def proj_mm(ps, c0, c1, kt):
        pv = ps[:, 0:c1 - c0]
        for kc in range(3):
            last = (kc == 2) and not has_bias
            nc.tensor.matmul(pv, sT3[:, kc, kt * 128:(kt + 1) * 128],
                             wall_sb[kc][:, c0:c1], start=(kc == 0), stop=last)
        if has_bias:
            nc.tensor.matmul(pv, ones_row[:, 0:128], wall_bias[:, c0:c1],
                             start=False, stop=True)

    def transform(pts, pco, rt, tr, W, coords=True, dirs=True):
        """pts/pco: planar [128, 6*W] (bf16 in, bf16/f32 out)."""
        for i in range(3):
            if coords:
                dco = pco[:, i * W:(i + 1) * W]
                nc.scalar.activation(dco, pts[:, 0:W], AF.Identity,
                                     bias=tr[:, i:i + 1],
                                     scale=rt[:, 3 * i:3 * i + 1])
                nc.vector.scalar_tensor_tensor(dco, pts[:, W:2 * W],
                                               rt[:, 3 * i + 1:3 * i + 2], dco,
                                               OP.mult, OP.add)
                nc.vector.scalar_tensor_tensor(dco, pts[:, 2 * W:3 * W],
                                               rt[:, 3 * i + 2:3 * i + 3], dco,
                                               OP.mult, OP.add)
            if dirs:
                ddi = pco[:, (3 + i) * W:(4 + i) * W]
                nc.scalar.activation(ddi, pts[:, 3 * W:4 * W], AF.Copy,
                                     scale=rt[:, 3 * i:3 * i + 1])
                nc.vector.scalar_tensor_tensor(ddi, pts[:, 4 * W:5 * W],
                                               rt[:, 3 * i + 1:3 * i + 2], ddi,
                                               OP.mult, OP.add)
                nc.vector.scalar_tensor_tensor(ddi, pts[:, 5 * W:6 * W],
                                               rt[:, 3 * i + 2:3 * i + 3], ddi,
                                               OP.mult, OP.add)

    for kt in range(NKT):
        # projections: K+V | pts-a | pts-b
        ps_kv = ppsum.tile([128, 384], F32, tag="pg384", name="pskv")
        proj_mm(ps_kv, 0, 384, kt)
        ps_p1 = ppsum.tile([128, 512], F32, tag="pg512", name="psp1")
        proj_mm(ps_p1, 384, 896, kt)
        ps_p2 = ppsum.tile([128, 352], F32, tag="pg352", name="psp2")
        proj_mm(ps_p2, 896, 1248, kt)

        kf = kio.tile([128, H * FEAT], F32, tag="kf", name="kf")
        if kt < 2:
            # zero the pad cols 42:64 of this physical buffer once
            nc.gpsimd.memset(kf[:], 0.0)
        kfv = kf[:].rearrange("p (h f) -> p h f", f=FEAT)
        va = vaG[kt]
        vav = va[:].rearrange("p (h f) -> p h f", f=OCH)
        pts = kio.tile([128, 864], BF16, tag="pts", name="pts")

        # evacuations
        nc.scalar.copy(kfv[:, :, 0:16],
                       ps_kv[:, 0:192].rearrange("p (h c) -> p h c", c=16))
        nc.vector.tensor_copy(vav[:, :, 0:16],
                              ps_kv[:, 192:384].rearrange("p (h c) -> p h c", c=16))
        nc.vector.tensor_scalar_max(pts[:, 0:512], ps_p1[:], 0.0)
        nc.scalar.activation(pts[:, 512:864], ps_p2[:], AF.Relu)

        # rigid transform (planar, bf16)
        pco = kio.tile([128, 864], BF16, tag="pco", name="pco")
        transform(pts[:], pco[:], rot_sb[kt], trans_sb[kt], 144)
        pco3 = pco[:].rearrange("p (c h x) -> p c h x", c=6, x=NPK)

        # kf coord/dir features ([cc*4+p] per head) + va pts
        for cc in range(3):
            nc.gpsimd.tensor_copy(kfv[:, :, 16 + cc * 4:20 + cc * 4],
                                  pco3[:, cc, :, 0:4])
            nc.gpsimd.tensor_copy(kfv[:, :, 28 + cc * 4:32 + cc * 4],
                                  pco3[:, 3 + cc, :, 0:4])
        for cc in range(6):
            eng = nc.vector if cc % 2 else nc.gpsimd
            eng.tensor_copy(vav[:, :, 16 + cc * 8:24 + cc * 8],
                            pco3[:, cc, :, 4:12])
        nc.gpsimd.memset(vav[:, :, 64:65], 1.0)
        nc.gpsimd.memset(vav[:, :, 65:68], 0.0)

        # k2 (negated sum of squared coord features)
        sqs = tmp.tile([128, 144], F32, tag="sqs", name="sqs")
        nc.vector.tensor_tensor(
            sqs[:].rearrange("p (h x) -> p h x", x=12),
            kfv[:, :, 16:28], kfv[:, :, 16:28], OP.mult)
        k2 = tmp.tile([128, 12], F32, tag="k2", name="k2")
        nc.vector.tensor_reduce(
            k2[:], sqs[:].rearrange("p (h c x) -> p h c x", c=3, x=4),
            AX.XY, OP.add, negate=True)
        nc.vector.tensor_copy(kfv[:, :, 40], k2[:])
        nc.gpsimd.memset(kfv[:, :, 41], 1.0)
        if kt < 2:
            nc.gpsimd.tensor_copy(kds[kt][:].rearrange("p (h x) -> p h x", x=12),
                                  kfv[:, :, 28:40])

        # transpose kf -> kfT (2 head-pairs per psum tile)
        for t0 in range(0, 6, 2):
            tps = tpsum.tile([128, 384], F32, tag="tps")
            nc.tensor.transpose(tps[:, 0:128],
                                kf[:, t0 * 128:(t0 + 1) * 128], ident[:])
            nc.tensor.transpose(tps[:, 128:256],
                                kf[:, (t0 + 1) * 128:(t0 + 2) * 128], ident[:])
            dst = kfT3[:, t0:t0 + 2, kt * 128:(kt + 1) * 128]
            src = tps[:, 0:256].rearrange("p (t k) -> p t k", k=128)
            if t0 == 2:
                nc.scalar.copy(dst, src)
            else:
                nc.vector.tensor_copy(dst, src)

    # ---- Q side (own rows = tiles 0..1) ---------------------------------
    qf_sb = [work.tile([128, H * FEAT], F32, name=f"qf{qt}") for qt in range(2)]
    for qt in range(2):
        qf = qf_sb[qt]
        qfv = qf[:].rearrange("p (h f) -> p h f", f=FEAT)
        ps_a = ppsum.tile([128, 384], F32, tag="pg384", name="psqa")
        proj_mm(ps_a, Q_OFF, Q_OFF + 384, qt)
        ps_b = ppsum.tile([128, 352], F32, tag="pg352", name="psqb")
        proj_mm(ps_b, Q_OFF + 384, WALL_COLS, qt)

        nc.scalar.copy(qfv[:, :, 0:16],
                       ps_a[:, 0:192].rearrange("p (h c) -> p h c", c=16))
        g_sb = tmp.tile([128, 48], F32, tag="gsb", name="gsb")
        nc.scalar.activation(g_sb[:], ps_a[:, 192:240], AF.Sigmoid)
        qpts = tmp.tile([128, 288], BF16, tag="qpts", name="qpts")
        nc.vector.tensor_scalar_max(qpts[:, 0:144], ps_a[:, 240:384], 0.0)
        nc.vector.tensor_scalar_max(qpts[:, 144:288], ps_b[:, 0:144], 0.0)

        qpco = tmp.tile([128, 288], F32, tag="qpco", name="qpco")
        transform(qpts[:], qpco[:], rot_sb[qt], trans_sb[qt], 48)
        qpco3 = qpco[:].rearrange("p (c h x) -> p c h x", c=6, x=4)

        gc = tmp.tile([128, 48], F32, tag="gc", name="gc")
        gd = tmp.tile([128, 48], F32, tag="gd", name="gd")
        nc.vector.tensor_tensor(gc[:], g_sb[:], qcst[:, 0:48], OP.mult)
        nc.vector.tensor_tensor(gd[:], g_sb[:], qcst[:, 48:96], OP.mult)
        gc3 = gc[:].rearrange("p (h x) -> p h x", x=4)
        gd3 = gd[:].rearrange("p (h x) -> p h x", x=4)
        for cc in range(3):
            nc.vector.tensor_tensor(qfv[:, :, 16 + cc * 4:20 + cc * 4],
                                    qpco3[:, cc], gc3, OP.mult)
            nc.gpsimd.tensor_tensor(qfv[:, :, 28 + cc * 4:32 + cc * 4],
                                    qpco3[:, 3 + cc], gd3, OP.mult)
        nc.vector.tensor_copy(qfv[:, :, 40], qcst[:, 96:108])

        # q2 from coord features
        sqs = tmp.tile([128, 144], F32, tag="sqs", name="sqs")
        nc.vector.tensor_tensor(
            sqs[:].rearrange("p (h x) -> p h x", x=12),
            qfv[:, :, 16:28], qfv[:, :, 16:28], OP.mult)
        q2s = tmp.tile([128, 12], F32, tag="q2s", name="q2s")
        nc.vector.tensor_reduce(
            q2s[:], sqs[:].rearrange("p (h c x) -> p h c x", c=3, x=4),
            AX.XY, OP.add)

        # curvature from dir features vs raw kd features of same rows
        qdv = qfv[:, :, 28:40]
        kdv = kds[qt][:].rearrange("p (h x) -> p h x", x=12)
        crs = tmp.tile([128, 144], F32, tag="crs", name="crs")
        t1 = tmp.tile([128, 48], F32, tag="t1", name="t1")
        t2 = tmp.tile([128, 48], F32, tag="t2", name="t2")
        t13 = t1[:].rearrange("p (h x) -> p h x", x=4)
        t23 = t2[:].rearrange("p (h x) -> p h x", x=4)
        for c, (a, b2) in enumerate(((1, 2), (2, 0), (0, 1))):
            nc.vector.tensor_tensor(t13, qdv[:, :, a * 4:a * 4 + 4],
                                    kdv[:, :, b2 * 4:b2 * 4 + 4], OP.mult)
            nc.gpsimd.tensor_tensor(t23, qdv[:, :, b2 * 4:b2 * 4 + 4],
                                    kdv[:, :, a * 4:a * 4 + 4], OP.mult)
            nc.vector.tensor_tensor(crs[:, c * 48:(c + 1) * 48], t1[:], t2[:],
                                    OP.subtract)
        nc.vector.tensor_tensor(crs[:], crs[:], crs[:], OP.mult)
        csum = tmp.tile([128, 48], F32, tag="csum", name="csum")
        nc.vector.tensor_reduce(
            csum[:], crs[:].rearrange("p (c x) -> p x c", c=3), AX.X, OP.add)
        # |qfd|^2, |kd|^2 per (h,p)
        sqd = tmp.tile([128, 144], F32, tag="sqd", name="sqd")
        nq2 = tmp.tile([128, 48], F32, tag="nq2", name="nq2")
        nk2 = tmp.tile([128, 48], F32, tag="nk2", name="nk2")
        nc.gpsimd.tensor_tensor(sqd[:].rearrange("p (h x) -> p h x", x=12),
                                qdv, qdv, OP.mult)
        nc.vector.tensor_reduce(
            nq2[:].rearrange("p (h x) -> p h x", x=4),
            sqd[:].rearrange("p (h c x) -> p h x c", c=3, x=4), AX.X, OP.add)
        nc.gpsimd.tensor_tensor(sqd[:].rearrange("p (h x) -> p h x", x=12),
                                kdv, kdv, OP.mult)
        nc.vector.tensor_reduce(
            nk2[:].rearrange("p (h x) -> p h x", x=4),
            sqd[:].rearrange("p (h c x) -> p h x c", c=3, x=4), AX.X, OP.add)
        nc.vector.tensor_tensor(nq2[:], nq2[:], nk2[:], OP.mult)
        nc.scalar.activation(nq2[:], nq2[:], AF.Sqrt)
        nc.vector.tensor_scalar_add(nq2[:], nq2[:], EPS)
        nc.vector.reciprocal(nq2[:], nq2[:])
        nc.scalar.activation(csum[:], csum[:], AF.Sqrt)
        nc.vector.tensor_tensor(csum[:], csum[:], nq2[:], OP.mult)
        curv = tmp.tile([128, 12], F32, tag="curv", name="curv")
        nc.vector.tensor_reduce(
            curv[:], csum[:].rearrange("p (h x) -> p h x", x=4), AX.X, OP.add)
        # qf[41] = c2*q2s + c3*curv
        nc.vector.tensor_tensor(q2s[:], q2s[:], qcst[:, 108:120], OP.mult)
        nc.vector.tensor_tensor(curv[:], curv[:], qcst[:, 120:132], OP.mult)
        nc.vector.tensor_tensor(qfv[:, :, 41], q2s[:], curv[:], OP.add)

    # ---- qfT: masked transposes (even head | odd head halves) -----------
    qfT = [work.tile([128, 2 * NB], BF16, name=f"qfT{t}") for t in range(6)]
    for t in range(6):
        nc.gpsimd.memset(qfT[t][:], 0.0)
    for t in range(6):
        for qt in range(2):
            tps = tpsum.tile([128, 384], F32, tag="tps")
            nc.tensor.transpose(tps[:, 0:128],
                                qf_sb[qt][:, t * 128:(t + 1) * 128], ident[:])
            eng = nc.scalar if (t + qt) % 2 else nc.vector
            eng_copy = eng.copy if eng is nc.scalar else eng.tensor_copy
            eng_copy(qfT[t][0:FS, qt * 128:(qt + 1) * 128], tps[0:FS, 0:128])
            eng2 = nc.vector if (t + qt) % 2 else nc.scalar
            eng2_copy = eng2.copy if eng2 is nc.scalar else eng2.tensor_copy
            eng2_copy(qfT[t][64:64 + FS, NB + qt * 128:NB + (qt + 1) * 128],
                      tps[64:64 + FS, 0:128])

    # ---- attention -------------------------------------------------------
    pre_ctx.close()
    att_ctx = ExitStack()
    apsum = att_ctx.enter_context(tc.tile_pool(name="apsum", bufs=2, space=PS))
    opsum = att_ctx.enter_context(tc.tile_pool(name="opsum", bufs=2, space=PS))
    otp = att_ctx.enter_context(tc.tile_pool(name="otp", bufs=2, space=PS))
    expT_tiles = [work.tile([128, 4096], BF16, name=f"expT{i}")
                  for i in range(3)]
    o_all = [work.tile([128, FEAT * H], F32, name=f"oall{qt}")
             for qt in range(2)]
    feats = [work.tile([128, FOUT], F32, name=f"feats{qt}") for qt in range(2)]
    ld_sb = [work.tile([128, 288], F32, name=f"ld{qt}") for qt in range(2)]
    RUNP = 2

    def emit_qk_exp(t):
        expT = expT_tiles[t % 3]
        for p4 in range(4):
            aps = apsum.tile([128, 1024], F32, tag="aps", name="aps")
            for j in range(2):
                kb = p4 * 2 + j
                nc.tensor.matmul(aps[:, j * 512:(j + 1) * 512],
                                 kfT3[:, t, kb * 128:(kb + 1) * 128],
                                 qfT[t][:, :], start=True, stop=True)
            nc.scalar.activation(expT[:, p4 * 1024:(p4 + 1) * 1024], aps[:],
                                 AF.Exp)

    def emit_av(h):
        t, e = h // 2, h % 2
        expT = expT_tiles[t % 3]
        ot_ps = opsum.tile([OCH, NB], F32, tag="ot", name="ot_ps")
        for kb in range(NKT):
            nc.tensor.matmul(
                ot_ps[:], vaG[kb][:, h * OCH:(h + 1) * OCH],
                expT[:, kb * 512 + e * NB:kb * 512 + (e + 1) * NB],
                start=(kb == 0), stop=(kb == NKT - 1))
        ot_sb = tmp.tile([OCH, NB], F32R, tag="otsb", name="otsb", bufs=2)
        nc.vector.tensor_copy(ot_sb[:], ot_ps[:])
        for qt in range(2):
            tp = otp.tile([128, OCH], F32R, tag="tp", name="tp")
            nc.tensor.transpose(tp[:], ot_sb[:, qt * 128:(qt + 1) * 128],
                                ident_r[0:OCH, 0:OCH])
            rec = tmp.tile([128, 1], F32, tag="rec", name="rec", bufs=2)
            nc.vector.reciprocal(rec[:], tp[:, 64:65].bitcast(F32))
            nc.vector.tensor_scalar_mul(
                o_all[qt][:, h * FEAT:h * FEAT + 64], tp[:, 0:64].bitcast(F32),
                rec[:])

    def emit_inv_rot(qt, hh):
        """Rotate o_geom back to local frame for heads hh*6..hh*6+5."""
        rt, tr = rot_sb[qt], trans_sb[qt]
        hs = slice(hh * 6, hh * 6 + 6)
        ov = o_all[qt][:].rearrange("p (h f) -> p h f", f=FEAT)[:, hs]
        gv = feats[qt][:, 192:FOUT].rearrange(
            "p (h x c) -> p h x c", h=H, c=7)[:, hs]

        def og(j):
            return ov[:, :, 16 + 8 * j:24 + 8 * j]

        ogs = tmp.tile([128, 144], F32, tag="ogs", name="ogs", bufs=2)
        ogs3 = ogs[:].rearrange("p (c x) -> p c x", c=3)
        for j in range(3):
            nc.vector.tensor_scalar(
                ogs3[:, j].rearrange("p (h x) -> p h x", x=V), og(j),
                tr[:, j:j + 1], None, OP.subtract)
        lci = tmp.tile([128, 48], F32, tag="lci", name="lci", bufs=2)
        for i in range(3):
            nc.vector.tensor_scalar_mul(lci[:], ogs3[:, 0], rt[:, i:i + 1])
            nc.vector.scalar_tensor_tensor(lci[:], ogs3[:, 1],
                                           rt[:, 3 + i:4 + i], lci[:],
                                           OP.mult, OP.add)
            nc.vector.scalar_tensor_tensor(
                gv[:, :, :, i], ogs3[:, 2].rearrange("p (h x) -> p h x", x=V),
                rt[:, 6 + i:7 + i],
                lci[:].rearrange("p (h x) -> p h x", x=V), OP.mult, OP.add)
            ldd = ld_sb[qt][:, i * 96 + hh * 48:i * 96 + (hh + 1) * 48]
            ldd3 = ldd.rearrange("p (h x) -> p h x", x=V)
            nc.vector.tensor_scalar_mul(ldd3, og(3), rt[:, i:i + 1])
            nc.vector.scalar_tensor_tensor(ldd3, og(4), rt[:, 3 + i:4 + i],
                                           ldd3, OP.mult, OP.add)
            nc.vector.scalar_tensor_tensor(ldd3, og(5), rt[:, 6 + i:7 + i],
                                           ldd3, OP.mult, OP.add)

    def emit_inv_norm(qt):
        gv = feats[qt][:, 192:FOUT].rearrange("p (h x c) -> p h x c", h=H, c=7)
        lsq = tmp.tile([128, 288], F32, tag="lsq", name="lsq")
        lsq4 = lsq[:].rearrange("p (h x c) -> p h x c", c=3, x=V)
        nc.vector.tensor_tensor(lsq4, gv[:, :, :, 0:3], gv[:, :, :, 0:3],
                                OP.mult)
        ncs = tmp.tile([128, 96], F32, tag="ncs", name="ncs")
        nc.vector.tensor_reduce(
            ncs[:], lsq[:].rearrange("p (x c) -> p x c", c=3), AX.X, OP.add)
        nc.scalar.activation(gv[:, :, :, 6],
                             ncs[:].rearrange("p (h x) -> p h x", x=V), AF.Sqrt)
        # ld normalization
        ldq = ld_sb[qt]
        nc.gpsimd.tensor_tensor(lsq[:], ldq[:], ldq[:], OP.mult)
        nds = tmp.tile([128, 96], F32, tag="nds", name="nds")
        nc.vector.tensor_reduce(
            nds[:], lsq[:].rearrange("p (c x) -> p x c", c=3), AX.X, OP.add)
        nc.scalar.activation(nds[:], nds[:], AF.Sqrt)
        nc.vector.tensor_scalar_max(nds[:], nds[:], EPS)
        nc.vector.reciprocal(nds[:], nds[:])
        nds3 = nds[:].rearrange("p (h x) -> p h x", x=V)
        for i in range(3):
            nc.gpsimd.tensor_tensor(
                gv[:, :, :, 3 + i],
                ldq[:, i * 96:(i + 1) * 96].rearrange("p (h x) -> p h x", x=V),
                nds3, OP.mult)
        nc.gpsimd.tensor_copy(
            feats[qt][:, 0:192].rearrange("p (h c) -> p h c", c=16),
            o_all[qt][:].rearrange("p (h f) -> p h f", f=FEAT)[:, :, 0:16])

    for t in range(6 + RUNP):
        if t < 6:
            emit_qk_exp(t)
        if t >= RUNP:
            emit_av(2 * (t - RUNP))
            emit_av(2 * (t - RUNP) + 1)
            if t - RUNP == 2:
                emit_inv_rot(0, 0)
                emit_inv_rot(1, 0)
    emit_inv_rot(0, 1)
    emit_inv_rot(1, 1)
    emit_inv_norm(0)
    emit_inv_norm(1)

    # ---- output projection ----------------------------------------------
    att_ctx.close()
    tpsum2 = ctx.enter_context(tc.tile_pool(name="tpsum2", bufs=2, space=PS))
    opsum2 = ctx.enter_context(tc.tile_pool(name="opsum2", bufs=2, space=PS))
    fT = []
    for kc in range(KCH):
        r0 = kc * 128
        rw = min(FOUT, r0 + 128) - r0
        pw = rw + 2 if kc == KCH - 1 else rw
        fT.append(work.tile([pw, NB], F32R, name=f"fT{kc}"))
    lastr = FOUT - (KCH - 1) * 128
    nc.vector.tensor_copy(fT[KCH - 1][lastr:lastr + 2, :], ones2_f32[:])
    for kc in range(KCH):
        r0 = kc * 128
        rw = min(FOUT, r0 + 128) - r0
        for qt in range(2):
            ps = tpsum2.tile([128, 128], F32, tag="tps2")
            nc.tensor.transpose(ps[:rw, :], feats[qt][:, r0:r0 + rw], ident[:])
            if kc % 2:
                nc.scalar.copy(fT[kc][:rw, qt * 128:(qt + 1) * 128], ps[:rw, :])
            else:
                nc.vector.tensor_copy(fT[kc][:rw, qt * 128:(qt + 1) * 128],
                                      ps[:rw, :])
    for qt in range(2):
        ps = opsum2.tile([128, CS], F32, tag="oproj")
        for kc in range(KCH):
            nc.tensor.matmul(ps[:], fT[kc][:, qt * 128:(qt + 1) * 128],
                             wout_sb[kc][:], start=(kc == 0),
                             stop=(kc == KCH - 1))
        osb = tmp.tile([128, CS], F32, tag="osb", name="osb")
        nc.scalar.copy(osb[:], ps[:])
        nc.sync.dma_start(out_loc[qt * 128:(qt + 1) * 128, :], osb[:])


def _run(inputs, trace=False):
    s, rot9, trans, wall, wout_b, qconst, has_bias = _host_prep(inputs)
    nc = _build_program(has_bias)
    in_maps = []
    for c in range(8):
        b, qb = c // 4, c % 4
        # rotate key rows so this core's queries are rows 0:256
        idx = np.r_[qb * NB:N, 0:qb * NB]
        in_maps.append({
            "s_all": np.ascontiguousarray(s[b][idx]),
            "rot_all": np.ascontiguousarray(rot9[b][idx]),
            "trans_all": np.ascontiguousarray(trans[b][idx]),
            "wall": wall, "wout_b": wout_b, "qconst": qconst,
        })
    res = run_bass_kernel_spmd(nc, in_maps, list(range(8)), trace=trace)
    out = np.empty((B, N, CS), np.float32)
    for c in range(8):
        b, qb = c // 4, c % 4
        out[b, qb * NB:(qb + 1) * NB] = res.results[c]["out_loc"]
    return out, res


def kernel(**inputs):
    out, _ = _run(inputs, trace=False)
    return out


def kernel_traced(**inputs):
    return _run(inputs, trace=True)


# revision 26
# speedup vs baseline: 1.9619x; 1.0006x over previous
"""Bass/Tile TRN2 kernel for EnhancedIPA3 — collective-free redesign.

8 cores = batch(2) x query-block(4).  Each core redundantly computes the
K/V-side features for ALL 1024 keys of its batch (projections + rigid
frame transforms), then runs attention for its own 256 queries only.  No
inter-core communication: the collective-bootstrap barrier and the two
serialized AllGathers of the previous design are gone, and the cores are
fully independent, so cross-core launch skew no longer costs anything.

Key rows are rotated per core so the core's own query rows are always
tiles 0..1 (softmax over keys is permutation invariant) — one SPMD
program serves all 8 cores.

Self-contained: hardcodes all shapes; only depends on numpy + concourse.
"""

import numpy as np
from contextlib import ExitStack

import concourse.bass as bass
import concourse.bacc as bacc
import concourse.mybir as mybir
import concourse.tile as tile
from concourse.bass_utils import run_bass_kernel_spmd
from concourse.masks import make_identity

F32 = mybir.dt.float32
F32R = mybir.dt.float32r
BF16 = mybir.dt.bfloat16
AF = mybir.ActivationFunctionType
OP = mybir.AluOpType
AX = mybir.AxisListType

B, N, CS, H, C, P, V = 2, 1024, 384, 12, 16, 4, 8
EPS = 1e-8
NB = 256               # query rows per core (2 tiles)
NKT = 8                # key tiles of 128
# wall column map
K_OFF, V_OFF, PTS_OFF, Q_OFF, G_OFF, QPTS_OFF = 0, 192, 384, 1248, 1440, 1488
WALL_COLS = 1776
NPK = 12               # kv points per head (0:4 k_pts, 4:12 v_pts)
FEAT = 64              # per-head feature stride in kf/qf
FS = 42                # live features per head
OCH = 68               # va per-head stride: v16 | pts48 | ones | pad3
FOUT = H * (C + 7 * V)  # 864
KCH = 7                # output-proj contraction chunks


def _host_prep(inputs):
    wq = np.asarray(inputs["wq"], np.float32)
    bq = np.asarray(inputs["bq"], np.float32)
    wkv = np.asarray(inputs["wkv"], np.float32)
    bkv = np.asarray(inputs["bkv"], np.float32)
    wqp = np.asarray(inputs["wqp"], np.float32)
    bqp = np.asarray(inputs["bqp"], np.float32)
    wkvp = np.asarray(inputs["wkvp"], np.float32)
    bkvp = np.asarray(inputs["bkvp"], np.float32)
    wg = np.asarray(inputs["wg"], np.float32)
    bg = np.asarray(inputs["bg"], np.float32)
    gw = np.asarray(inputs["geom_weight"], np.float32)
    hw = np.asarray(inputs["head_weights"], np.float32)
    sh = 1.0 / (1.0 + np.exp(-hw))
    gw0, gw1 = float(gw[0]), float(gw[1])

    wall = np.zeros((CS + 2, WALL_COLS), np.float32)
    wall[:CS, K_OFF:K_OFF + 192] = wkv[:, :192]
    wall[CS, K_OFF:K_OFF + 192] = bkv[:192]
    wall[:CS, V_OFF:V_OFF + 192] = wkv[:, 192:]
    wall[CS, V_OFF:V_OFF + 192] = bkv[192:]
    # kv pts planar: dst col = PTS_OFF + cc*144 + h*12 + p  <-  src h*72 + p*6 + cc
    cc, h, p = np.meshgrid(np.arange(6), np.arange(H), np.arange(12),
                           indexing="ij")
    dst = (PTS_OFF + cc * 144 + h * 12 + p).ravel()
    src = (h * 72 + p * 6 + cc).ravel()
    wall[:CS, dst] = wkvp[:, src]
    wall[CS, dst] = bkvp[src]
    # q scaled by sh/sqrt(C)
    qs = np.repeat(sh / np.sqrt(C), 16)
    wall[:CS, Q_OFF:Q_OFF + 192] = wq * qs[None, :]
    wall[CS, Q_OFF:Q_OFF + 192] = bq * qs
    wall[:CS, G_OFF:G_OFF + 48] = wg
    wall[CS, G_OFF:G_OFF + 48] = bg
    # q pts planar: dst col = QPTS_OFF + cc*48 + h*4 + p  <-  src h*24 + p*6 + cc
    cc, h, p = np.meshgrid(np.arange(6), np.arange(H), np.arange(P),
                           indexing="ij")
    dst = (QPTS_OFF + cc * 48 + h * 4 + p).ravel()
    src = (h * 24 + p * 6 + cc).ravel()
    wall[:CS, dst] = wqp[:, src]
    wall[CS, dst] = bqp[src]
    has_bias = bool(np.any(wall[CS] != 0.0))
    wall[CS + 1] = wall[CS] * 0.5
    wall[CS] = wall[CS + 1]

    bout_half = np.asarray(inputs["bout"], np.float32)[None, :] * 0.5
    wout_b = np.concatenate(
        [np.asarray(inputs["wout"], np.float32), bout_half, bout_half], axis=0)

    # on-chip constants (broadcast to 128 partitions by a rank-1 matmul)
    qconst = np.zeros((1, 144), np.float32)
    SC = gw0 * 0.5 * sh            # coord feature scale (with gate)
    DC = gw1 * sh                  # dir feature scale (with gate)
    qconst[0, 0:48] = np.repeat(SC, P)
    qconst[0, 48:96] = np.repeat(DC, P)
    qconst[0, 96:108] = sh * gw0 / P                      # qf[40]
    c2 = np.where(np.abs(gw0 * sh) > 1e-12, -1.0 / (gw0 * sh + 1e-30), 0.0)
    qconst[0, 108:120] = c2                               # q2 coefficient
    qconst[0, 120:132] = -sh * gw1                        # curvature coeff

    rot9 = np.ascontiguousarray(
        np.asarray(inputs["rot"], np.float32).reshape(B, N, 9))
    trans = np.asarray(inputs["trans"], np.float32)
    s = np.asarray(inputs["s"], np.float32)
    return s, rot9, trans, wall, wout_b, qconst, has_bias


_PROGRAM_CACHE = {}


def _build_program(has_bias):
    key = (bool(has_bias),)
    if key in _PROGRAM_CACHE:
        return _PROGRAM_CACHE[key]
    nc = bacc.Bacc("TRN2", target_bir_lowering=False, debug=False,
                   num_devices=8)
    s_all = nc.dram_tensor("s_all", [N, CS], F32, kind="ExternalInput")
    rot_all = nc.dram_tensor("rot_all", [N, 9], F32, kind="ExternalInput")
    trans_all = nc.dram_tensor("trans_all", [N, 3], F32, kind="ExternalInput")
    wall_d = nc.dram_tensor("wall", [CS + 2, WALL_COLS], F32,
                            kind="ExternalInput")
    wout_d = nc.dram_tensor("wout_b", [FOUT + 2, CS], F32,
                            kind="ExternalInput")
    qconst_d = nc.dram_tensor("qconst", [1, 144], F32, kind="ExternalInput")
    out_loc = nc.dram_tensor("out_loc", [NB, CS], F32, kind="ExternalOutput")

    with tile.TileContext(nc) as tc:
        with ExitStack() as ctx:
            _emit(ctx, tc, nc, s_all, rot_all, trans_all, wall_d, wout_d,
                  qconst_d, out_loc, has_bias)
    nc.compile()
    _PROGRAM_CACHE[key] = nc
    return nc


def _emit(ctx, tc, nc, s_all, rot_all, trans_all, wall_d, wout_d, qconst_d,
          out_loc, has_bias):
    PS = bass.MemorySpace.PSUM

    const = ctx.enter_context(tc.tile_pool(name="const", bufs=1))
    work = ctx.enter_context(tc.tile_pool(name="work", bufs=1))
    tmp = ctx.enter_context(tc.tile_pool(name="tmp", bufs=2))
    pre_ctx = ExitStack()
    pA = pre_ctx.enter_context(tc.tile_pool(name="pA", bufs=1))
    kio = pre_ctx.enter_context(tc.tile_pool(name="kio", bufs=2))
    ppsum = pre_ctx.enter_context(tc.tile_pool(name="ppsum", bufs=2, space=PS))
    tpsum = pre_ctx.enter_context(tc.tile_pool(name="tpsum", bufs=2, space=PS))

    # ---- constants -------------------------------------------------------
    ident = const.tile([128, 128], F32)
    make_identity(nc, ident[:])
    ident_r = const.tile([OCH, OCH], F32R)
    nc.vector.tensor_copy(ident_r[:], ident[0:OCH, 0:OCH])
    ones2_f32 = const.tile([2, NB], F32)
    nc.gpsimd.memset(ones2_f32[:], 1.0)
    ones_r = const.tile([2, 128], F32R)
    nc.vector.tensor_copy(ones_r[:], ones2_f32[:, 0:128])

    # ---- DMAs (s first: the transposes+projections are the critical path)
    s_sb = []
    for kt in range(NKT):
        t = kio.tile([128, CS], F32, tag="s", name=f"s{kt}")
        nc.sync.dma_start(t[:], s_all[kt * 128:(kt + 1) * 128, :])
        s_sb.append(t)
    rot_sb, trans_sb = [], []
    for kt in range(NKT):
        r = slice(kt * 128, (kt + 1) * 128)
        t = const.tile([128, 9], F32, name=f"rot{kt}")
        nc.sync.dma_start(t[:], rot_all[r, :])
        rot_sb.append(t)
        t = const.tile([128, 3], F32, name=f"trans{kt}")
        nc.sync.dma_start(t[:], trans_all[r, :])
        trans_sb.append(t)

    wall_sb = []
    for kc in range(3):
        t = pA.tile([128, WALL_COLS], F32R, name=f"wall{kc}")
        nc.sync.dma_start(t[:], wall_d[kc * 128:(kc + 1) * 128, :].bitcast(F32R))
        wall_sb.append(t)
    wall_bias = pA.tile([2, WALL_COLS], F32R)
    if has_bias:
        nc.sync.dma_start(wall_bias[:], wall_d[CS:CS + 2, :].bitcast(F32R))

    qconst_sb = const.tile([1, 144], F32R)
    nc.sync.dma_start(qconst_sb[:], qconst_d[:, :].bitcast(F32R))

    wout_sb = []
    for kc in range(KCH):
        r0 = kc * 128
        r1 = min(FOUT + 2, r0 + 128)
        t = const.tile([r1 - r0, CS], F32R, name=f"wout{kc}")
        nc.sync.dma_start(t[:], wout_d[r0:r1, :].bitcast(F32R))
        wout_sb.append(t)

    # ---- sT (transpose all of s) ----------------------------------------
    sT = pA.tile([128, 3 * N], F32R, name="sT")   # [:, kc*1024 + key]
    sT3 = sT[:].rearrange("p (c k) -> p c k", k=N)
    for kt in range(NKT):
        tps = tpsum.tile([128, 384], F32, tag="tps")
        for kc in range(3):
            nc.tensor.transpose(tps[:, kc * 128:(kc + 1) * 128],
                                s_sb[kt][:, kc * 128:(kc + 1) * 128], ident[:])
        dst = sT3[:, :, kt * 128:(kt + 1) * 128]
        src = tps[:].rearrange("p (c k) -> p c k", k=128)
        if kt % 2:
            nc.scalar.copy(dst, src)
        else:
            nc.vector.tensor_copy(dst, src)

    # ---- broadcast qconst row to 128 partitions --------------------------
    tps = tpsum.tile([128, 384], F32, tag="tps")
    nc.tensor.matmul(tps[:, 0:144], ones_r[0:1, :], qconst_sb[:, :],
                     start=True, stop=True)
    qcst = const.tile([128, 144], F32)
    nc.vector.tensor_copy(qcst[:], tps[:, 0:144])
    # slices: SC48 0:48 | DC48 48:96 | A12 96:108 | c2 108:120 | c3 120:132

    # ---- K/V side: all 8 key tiles --------------------------------------
    kfT = work.tile([128, 6 * N], BF16, name="kfT")   # [:, t*1024 + key]
    kfT3 = kfT[:].rearrange("p (t k) -> p t k", k=N)
    vaG = [work.tile([128, H * OCH], BF16, name=f"vaG{kb}")
           for kb in range(NKT)]
    kds = [work.tile([128, 144], F32, name=f"kds{qt}") for qt in range(2)]

    GROUPS_K = [(0, 384), (384, 896), (896, 1248)]

    def proj_mm(ps, c0, c1, kt):
        pv = ps[:, 0:c1 - c0]
        for kc in range(3):
            last = (kc == 2) and not has_bias
            nc.tensor.matmul(pv, sT3[:, kc, kt * 128:(kt + 1) * 128],
                             wall_sb[kc][:, c0:c1], start=(kc == 0), stop=last)
        if has_bias:
            nc.tensor.matmul(pv, ones_r[:, :], wall_bias[:, c0:c1],
                             start=False, stop=True)

    def transform(pts, pco, rt, tr, W, coords=True, dirs=True):
        """pts/pco: planar [128, 6*W] (bf16 in, bf16/f32 out)."""
        for i in range(3):
            if coords:
                dco = pco[:, i * W:(i + 1) * W]
                nc.scalar.activation(dco, pts[:, 0:W], AF.Identity,
                                     bias=tr[:, i:i + 1],
                                     scale=rt[:, 3 * i:3 * i + 1])
                nc.vector.scalar_tensor_tensor(dco, pts[:, W:2 * W],
                                               rt[:, 3 * i + 1:3 * i + 2], dco,
                                               OP.mult, OP.add)
                nc.vector.scalar_tensor_tensor(dco, pts[:, 2 * W:3 * W],
                                               rt[:, 3 * i + 2:3 * i + 3], dco,
                                               OP.mult, OP.add)
            if dirs:
                ddi = pco[:, (3 + i) * W:(4 + i) * W]
                nc.scalar.activation(ddi, pts[:, 3 * W:4 * W], AF.Copy,
                                     scale=rt[:, 3 * i:3 * i + 1])
                nc.vector.scalar_tensor_tensor(ddi, pts[:, 4 * W:5 * W],
                                               rt[:, 3 * i + 1:3 * i + 2], ddi,
                                               OP.mult, OP.add)
                nc.vector.scalar_tensor_tensor(ddi, pts[:, 5 * W:6 * W],
                                               rt[:, 3 * i + 2:3 * i + 3], ddi,
                                               OP.mult, OP.add)

    for kt in range(NKT):
        # projections: K+V | pts-a | pts-b
        ps_kv = ppsum.tile([128, 384], F32, tag="pg384", name="pskv")
        proj_mm(ps_kv, 0, 384, kt)
        ps_p1 = ppsum.tile([128, 512], F32, tag="pg512", name="psp1")
        proj_mm(ps_p1, 384, 896, kt)
        ps_p2 = ppsum.tile([128, 352], F32, tag="pg352", name="psp2")
        proj_mm(ps_p2, 896, 1248, kt)

        kf = kio.tile([128, H * FEAT], F32, tag="kf", name="kf")
        kfv = kf[:].rearrange("p (h f) -> p h f", f=FEAT)
        if kt < 2:
            # zero the pad cols 42:64 of this physical buffer once
            nc.gpsimd.memset(kfv[:, :, 42:64], 0.0)
        va = vaG[kt]
        vav = va[:].rearrange("p (h f) -> p h f", f=OCH)
        pts = kio.tile([128, 864], BF16, tag="pts", name="pts")

        # evacuations
        nc.scalar.copy(kfv[:, :, 0:16],
                       ps_kv[:, 0:192].rearrange("p (h c) -> p h c", c=16))
        nc.vector.tensor_copy(vav[:, :, 0:16],
                              ps_kv[:, 192:384].rearrange("p (h c) -> p h c", c=16))
        nc.vector.tensor_scalar_max(pts[:, 0:512], ps_p1[:], 0.0)
        nc.scalar.activation(pts[:, 512:864], ps_p2[:], AF.Relu)

        # rigid transform (planar, bf16)
        pco = kio.tile([128, 864], BF16, tag="pco", name="pco")
        transform(pts[:], pco[:], rot_sb[kt], trans_sb[kt], 144)
        pco3 = pco[:].rearrange("p (c h x) -> p c h x", c=6, x=NPK)

        # kf coord/dir features ([cc*4+p] per head) + va pts (fused copies)
        nc.gpsimd.tensor_copy(
            kfv[:, :, 16:28].rearrange("p h (c x) -> p c h x", c=3),
            pco3[:, 0:3, :, 0:4])
        nc.gpsimd.tensor_copy(
            kfv[:, :, 28:40].rearrange("p h (c x) -> p c h x", c=3),
            pco3[:, 3:6, :, 0:4])
        nc.gpsimd.tensor_copy(
            vav[:, :, 16:64].rearrange("p h (c x) -> p c h x", c=6),
            pco3[:, :, :, 4:12])
        nc.gpsimd.memset(vav[:, :, 64:65], 1.0)
        nc.gpsimd.memset(vav[:, :, 65:68], 0.0)

        # k2 (negated sum of squared coord features)
        sqs = tmp.tile([128, 144], F32, tag="sqs", name="sqs")
        nc.vector.tensor_tensor(
            sqs[:].rearrange("p (h x) -> p h x", x=12),
            kfv[:, :, 16:28], kfv[:, :, 16:28], OP.mult)
        k2 = tmp.tile([128, 12], F32, tag="k2", name="k2")
        nc.vector.tensor_reduce(
            k2[:], sqs[:].rearrange("p (h c x) -> p h c x", c=3, x=4),
            AX.XY, OP.add, negate=True)
        nc.vector.tensor_copy(kfv[:, :, 40], k2[:])
        nc.gpsimd.memset(kfv[:, :, 41], 1.0)
        if kt < 2:
            nc.gpsimd.tensor_copy(kds[kt][:].rearrange("p (h x) -> p h x", x=12),
                                  kfv[:, :, 28:40])

        # transpose kf -> kfT (2 head-pairs per psum tile)
        for t0 in range(0, 6, 2):
            tps = tpsum.tile([128, 384], F32, tag="tps")
            nc.tensor.transpose(tps[:, 0:128],
                                kf[:, t0 * 128:(t0 + 1) * 128], ident[:])
            nc.tensor.transpose(tps[:, 128:256],
                                kf[:, (t0 + 1) * 128:(t0 + 2) * 128], ident[:])
            dst = kfT3[:, t0:t0 + 2, kt * 128:(kt + 1) * 128]
            src = tps[:, 0:256].rearrange("p (t k) -> p t k", k=128)
            if t0 == 2:
                nc.scalar.copy(dst, src)
            else:
                nc.vector.tensor_copy(dst, src)

    # ---- Q side (own rows = tiles 0..1) ---------------------------------
    qf_sb = [work.tile([128, H * FEAT], F32, name=f"qf{qt}") for qt in range(2)]
    for qt in range(2):
        qf = qf_sb[qt]
        qfv = qf[:].rearrange("p (h f) -> p h f", f=FEAT)
        ps_a = ppsum.tile([128, 384], F32, tag="pg384", name="psqa")
        proj_mm(ps_a, Q_OFF, Q_OFF + 384, qt)
        ps_b = ppsum.tile([128, 352], F32, tag="pg352", name="psqb")
        proj_mm(ps_b, Q_OFF + 384, WALL_COLS, qt)

        nc.scalar.copy(qfv[:, :, 0:16],
                       ps_a[:, 0:192].rearrange("p (h c) -> p h c", c=16))
        g_sb = tmp.tile([128, 48], F32, tag="gsb", name="gsb")
        nc.scalar.activation(g_sb[:], ps_a[:, 192:240], AF.Sigmoid)
        qpts = tmp.tile([128, 288], BF16, tag="qpts", name="qpts")
        nc.vector.tensor_scalar_max(qpts[:, 0:144], ps_a[:, 240:384], 0.0)
        nc.vector.tensor_scalar_max(qpts[:, 144:288], ps_b[:, 0:144], 0.0)

        qpco = tmp.tile([128, 288], F32, tag="qpco", name="qpco")
        transform(qpts[:], qpco[:], rot_sb[qt], trans_sb[qt], 48)
        qpco3 = qpco[:].rearrange("p (c h x) -> p c h x", c=6, x=4)

        gc = tmp.tile([128, 48], F32, tag="gc", name="gc")
        gd = tmp.tile([128, 48], F32, tag="gd", name="gd")
        nc.vector.tensor_tensor(gc[:], g_sb[:], qcst[:, 0:48], OP.mult)
        nc.vector.tensor_tensor(gd[:], g_sb[:], qcst[:, 48:96], OP.mult)
        gc3 = gc[:].rearrange("p (h x) -> p h x", x=4)
        gd3 = gd[:].rearrange("p (h x) -> p h x", x=4)
        for cc in range(3):
            nc.vector.tensor_tensor(qfv[:, :, 16 + cc * 4:20 + cc * 4],
                                    qpco3[:, cc], gc3, OP.mult)
            nc.gpsimd.tensor_tensor(qfv[:, :, 28 + cc * 4:32 + cc * 4],
                                    qpco3[:, 3 + cc], gd3, OP.mult)
        nc.vector.tensor_copy(qfv[:, :, 40], qcst[:, 96:108])

        # q2 from coord features
        sqs = tmp.tile([128, 144], F32, tag="sqs", name="sqs")
        nc.vector.tensor_tensor(
            sqs[:].rearrange("p (h x) -> p h x", x=12),
            qfv[:, :, 16:28], qfv[:, :, 16:28], OP.mult)
        q2s = tmp.tile([128, 12], F32, tag="q2s", name="q2s")
        nc.vector.tensor_reduce(
            q2s[:], sqs[:].rearrange("p (h c x) -> p h c x", c=3, x=4),
            AX.XY, OP.add)

        # curvature from dir features vs raw kd features of same rows
        qdv = qfv[:, :, 28:40]
        kdv = kds[qt][:].rearrange("p (h x) -> p h x", x=12)
        crs = tmp.tile([128, 144], F32, tag="crs", name="crs")
        t1 = tmp.tile([128, 48], F32, tag="t1", name="t1")
        t2 = tmp.tile([128, 48], F32, tag="t2", name="t2")
        t13 = t1[:].rearrange("p (h x) -> p h x", x=4)
        t23 = t2[:].rearrange("p (h x) -> p h x", x=4)
        for c, (a, b2) in enumerate(((1, 2), (2, 0), (0, 1))):
            nc.vector.tensor_tensor(t13, qdv[:, :, a * 4:a * 4 + 4],
                                    kdv[:, :, b2 * 4:b2 * 4 + 4], OP.mult)
            nc.gpsimd.tensor_tensor(t23, qdv[:, :, b2 * 4:b2 * 4 + 4],
                                    kdv[:, :, a * 4:a * 4 + 4], OP.mult)
            nc.vector.tensor_tensor(crs[:, c * 48:(c + 1) * 48], t1[:], t2[:],
                                    OP.subtract)
        nc.vector.tensor_tensor(crs[:], crs[:], crs[:], OP.mult)
        csum = tmp.tile([128, 48], F32, tag="csum", name="csum")
        nc.vector.tensor_reduce(
            csum[:], crs[:].rearrange("p (c x) -> p x c", c=3), AX.X, OP.add)
        # |qfd|^2, |kd|^2 per (h,p)
        sqd = tmp.tile([128, 144], F32, tag="sqd", name="sqd")
        nq2 = tmp.tile([128, 48], F32, tag="nq2", name="nq2")
        nk2 = tmp.tile([128, 48], F32, tag="nk2", name="nk2")
        nc.gpsimd.tensor_tensor(sqd[:].rearrange("p (h x) -> p h x", x=12),
                                qdv, qdv, OP.mult)
        nc.vector.tensor_reduce(
            nq2[:].rearrange("p (h x) -> p h x", x=4),
            sqd[:].rearrange("p (h c x) -> p h x c", c=3, x=4), AX.X, OP.add)
        nc.gpsimd.tensor_tensor(sqd[:].rearrange("p (h x) -> p h x", x=12),
                                kdv, kdv, OP.mult)
        nc.vector.tensor_reduce(
            nk2[:].rearrange("p (h x) -> p h x", x=4),
            sqd[:].rearrange("p (h c x) -> p h x c", c=3, x=4), AX.X, OP.add)
        nc.vector.tensor_tensor(nq2[:], nq2[:], nk2[:], OP.mult)
        nc.scalar.activation(nq2[:], nq2[:], AF.Sqrt)
        nc.vector.tensor_scalar_add(nq2[:], nq2[:], EPS)
        nc.vector.reciprocal(nq2[:], nq2[:])
        nc.scalar.activation(csum[:], csum[:], AF.Sqrt)
        nc.vector.tensor_tensor(csum[:], csum[:], nq2[:], OP.mult)
        curv = tmp.tile([128, 12], F32, tag="curv", name="curv")
        nc.vector.tensor_reduce(
            curv[:], csum[:].rearrange("p (h x) -> p h x", x=4), AX.X, OP.add)
        # qf[41] = c2*q2s + c3*curv
        nc.vector.tensor_tensor(q2s[:], q2s[:], qcst[:, 108:120], OP.mult)
        nc.vector.tensor_tensor(curv[:], curv[:], qcst[:, 120:132], OP.mult)
        nc.vector.tensor_tensor(qfv[:, :, 41], q2s[:], curv[:], OP.add)

    # ---- qfT: masked transposes (even head | odd head halves) -----------
    qfT = [work.tile([128, 2 * NB], BF16, name=f"qfT{t}") for t in range(6)]
    for t in range(6):
        nc.gpsimd.memset(qfT[t][:], 0.0)
    for t in range(6):
        for qt in range(2):
            tps = tpsum.tile([128, 384], F32, tag="tps")
            nc.tensor.transpose(tps[:, 0:128],
                                qf_sb[qt][:, t * 128:(t + 1) * 128], ident[:])
            eng = nc.scalar if (t + qt) % 2 else nc.vector
            eng_copy = eng.copy if eng is nc.scalar else eng.tensor_copy
            eng_copy(qfT[t][0:FS, qt * 128:(qt + 1) * 128], tps[0:FS, 0:128])
            eng2 = nc.vector if (t + qt) % 2 else nc.scalar
            eng2_copy = eng2.copy if eng2 is nc.scalar else eng2.tensor_copy
            eng2_copy(qfT[t][64:64 + FS, NB + qt * 128:NB + (qt + 1) * 128],
                      tps[64:64 + FS, 0:128])

    # ---- attention -------------------------------------------------------
    pre_ctx.close()
    att_ctx = ExitStack()
    apsum = att_ctx.enter_context(tc.tile_pool(name="apsum", bufs=2, space=PS))
    opsum = att_ctx.enter_context(tc.tile_pool(name="opsum", bufs=2, space=PS))
    otp = att_ctx.enter_context(tc.tile_pool(name="otp", bufs=2, space=PS))
    expT_tiles = [work.tile([128, 4096], BF16, name=f"expT{i}")
                  for i in range(3)]
    o_all = [work.tile([128, FEAT * H], F32, name=f"oall{qt}")
             for qt in range(2)]
    feats = [work.tile([128, FOUT], F32, name=f"feats{qt}") for qt in range(2)]
    ld_sb = [work.tile([128, 288], F32, name=f"ld{qt}") for qt in range(2)]
    RUNP = 2

    def emit_qk_exp(t):
        expT = expT_tiles[t % 3]
        for p4 in range(4):
            aps = apsum.tile([128, 1024], F32, tag="aps", name="aps")
            for j in range(2):
                kb = p4 * 2 + j
                nc.tensor.matmul(aps[:, j * 512:(j + 1) * 512],
                                 kfT3[:, t, kb * 128:(kb + 1) * 128],
                                 qfT[t][:, :], start=True, stop=True)
            nc.scalar.activation(expT[:, p4 * 1024:(p4 + 1) * 1024], aps[:],
                                 AF.Exp)

    def emit_av(h):
        t, e = h // 2, h % 2
        expT = expT_tiles[t % 3]
        ot_ps = opsum.tile([OCH, NB], F32, tag="ot", name="ot_ps")
        for kb in range(NKT):
            nc.tensor.matmul(
                ot_ps[:], vaG[kb][:, h * OCH:(h + 1) * OCH],
                expT[:, kb * 512 + e * NB:kb * 512 + (e + 1) * NB],
                start=(kb == 0), stop=(kb == NKT - 1))
        ot_sb = tmp.tile([OCH, NB], F32R, tag="otsb", name="otsb", bufs=2)
        nc.vector.tensor_copy(ot_sb[:], ot_ps[:])
        for qt in range(2):
            tp = otp.tile([128, OCH], F32R, tag="tp", name="tp")
            nc.tensor.transpose(tp[:], ot_sb[:, qt * 128:(qt + 1) * 128],
                                ident_r[:, :])
            rec = tmp.tile([128, 1], F32, tag="rec", name="rec", bufs=2)
            nc.vector.reciprocal(rec[:], tp[:, 64:65].bitcast(F32))
            nc.vector.tensor_scalar_mul(
                o_all[qt][:, h * FEAT:h * FEAT + 64], tp[:, 0:64].bitcast(F32),
                rec[:])

    def emit_inv_rot(qt, hh):
        """Rotate o_geom back to local frame for heads hh*6..hh*6+5."""
        rt, tr = rot_sb[qt], trans_sb[qt]
        hs = slice(hh * 6, hh * 6 + 6)
        ov = o_all[qt][:].rearrange("p (h f) -> p h f", f=FEAT)[:, hs]
        gv = feats[qt][:, 192:FOUT].rearrange(
            "p (h x c) -> p h x c", h=H, c=7)[:, hs]

        def og(j):
            return ov[:, :, 16 + 8 * j:24 + 8 * j]

        ogs = tmp.tile([128, 144], F32, tag="ogs", name="ogs", bufs=2)
        ogs3 = ogs[:].rearrange("p (c x) -> p c x", c=3)
        for j in range(3):
            nc.vector.tensor_scalar(
                ogs3[:, j].rearrange("p (h x) -> p h x", x=V), og(j),
                tr[:, j:j + 1], None, OP.subtract)
        lci = tmp.tile([128, 48], F32, tag="lci", name="lci", bufs=2)
        for i in range(3):
            nc.vector.tensor_scalar_mul(lci[:], ogs3[:, 0], rt[:, i:i + 1])
            nc.vector.scalar_tensor_tensor(lci[:], ogs3[:, 1],
                                           rt[:, 3 + i:4 + i], lci[:],
                                           OP.mult, OP.add)
            nc.vector.scalar_tensor_tensor(
                gv[:, :, :, i], ogs3[:, 2].rearrange("p (h x) -> p h x", x=V),
                rt[:, 6 + i:7 + i],
                lci[:].rearrange("p (h x) -> p h x", x=V), OP.mult, OP.add)

    def emit_inv_norm(qt):
        rt = rot_sb[qt]
        gv = feats[qt][:, 192:FOUT].rearrange("p (h x c) -> p h x c", h=H, c=7)
        # ld chains, full width (all 12 heads)
        ovd = o_all[qt][:].rearrange("p (h f) -> p h f", f=FEAT)
        for i in range(3):
            ldd = ld_sb[qt][:, i * 96:(i + 1) * 96]
            ldd3 = ldd.rearrange("p (h x) -> p h x", x=V)
            nc.vector.tensor_scalar_mul(ldd3, ovd[:, :, 40:48], rt[:, i:i + 1])
            nc.vector.scalar_tensor_tensor(ldd3, ovd[:, :, 48:56],
                                           rt[:, 3 + i:4 + i], ldd3,
                                           OP.mult, OP.add)
            nc.vector.scalar_tensor_tensor(ldd3, ovd[:, :, 56:64],
                                           rt[:, 6 + i:7 + i], ldd3,
                                           OP.mult, OP.add)
        lsq = tmp.tile([128, 288], F32, tag="lsq", name="lsq")
        lsq4 = lsq[:].rearrange("p (h x c) -> p h x c", c=3, x=V)
        nc.vector.tensor_tensor(lsq4, gv[:, :, :, 0:3], gv[:, :, :, 0:3],
                                OP.mult)
        ncs = tmp.tile([128, 96], F32, tag="ncs", name="ncs")
        nc.vector.tensor_reduce(
            ncs[:], lsq[:].rearrange("p (x c) -> p x c", c=3), AX.X, OP.add)
        nc.scalar.activation(gv[:, :, :, 6],
                             ncs[:].rearrange("p (h x) -> p h x", x=V), AF.Sqrt)
        # ld normalization
        ldq = ld_sb[qt]
        nc.gpsimd.tensor_tensor(lsq[:], ldq[:], ldq[:], OP.mult)
        nds = tmp.tile([128, 96], F32, tag="nds", name="nds")
        nc.vector.tensor_reduce(
            nds[:], lsq[:].rearrange("p (c x) -> p x c", c=3), AX.X, OP.add)
        nc.scalar.activation(nds[:], nds[:], AF.Sqrt)
        nc.vector.tensor_scalar_max(nds[:], nds[:], EPS)
        nc.vector.reciprocal(nds[:], nds[:])
        nds3 = nds[:].rearrange("p (h x) -> p h x", x=V)
        for i in range(3):
            nc.gpsimd.tensor_tensor(
                gv[:, :, :, 3 + i],
                ldq[:, i * 96:(i + 1) * 96].rearrange("p (h x) -> p h x", x=V),
                nds3, OP.mult)
        nc.gpsimd.tensor_copy(
            feats[qt][:, 0:192].rearrange("p (h c) -> p h c", c=16),
            o_all[qt][:].rearrange("p (h f) -> p h f", f=FEAT)[:, :, 0:16])

    for t in range(6 + RUNP):
        if t < 6:
            emit_qk_exp(t)
        if t >= RUNP:
            emit_av(2 * (t - RUNP))
            emit_av(2 * (t - RUNP) + 1)
            if t - RUNP == 2:
                emit_inv_rot(0, 0)
                emit_inv_rot(1, 0)
    emit_inv_rot(0, 1)
    emit_inv_rot(1, 1)
    emit_inv_norm(0)
    emit_inv_norm(1)

    # ---- output projection ----------------------------------------------
    att_ctx.close()
    tpsum2 = ctx.enter_context(tc.tile_pool(name="tpsum2", bufs=2, space=PS))
    opsum2 = ctx.enter_context(tc.tile_pool(name="opsum2", bufs=2, space=PS))
    fT = []
    for kc in range(KCH):
        r0 = kc * 128
        rw = min(FOUT, r0 + 128) - r0
        pw = rw + 2 if kc == KCH - 1 else rw
        fT.append(work.tile([pw, NB], F32R, name=f"fT{kc}"))
    lastr = FOUT - (KCH - 1) * 128
    nc.vector.tensor_copy(fT[KCH - 1][lastr:lastr + 2, :], ones2_f32[:])
    for kc in range(KCH):
        r0 = kc * 128
        rw = min(FOUT, r0 + 128) - r0
        for qt in range(2):
            ps = tpsum2.tile([128, 128], F32, tag="tps2")
            nc.tensor.transpose(ps[:rw, :], feats[qt][:, r0:r0 + rw], ident[:])
            if kc % 2:
                nc.scalar.copy(fT[kc][:rw, qt * 128:(qt + 1) * 128], ps[:rw, :])
            else:
                nc.vector.tensor_copy(fT[kc][:rw, qt * 128:(qt + 1) * 128],
                                      ps[:rw, :])
    for qt in range(2):
        ps = opsum2.tile([128, CS], F32, tag="oproj")
        for kc in range(KCH):
            nc.tensor.matmul(ps[:], fT[kc][:, qt * 128:(qt + 1) * 128],
                             wout_sb[kc][:], start=(kc == 0),
                             stop=(kc == KCH - 1))
        osb = tmp.tile([128, CS], F32, tag="osb", name="osb")
        nc.scalar.copy(osb[:], ps[:])
        nc.sync.dma_start(out_loc[qt * 128:(qt + 1) * 128, :], osb[:])


def _run(inputs, trace=False):
    s, rot9, trans, wall, wout_b, qconst, has_bias = _host_prep(inputs)
    nc = _build_program(has_bias)
    in_maps = []
    for c in range(8):
        b, qb = c // 4, c % 4
        # rotate key rows so this core's queries are rows 0:256
        idx = np.r_[qb * NB:N, 0:qb * NB]
        in_maps.append({
            "s_all": np.ascontiguousarray(s[b][idx]),
            "rot_all": np.ascontiguousarray(rot9[b][idx]),
            "trans_all": np.ascontiguousarray(trans[b][idx]),
            "wall": wall, "wout_b": wout_b, "qconst": qconst,
        })
    res = run_bass_kernel_spmd(nc, in_maps, list(range(8)), trace=trace)
    out = np.empty((B, N, CS), np.float32)
    for c in range(8):
        b, qb = c // 4, c % 4
        out[b, qb * NB:(qb + 1) * NB] = res.results[c]["out_loc"]
    return out, res


def kernel(**inputs):
    out, _ = _run(inputs, trace=False)
    return out


def kernel_traced(**inputs):
    return _run(inputs, trace=True)


# revision 41
# speedup vs baseline: 2.1548x; 1.0983x over previous
"""Bass/Tile TRN2 kernel for EnhancedIPA3 — collective-free redesign.

8 cores = batch(2) x query-block(4).  Each core redundantly computes the
K/V-side features for ALL 1024 keys of its batch (projections + rigid
frame transforms), then runs attention for its own 256 queries only.  No
inter-core communication: the collective-bootstrap barrier and the two
serialized AllGathers of the previous design are gone, and the cores are
fully independent, so cross-core launch skew no longer costs anything.

Key rows are rotated per core so the core's own query rows are always
tiles 0..1 (softmax over keys is permutation invariant) — one SPMD
program serves all 8 cores.

Self-contained: hardcodes all shapes; only depends on numpy + concourse.
"""

import numpy as np
from contextlib import ExitStack

import concourse.bass as bass
import concourse.bacc as bacc
import concourse.mybir as mybir
import concourse.tile as tile
from concourse.bass_utils import run_bass_kernel_spmd
from concourse.masks import make_identity

F32 = mybir.dt.float32
F32R = mybir.dt.float32r
BF16 = mybir.dt.bfloat16
AF = mybir.ActivationFunctionType
OP = mybir.AluOpType
AX = mybir.AxisListType

B, N, CS, H, C, P, V = 2, 1024, 384, 12, 16, 4, 8
EPS = 1e-8
NB = 256               # query rows per core (2 tiles)
NKT = 8                # key tiles of 128
# wall column map
K_OFF, V_OFF, PTS_OFF, Q_OFF, G_OFF, QPTS_OFF = 0, 192, 384, 1248, 1440, 1488
WALL_COLS = 1776
NPK = 12               # kv points per head (0:4 k_pts, 4:12 v_pts)
FEAT = 64              # per-head feature stride in kf/qf
FS = 42                # live features per head
OCH = 68               # va per-head stride: v16 | pts48 | ones | pad3
FOUT = H * (C + 7 * V)  # 864
KCH = 7                # output-proj contraction chunks


def _host_prep(inputs):
    wq = np.asarray(inputs["wq"], np.float32)
    bq = np.asarray(inputs["bq"], np.float32)
    wkv = np.asarray(inputs["wkv"], np.float32)
    bkv = np.asarray(inputs["bkv"], np.float32)
    wqp = np.asarray(inputs["wqp"], np.float32)
    bqp = np.asarray(inputs["bqp"], np.float32)
    wkvp = np.asarray(inputs["wkvp"], np.float32)
    bkvp = np.asarray(inputs["bkvp"], np.float32)
    wg = np.asarray(inputs["wg"], np.float32)
    bg = np.asarray(inputs["bg"], np.float32)
    gw = np.asarray(inputs["geom_weight"], np.float32)
    hw = np.asarray(inputs["head_weights"], np.float32)
    sh = 1.0 / (1.0 + np.exp(-hw))
    gw0, gw1 = float(gw[0]), float(gw[1])

    wall = np.zeros((CS + 2, WALL_COLS), np.float32)
    wall[:CS, K_OFF:K_OFF + 192] = wkv[:, :192]
    wall[CS, K_OFF:K_OFF + 192] = bkv[:192]
    wall[:CS, V_OFF:V_OFF + 192] = wkv[:, 192:]
    wall[CS, V_OFF:V_OFF + 192] = bkv[192:]
    # kv pts planar: dst col = PTS_OFF + cc*144 + h*12 + p  <-  src h*72 + p*6 + cc
    cc, h, p = np.meshgrid(np.arange(6), np.arange(H), np.arange(12),
                           indexing="ij")
    dst = (PTS_OFF + cc * 144 + h * 12 + p).ravel()
    src = (h * 72 + p * 6 + cc).ravel()
    wall[:CS, dst] = wkvp[:, src]
    wall[CS, dst] = bkvp[src]
    # q scaled by sh/sqrt(C)
    qs = np.repeat(sh / np.sqrt(C), 16)
    wall[:CS, Q_OFF:Q_OFF + 192] = wq * qs[None, :]
    wall[CS, Q_OFF:Q_OFF + 192] = bq * qs
    wall[:CS, G_OFF:G_OFF + 48] = wg
    wall[CS, G_OFF:G_OFF + 48] = bg
    # q pts planar: dst col = QPTS_OFF + cc*48 + h*4 + p  <-  src h*24 + p*6 + cc
    cc, h, p = np.meshgrid(np.arange(6), np.arange(H), np.arange(P),
                           indexing="ij")
    dst = (QPTS_OFF + cc * 48 + h * 4 + p).ravel()
    src = (h * 24 + p * 6 + cc).ravel()
    wall[:CS, dst] = wqp[:, src]
    wall[CS, dst] = bqp[src]
    has_bias = bool(np.any(wall[CS] != 0.0))
    wall[CS + 1] = wall[CS] * 0.5
    wall[CS] = wall[CS + 1]

    bout_half = np.asarray(inputs["bout"], np.float32)[None, :] * 0.5
    wout_b = np.concatenate(
        [np.asarray(inputs["wout"], np.float32), bout_half, bout_half], axis=0)

    # on-chip constants (broadcast to 128 partitions by a rank-1 matmul)
    qconst = np.zeros((1, 144), np.float32)
    SC = gw0 * 0.5 * sh            # coord feature scale (with gate)
    DC = gw1 * sh                  # dir feature scale (with gate)
    qconst[0, 0:48] = np.repeat(SC, P)
    qconst[0, 48:96] = np.repeat(DC, P)
    qconst[0, 96:108] = sh * gw0 / P                      # qf[40]
    c2 = np.where(np.abs(gw0 * sh) > 1e-12, -1.0 / (gw0 * sh + 1e-30), 0.0)
    qconst[0, 108:120] = c2                               # q2 coefficient
    qconst[0, 120:132] = -sh * gw1                        # curvature coeff

    rot9 = np.asarray(inputs["rot"], np.float32).reshape(B, N, 9)
    trans = np.asarray(inputs["trans"], np.float32)
    rt_all = np.ascontiguousarray(np.concatenate([rot9, trans], axis=2))
    s = np.asarray(inputs["s"], np.float32)
    return s, rt_all, wall, wout_b, qconst, has_bias


_PROGRAM_CACHE = {}


def _build_program(has_bias):
    key = (bool(has_bias),)
    if key in _PROGRAM_CACHE:
        return _PROGRAM_CACHE[key]
    nc = bacc.Bacc("TRN2", target_bir_lowering=False, debug=False,
                   num_devices=8)
    s_all = nc.dram_tensor("s_all", [N, CS], F32, kind="ExternalInput")
    rt_d = nc.dram_tensor("rt_all", [N, 12], F32, kind="ExternalInput")
    wall_d = nc.dram_tensor("wall", [CS + 2, WALL_COLS], F32,
                            kind="ExternalInput")
    wout_d = nc.dram_tensor("wout_b", [FOUT + 2, CS], F32,
                            kind="ExternalInput")
    qconst_d = nc.dram_tensor("qconst", [1, 144], F32, kind="ExternalInput")
    out_loc = nc.dram_tensor("out_loc", [NB, CS], F32, kind="ExternalOutput")

    with tile.TileContext(nc) as tc:
        with ExitStack() as ctx:
            _emit(ctx, tc, nc, s_all, rt_d, wall_d, wout_d,
                  qconst_d, out_loc, has_bias)
    nc.compile()
    _PROGRAM_CACHE[key] = nc
    return nc


def _emit(ctx, tc, nc, s_all, rt_d, wall_d, wout_d, qconst_d,
          out_loc, has_bias):
    PS = bass.MemorySpace.PSUM

    const = ctx.enter_context(tc.tile_pool(name="const", bufs=1))
    work = ctx.enter_context(tc.tile_pool(name="work", bufs=1))
    tmp = ctx.enter_context(tc.tile_pool(name="tmp", bufs=2))
    pre_ctx = ExitStack()
    pA = pre_ctx.enter_context(tc.tile_pool(name="pA", bufs=1))
    kio = pre_ctx.enter_context(tc.tile_pool(name="kio", bufs=2))
    ppsum = pre_ctx.enter_context(tc.tile_pool(name="ppsum", bufs=2, space=PS))
    tpsum = pre_ctx.enter_context(tc.tile_pool(name="tpsum", bufs=2, space=PS))

    # ---- constants -------------------------------------------------------
    ident = const.tile([128, 128], F32)
    make_identity(nc, ident[:])
    ident_r = const.tile([OCH, OCH], F32R)
    nc.vector.tensor_copy(ident_r[:], ident[0:OCH, 0:OCH])
    ones2_f32 = const.tile([2, NB], F32)
    nc.gpsimd.memset(ones2_f32[:], 1.0)
    ones_r = const.tile([2, 128], F32R)
    nc.vector.tensor_copy(ones_r[:], ones2_f32[:, 0:128])

    # ---- DMAs (s first: the transposes+projections are the critical path;
    # issue on multiple engine queues to parallelize descriptor setup)
    s_sb = []
    for kt in range(NKT):
        t = pA.tile([128, CS], F32, name=f"s{kt}")
        nc.sync.dma_start(t[:], s_all[kt * 128:(kt + 1) * 128, :])
        s_sb.append(t)
    # all rot+trans rows in one DMA: rt_sb[:, kt*12+c] = rt_all[kt*128+p, c]
    rt_sb = const.tile([128, 96], F32, name="rt_sb")
    nc.gpsimd.dma_start(rt_sb[:],
                        rt_d[:, :].rearrange("(t p) c -> p t c", p=128))
    rtb_sb = const.tile([128, 96], BF16, name="rtb_sb")
    nc.gpsimd.tensor_copy(rtb_sb[:], rt_sb[:])

    def Rc(kt, j, b=False):
        t = rtb_sb if b else rt_sb
        return t[:, kt * 12 + j:kt * 12 + j + 1]

    def Tc(kt, j, b=False):
        t = rtb_sb if b else rt_sb
        return t[:, kt * 12 + 9 + j:kt * 12 + 9 + j + 1]

    wall_sb = []
    for kc in range(3):
        t = pA.tile([128, WALL_COLS], F32R, name=f"wall{kc}")
        nc.sync.dma_start(t[:], wall_d[kc * 128:(kc + 1) * 128, :].bitcast(F32R))
        wall_sb.append(t)
    wall_bias = pA.tile([2, WALL_COLS], F32R)
    if has_bias:
        nc.sync.dma_start(wall_bias[:], wall_d[CS:CS + 2, :].bitcast(F32R))

    qconst_sb = const.tile([1, 144], F32R)
    nc.gpsimd.dma_start(qconst_sb[:], qconst_d[:, :].bitcast(F32R))

    wout_sb = []
    for kc in range(KCH):
        r0 = kc * 128
        r1 = min(FOUT + 2, r0 + 128)
        t = const.tile([r1 - r0, CS], F32R, name=f"wout{kc}")
        nc.scalar.dma_start(t[:], wout_d[r0:r1, :].bitcast(F32R))
        wout_sb.append(t)

    # ---- sT (transpose all of s) ----------------------------------------
    sT = pA.tile([128, 3 * N], F32R, name="sT")   # [:, kc*1024 + key]
    sT3 = sT[:].rearrange("p (c k) -> p c k", k=N)
    for kt in range(NKT):
        tps = tpsum.tile([128, 384], F32, tag="tps")
        for kc in range(3):
            nc.tensor.transpose(tps[:, kc * 128:(kc + 1) * 128],
                                s_sb[kt][:, kc * 128:(kc + 1) * 128], ident[:])
        dst = sT3[:, :, kt * 128:(kt + 1) * 128]
        src = tps[:].rearrange("p (c k) -> p c k", k=128)
        if kt % 2:
            nc.scalar.copy(dst, src)
        else:
            nc.vector.tensor_copy(dst, src)

    # ---- broadcast qconst row to 128 partitions --------------------------
    tps = tpsum.tile([128, 384], F32, tag="tps")
    nc.tensor.matmul(tps[:, 0:144], ones_r[0:1, :], qconst_sb[:, :],
                     start=True, stop=True)
    qcst = const.tile([128, 144], F32)
    nc.vector.tensor_copy(qcst[:], tps[:, 0:144])
    # slices: SC48 0:48 | DC48 48:96 | A12 96:108 | c2 108:120 | c3 120:132

    # ---- K/V side: all 8 key tiles --------------------------------------
    kfT = work.tile([128, 6 * N], BF16, name="kfT")   # [:, t*1024 + key]
    kfT3 = kfT[:].rearrange("p (t k) -> p t k", k=N)
    vaG = [work.tile([128, H * OCH], BF16, name=f"vaG{kb}")
           for kb in range(NKT)]
    kds = [work.tile([128, 144], F32, name=f"kds{qt}") for qt in range(2)]

    GROUPS_K = [(0, 384), (384, 896), (896, 1248)]

    def proj_mm(ps, c0, c1, kt):
        pv = ps[:, 0:c1 - c0]
        for kc in range(3):
            last = (kc == 2) and not has_bias
            nc.tensor.matmul(pv, sT3[:, kc, kt * 128:(kt + 1) * 128],
                             wall_sb[kc][:, c0:c1], start=(kc == 0), stop=last)
        if has_bias:
            nc.tensor.matmul(pv, ones_r[:, :], wall_bias[:, c0:c1],
                             start=False, stop=True)

    def transform(pts, pco, kt, W):
        """pts/pco: planar [128, 6*W] bf16; rigid transform per comp."""
        for i in range(3):
            dco = pco[:, i * W:(i + 1) * W]
            nc.scalar.activation(dco, pts[:, 0:W], AF.Identity,
                                 bias=Tc(kt, i), scale=Rc(kt, 3 * i))
            nc.vector.scalar_tensor_tensor(dco, pts[:, W:2 * W],
                                           Rc(kt, 3 * i + 1, True), dco,
                                           OP.mult, OP.add)
            nc.vector.scalar_tensor_tensor(dco, pts[:, 2 * W:3 * W],
                                           Rc(kt, 3 * i + 2, True), dco,
                                           OP.mult, OP.add)
            ddi = pco[:, (3 + i) * W:(4 + i) * W]
            nc.scalar.activation(ddi, pts[:, 3 * W:4 * W], AF.Copy,
                                 scale=Rc(kt, 3 * i))
            nc.vector.scalar_tensor_tensor(ddi, pts[:, 4 * W:5 * W],
                                           Rc(kt, 3 * i + 1, True), ddi,
                                           OP.mult, OP.add)
            nc.vector.scalar_tensor_tensor(ddi, pts[:, 5 * W:6 * W],
                                           Rc(kt, 3 * i + 2, True), ddi,
                                           OP.mult, OP.add)

    for kt in range(NKT):
        # projections: K+V | pts-a | pts-b
        ps_kv = ppsum.tile([128, 384], F32, tag="pg384", name="pskv")
        proj_mm(ps_kv, 0, 384, kt)
        ps_p1 = ppsum.tile([128, 512], F32, tag="pg512", name="psp1")
        proj_mm(ps_p1, 384, 896, kt)
        ps_p2 = ppsum.tile([128, 352], F32, tag="pg352", name="psp2")
        proj_mm(ps_p2, 896, 1248, kt)

        kf = kio.tile([128, H * FEAT], F32, tag="kf", name="kf", bufs=3)
        kfv = kf[:].rearrange("p (h f) -> p h f", f=FEAT)
        if kt < 2:
            # zero the pad cols 42:64 of this physical buffer once
            nc.gpsimd.memset(kfv[:, :, 42:64], 0.0)
        va = vaG[kt]
        vav = va[:].rearrange("p (h f) -> p h f", f=OCH)
        pts = kio.tile([128, 864], BF16, tag="pts", name="pts", bufs=3)

        # evacuations
        nc.scalar.copy(kfv[:, :, 0:16],
                       ps_kv[:, 0:192].rearrange("p (h c) -> p h c", c=16))
        nc.vector.tensor_copy(vav[:, :, 0:16],
                              ps_kv[:, 192:384].rearrange("p (h c) -> p h c", c=16))
        nc.vector.tensor_scalar_max(pts[:, 0:512], ps_p1[:], 0.0)
        nc.scalar.activation(pts[:, 512:864], ps_p2[:], AF.Relu)

        # rigid transform (planar, bf16)
        pco = kio.tile([128, 864], BF16, tag="pco", name="pco", bufs=3)
        transform(pts[:], pco[:], kt, 144)
        pco3 = pco[:].rearrange("p (c h x) -> p c h x", c=6, x=NPK)

        # kf coord/dir features ([cc*4+p] per head) + va pts (fused copies)
        nc.gpsimd.tensor_copy(
            kfv[:, :, 16:28].rearrange("p h (c x) -> p c h x", c=3),
            pco3[:, 0:3, :, 0:4])
        nc.gpsimd.tensor_copy(
            kfv[:, :, 28:40].rearrange("p h (c x) -> p c h x", c=3),
            pco3[:, 3:6, :, 0:4])
        nc.vector.tensor_copy(
            vav[:, :, 16:64].rearrange("p h (c x) -> p c h x", c=6),
            pco3[:, :, :, 4:12])
        nc.gpsimd.memset(vav[:, :, 64:65], 1.0)
        nc.gpsimd.memset(vav[:, :, 65:68], 0.0)

        # k2 (negated sum of squared coord features)
        sqs = tmp.tile([128, 144], F32, tag="sqs", name="sqs")
        nc.vector.tensor_tensor(
            sqs[:].rearrange("p (h x) -> p h x", x=12),
            kfv[:, :, 16:28], kfv[:, :, 16:28], OP.mult)
        k2 = tmp.tile([128, 12], F32, tag="k2", name="k2")
        nc.vector.tensor_reduce(
            k2[:], sqs[:].rearrange("p (h c x) -> p h c x", c=3, x=4),
            AX.XY, OP.add, negate=True)
        nc.vector.tensor_copy(kfv[:, :, 40], k2[:])
        nc.gpsimd.memset(kfv[:, :, 41], 1.0)
        if kt < 2:
            nc.gpsimd.tensor_copy(kds[kt][:].rearrange("p (h x) -> p h x", x=12),
                                  kfv[:, :, 28:40])

        # transpose kf -> kfT (2 head-pairs per psum tile)
        for t0 in range(0, 6, 2):
            tps = tpsum.tile([128, 384], F32, tag="tps")
            nc.tensor.transpose(tps[:, 0:128],
                                kf[:, t0 * 128:(t0 + 1) * 128], ident[:])
            nc.tensor.transpose(tps[:, 128:256],
                                kf[:, (t0 + 1) * 128:(t0 + 2) * 128], ident[:])
            dst = kfT3[:, t0:t0 + 2, kt * 128:(kt + 1) * 128]
            src = tps[:, 0:256].rearrange("p (t k) -> p t k", k=128)
            if t0 == 2:
                nc.scalar.copy(dst, src)
            else:
                nc.vector.tensor_copy(dst, src)

    # ---- Q side (own rows = tiles 0..1) ---------------------------------
    qf_sb = [work.tile([128, H * FEAT], F32, name=f"qf{qt}") for qt in range(2)]
    for qt in range(2):
        qf = qf_sb[qt]
        qfv = qf[:].rearrange("p (h f) -> p h f", f=FEAT)
        ps_a = ppsum.tile([128, 384], F32, tag="pg384", name="psqa")
        proj_mm(ps_a, Q_OFF, Q_OFF + 384, qt)
        ps_b = ppsum.tile([128, 352], F32, tag="pg352", name="psqb")
        proj_mm(ps_b, Q_OFF + 384, WALL_COLS, qt)

        nc.scalar.copy(qfv[:, :, 0:16],
                       ps_a[:, 0:192].rearrange("p (h c) -> p h c", c=16))
        g_sb = tmp.tile([128, 48], F32, tag="gsb", name="gsb")
        nc.scalar.activation(g_sb[:], ps_a[:, 192:240], AF.Sigmoid)
        qpts = tmp.tile([128, 288], BF16, tag="qpts", name="qpts")
        nc.vector.tensor_scalar_max(qpts[:, 0:144], ps_a[:, 240:384], 0.0)
        nc.vector.tensor_scalar_max(qpts[:, 144:288], ps_b[:, 0:144], 0.0)

        qpco = tmp.tile([128, 288], F32, tag="qpco", name="qpco")
        transform(qpts[:], qpco[:], qt, 48)
        qpco3 = qpco[:].rearrange("p (c h x) -> p c h x", c=6, x=4)

        gc = tmp.tile([128, 48], F32, tag="gc", name="gc")
        gd = tmp.tile([128, 48], F32, tag="gd", name="gd")
        nc.vector.tensor_tensor(gc[:], g_sb[:], qcst[:, 0:48], OP.mult)
        nc.vector.tensor_tensor(gd[:], g_sb[:], qcst[:, 48:96], OP.mult)
        gc3 = gc[:].rearrange("p (h x) -> p h x", x=4)
        gd3 = gd[:].rearrange("p (h x) -> p h x", x=4)
        for cc in range(3):
            nc.vector.tensor_tensor(qfv[:, :, 16 + cc * 4:20 + cc * 4],
                                    qpco3[:, cc], gc3, OP.mult)
            nc.gpsimd.tensor_tensor(qfv[:, :, 28 + cc * 4:32 + cc * 4],
                                    qpco3[:, 3 + cc], gd3, OP.mult)
        nc.vector.tensor_copy(qfv[:, :, 40], qcst[:, 96:108])

        # q2 from coord features
        sqs = tmp.tile([128, 144], F32, tag="sqs", name="sqs")
        nc.vector.tensor_tensor(
            sqs[:].rearrange("p (h x) -> p h x", x=12),
            qfv[:, :, 16:28], qfv[:, :, 16:28], OP.mult)
        q2s = tmp.tile([128, 12], F32, tag="q2s", name="q2s")
        nc.vector.tensor_reduce(
            q2s[:], sqs[:].rearrange("p (h c x) -> p h c x", c=3, x=4),
            AX.XY, OP.add)

        # curvature from dir features vs raw kd features of same rows
        qdv = qfv[:, :, 28:40]
        kdv = kds[qt][:].rearrange("p (h x) -> p h x", x=12)
        crs = tmp.tile([128, 144], F32, tag="crs", name="crs")
        t1 = tmp.tile([128, 48], F32, tag="t1", name="t1")
        t2 = tmp.tile([128, 48], F32, tag="t2", name="t2")
        t13 = t1[:].rearrange("p (h x) -> p h x", x=4)
        t23 = t2[:].rearrange("p (h x) -> p h x", x=4)
        for c, (a, b2) in enumerate(((1, 2), (2, 0), (0, 1))):
            nc.vector.tensor_tensor(t13, qdv[:, :, a * 4:a * 4 + 4],
                                    kdv[:, :, b2 * 4:b2 * 4 + 4], OP.mult)
            nc.gpsimd.tensor_tensor(t23, qdv[:, :, b2 * 4:b2 * 4 + 4],
                                    kdv[:, :, a * 4:a * 4 + 4], OP.mult)
            nc.vector.tensor_tensor(crs[:, c * 48:(c + 1) * 48], t1[:], t2[:],
                                    OP.subtract)
        nc.vector.tensor_tensor(crs[:], crs[:], crs[:], OP.mult)
        csum = tmp.tile([128, 48], F32, tag="csum", name="csum")
        nc.vector.tensor_reduce(
            csum[:], crs[:].rearrange("p (c x) -> p x c", c=3), AX.X, OP.add)
        # |qfd|^2, |kd|^2 per (h,p)
        sqd = tmp.tile([128, 144], F32, tag="sqd", name="sqd")
        nq2 = tmp.tile([128, 48], F32, tag="nq2", name="nq2")
        nk2 = tmp.tile([128, 48], F32, tag="nk2", name="nk2")
        nc.gpsimd.tensor_tensor(sqd[:].rearrange("p (h x) -> p h x", x=12),
                                qdv, qdv, OP.mult)
        nc.vector.tensor_reduce(
            nq2[:].rearrange("p (h x) -> p h x", x=4),
            sqd[:].rearrange("p (h c x) -> p h x c", c=3, x=4), AX.X, OP.add)
        nc.gpsimd.tensor_tensor(sqd[:].rearrange("p (h x) -> p h x", x=12),
                                kdv, kdv, OP.mult)
        nc.vector.tensor_reduce(
            nk2[:].rearrange("p (h x) -> p h x", x=4),
            sqd[:].rearrange("p (h c x) -> p h x c", c=3, x=4), AX.X, OP.add)
        nc.vector.tensor_tensor(nq2[:], nq2[:], nk2[:], OP.mult)
        nc.scalar.activation(nq2[:], nq2[:], AF.Sqrt)
        nc.vector.tensor_scalar_add(nq2[:], nq2[:], EPS)
        nc.vector.reciprocal(nq2[:], nq2[:])
        nc.scalar.activation(csum[:], csum[:], AF.Sqrt)
        nc.vector.tensor_tensor(csum[:], csum[:], nq2[:], OP.mult)
        curv = tmp.tile([128, 12], F32, tag="curv", name="curv")
        nc.vector.tensor_reduce(
            curv[:], csum[:].rearrange("p (h x) -> p h x", x=4), AX.X, OP.add)
        # qf[41] = c2*q2s + c3*curv
        nc.vector.tensor_tensor(q2s[:], q2s[:], qcst[:, 108:120], OP.mult)
        nc.vector.tensor_tensor(curv[:], curv[:], qcst[:, 120:132], OP.mult)
        nc.vector.tensor_tensor(qfv[:, :, 41], q2s[:], curv[:], OP.add)

    # ---- qfT: masked transposes (even head | odd head halves) -----------
    qfT = [work.tile([128, 2 * NB], BF16, name=f"qfT{t}") for t in range(6)]
    for t in range(6):
        nc.gpsimd.memset(qfT[t][:], 0.0)
    for t in range(6):
        for qt in range(2):
            tps = tpsum.tile([128, 384], F32, tag="tps")
            nc.tensor.transpose(tps[:, 0:128],
                                qf_sb[qt][:, t * 128:(t + 1) * 128], ident[:])
            eng = nc.scalar if (t + qt) % 2 else nc.vector
            eng_copy = eng.copy if eng is nc.scalar else eng.tensor_copy
            eng_copy(qfT[t][0:FS, qt * 128:(qt + 1) * 128], tps[0:FS, 0:128])
            eng2 = nc.vector if (t + qt) % 2 else nc.scalar
            eng2_copy = eng2.copy if eng2 is nc.scalar else eng2.tensor_copy
            eng2_copy(qfT[t][64:64 + FS, NB + qt * 128:NB + (qt + 1) * 128],
                      tps[64:64 + FS, 0:128])

    # ---- attention -------------------------------------------------------
    pre_ctx.close()
    att_ctx = ExitStack()
    apsum = att_ctx.enter_context(tc.tile_pool(name="apsum", bufs=2, space=PS))
    opsum = att_ctx.enter_context(tc.tile_pool(name="opsum", bufs=2, space=PS))
    otp = att_ctx.enter_context(tc.tile_pool(name="otp", bufs=2, space=PS))
    expT_tiles = [work.tile([128, 4096], BF16, name=f"expT{i}")
                  for i in range(3)]
    o_all = [work.tile([128, FEAT * H], F32, name=f"oall{qt}")
             for qt in range(2)]
    feats = [work.tile([128, FOUT], F32, name=f"feats{qt}") for qt in range(2)]
    ld_sb = [work.tile([128, 288], F32, name=f"ld{qt}") for qt in range(2)]
    RUNP = 2

    def emit_qk_exp(t):
        expT = expT_tiles[t % 3]
        for p4 in range(4):
            aps = apsum.tile([128, 1024], F32, tag="aps", name="aps")
            for j in range(2):
                kb = p4 * 2 + j
                nc.tensor.matmul(aps[:, j * 512:(j + 1) * 512],
                                 kfT3[:, t, kb * 128:(kb + 1) * 128],
                                 qfT[t][:, :], start=True, stop=True)
            nc.scalar.activation(expT[:, p4 * 1024:(p4 + 1) * 1024], aps[:],
                                 AF.Exp)

    def emit_av(h):
        t, e = h // 2, h % 2
        expT = expT_tiles[t % 3]
        ot_ps = opsum.tile([OCH, NB], F32, tag="ot", name="ot_ps")
        for kb in range(NKT):
            nc.tensor.matmul(
                ot_ps[:], vaG[kb][:, h * OCH:(h + 1) * OCH],
                expT[:, kb * 512 + e * NB:kb * 512 + (e + 1) * NB],
                start=(kb == 0), stop=(kb == NKT - 1))
        ot_sb = tmp.tile([OCH, NB], F32R, tag="otsb", name="otsb", bufs=2)
        nc.vector.tensor_copy(ot_sb[:], ot_ps[:])
        for qt in range(2):
            tp = otp.tile([128, OCH], F32R, tag="tp", name="tp")
            nc.tensor.transpose(tp[:], ot_sb[:, qt * 128:(qt + 1) * 128],
                                ident_r[:, :])
            rec = tmp.tile([128, 1], F32, tag="rec", name="rec", bufs=2)
            nc.vector.reciprocal(rec[:], tp[:, 64:65].bitcast(F32))
            nc.vector.tensor_scalar_mul(
                o_all[qt][:, h * FEAT:h * FEAT + 64], tp[:, 0:64].bitcast(F32),
                rec[:])

    def emit_inv_rot(qt, hh):
        """Rotate o_geom back to local frame for heads hh*6..hh*6+5."""
        hs = slice(hh * 6, hh * 6 + 6)
        ov = o_all[qt][:].rearrange("p (h f) -> p h f", f=FEAT)[:, hs]
        gv = feats[qt][:, 192:FOUT].rearrange(
            "p (h x c) -> p h x c", h=H, c=7)[:, hs]

        def og(j):
            return ov[:, :, 16 + 8 * j:24 + 8 * j]

        ogs = tmp.tile([128, 144], F32, tag="ogs", name="ogs", bufs=2)
        ogs3 = ogs[:].rearrange("p (c x) -> p c x", c=3)
        for j in range(3):
            nc.vector.tensor_scalar(
                ogs3[:, j].rearrange("p (h x) -> p h x", x=V), og(j),
                Tc(qt, j), None, OP.subtract)
        lci = tmp.tile([128, 48], F32, tag="lci", name="lci", bufs=2)
        for i in range(3):
            nc.vector.tensor_scalar_mul(lci[:], ogs3[:, 0], Rc(qt, i))
            nc.vector.scalar_tensor_tensor(lci[:], ogs3[:, 1],
                                           Rc(qt, 3 + i), lci[:],
                                           OP.mult, OP.add)
            nc.vector.scalar_tensor_tensor(
                gv[:, :, :, i], ogs3[:, 2].rearrange("p (h x) -> p h x", x=V),
                Rc(qt, 6 + i),
                lci[:].rearrange("p (h x) -> p h x", x=V), OP.mult, OP.add)

    def emit_inv_norm(qt):
        gv = feats[qt][:, 192:FOUT].rearrange("p (h x c) -> p h x c", h=H, c=7)
        # ld chains, full width (all 12 heads)
        ovd = o_all[qt][:].rearrange("p (h f) -> p h f", f=FEAT)
        for i in range(3):
            ldd = ld_sb[qt][:, i * 96:(i + 1) * 96]
            ldd3 = ldd.rearrange("p (h x) -> p h x", x=V)
            nc.vector.tensor_scalar_mul(ldd3, ovd[:, :, 40:48], Rc(qt, i))
            nc.vector.scalar_tensor_tensor(ldd3, ovd[:, :, 48:56],
                                           Rc(qt, 3 + i), ldd3,
                                           OP.mult, OP.add)
            nc.vector.scalar_tensor_tensor(ldd3, ovd[:, :, 56:64],
                                           Rc(qt, 6 + i), ldd3,
                                           OP.mult, OP.add)
        lsq = tmp.tile([128, 288], F32, tag="lsq", name="lsq")
        lsq4 = lsq[:].rearrange("p (h x c) -> p h x c", c=3, x=V)
        nc.vector.tensor_tensor(lsq4, gv[:, :, :, 0:3], gv[:, :, :, 0:3],
                                OP.mult)
        ncs = tmp.tile([128, 96], F32, tag="ncs", name="ncs")
        nc.vector.tensor_reduce(
            ncs[:], lsq[:].rearrange("p (x c) -> p x c", c=3), AX.X, OP.add)
        nc.scalar.activation(gv[:, :, :, 6],
                             ncs[:].rearrange("p (h x) -> p h x", x=V), AF.Sqrt)
        # ld normalization
        ldq = ld_sb[qt]
        nc.gpsimd.tensor_tensor(lsq[:], ldq[:], ldq[:], OP.mult)
        nds = tmp.tile([128, 96], F32, tag="nds", name="nds")
        nc.vector.tensor_reduce(
            nds[:], lsq[:].rearrange("p (c x) -> p x c", c=3), AX.X, OP.add)
        nc.scalar.activation(nds[:], nds[:], AF.Sqrt)
        nc.vector.tensor_scalar_max(nds[:], nds[:], EPS)
        nc.vector.reciprocal(nds[:], nds[:])
        nds3 = nds[:].rearrange("p (h x) -> p h x", x=V)
        for i in range(3):
            nc.gpsimd.tensor_tensor(
                gv[:, :, :, 3 + i],
                ldq[:, i * 96:(i + 1) * 96].rearrange("p (h x) -> p h x", x=V),
                nds3, OP.mult)
        nc.gpsimd.tensor_copy(
            feats[qt][:, 0:192].rearrange("p (h c) -> p h c", c=16),
            o_all[qt][:].rearrange("p (h f) -> p h f", f=FEAT)[:, :, 0:16])

    for t in range(6 + RUNP):
        if t < 6:
            emit_qk_exp(t)
        if t >= RUNP:
            emit_av(2 * (t - RUNP))
            emit_av(2 * (t - RUNP) + 1)
            if t - RUNP == 2:
                emit_inv_rot(0, 0)
                emit_inv_rot(1, 0)
    # ---- inverse norms + output projection, pipelined per query tile -----
    att_ctx.close()
    tpsum2 = ctx.enter_context(tc.tile_pool(name="tpsum2", bufs=2, space=PS))
    opsum2 = ctx.enter_context(tc.tile_pool(name="opsum2", bufs=2, space=PS))
    fT = []
    for kc in range(KCH):
        r0 = kc * 128
        rw = min(FOUT, r0 + 128) - r0
        pw = rw + 2 if kc == KCH - 1 else rw
        fT.append(work.tile([pw, NB], F32R, name=f"fT{kc}"))
    lastr = FOUT - (KCH - 1) * 128
    nc.gpsimd.tensor_copy(fT[KCH - 1][lastr:lastr + 2, :], ones2_f32[:])
    for qt in range(2):
        emit_inv_rot(qt, 1)
        emit_inv_norm(qt)
        for kc in range(KCH):
            r0 = kc * 128
            rw = min(FOUT, r0 + 128) - r0
            ps = tpsum2.tile([128, 128], F32, tag="tps2")
            nc.tensor.transpose(ps[:rw, :], feats[qt][:, r0:r0 + rw], ident[:])
            if kc % 2:
                nc.scalar.copy(fT[kc][:rw, qt * 128:(qt + 1) * 128], ps[:rw, :])
            else:
                nc.vector.tensor_copy(fT[kc][:rw, qt * 128:(qt + 1) * 128],
                                      ps[:rw, :])
        ps = opsum2.tile([128, CS], F32, tag="oproj")
        for kc in range(KCH):
            nc.tensor.matmul(ps[:], fT[kc][:, qt * 128:(qt + 1) * 128],
                             wout_sb[kc][:], start=(kc == 0),
                             stop=(kc == KCH - 1))
        osb = tmp.tile([128, CS], F32, tag="osb", name="osb")
        nc.scalar.copy(osb[:], ps[:])
        nc.sync.dma_start(out_loc[qt * 128:(qt + 1) * 128, :], osb[:])


def _run(inputs, trace=False):
    s, rt_all, wall, wout_b, qconst, has_bias = _host_prep(inputs)
    nc = _build_program(has_bias)
    in_maps = []
    for c in range(8):
        b, qb = c // 4, c % 4
        # rotate key rows so this core's queries are rows 0:256
        idx = np.r_[qb * NB:N, 0:qb * NB]
        in_maps.append({
            "s_all": np.ascontiguousarray(s[b][idx]),
            "rt_all": np.ascontiguousarray(rt_all[b][idx]),
            "wall": wall, "wout_b": wout_b, "qconst": qconst,
        })
    res = run_bass_kernel_spmd(nc, in_maps, list(range(8)), trace=trace)
    out = np.empty((B, N, CS), np.float32)
    for c in range(8):
        b, qb = c // 4, c % 4
        out[b, qb * NB:(qb + 1) * NB] = res.results[c]["out_loc"]
    return out, res


def kernel(**inputs):
    out, _ = _run(inputs, trace=False)
    return out


def kernel_traced(**inputs):
    return _run(inputs, trace=True)


# revision 58
# speedup vs baseline: 2.3136x; 1.0737x over previous
"""Bass/Tile TRN2 kernel for EnhancedIPA3 — collective-free redesign.

8 cores = batch(2) x query-block(4).  Each core redundantly computes the
K/V-side features for ALL 1024 keys of its batch (projections + rigid
frame transforms), then runs attention for its own 256 queries only.  No
inter-core communication: the collective-bootstrap barrier and the two
serialized AllGathers of the previous design are gone, and the cores are
fully independent, so cross-core launch skew no longer costs anything.

Key rows are rotated per core so the core's own query rows are always
tiles 0..1 (softmax over keys is permutation invariant) — one SPMD
program serves all 8 cores.

Self-contained: hardcodes all shapes; only depends on numpy + concourse.
"""

import numpy as np
from contextlib import ExitStack

import concourse.bass as bass
import concourse.bacc as bacc
import concourse.mybir as mybir
import concourse.tile as tile
from concourse.bass_utils import run_bass_kernel_spmd
from concourse.masks import make_identity

F32 = mybir.dt.float32
F32R = mybir.dt.float32r
BF16 = mybir.dt.bfloat16
AF = mybir.ActivationFunctionType
OP = mybir.AluOpType
AX = mybir.AxisListType

B, N, CS, H, C, P, V = 2, 1024, 384, 12, 16, 4, 8
EPS = 1e-8
NB = 256               # query rows per core (2 tiles)
NKT = 8                # key tiles of 128
# wall column map
K_OFF, V_OFF, PTS_OFF, Q_OFF, G_OFF, QPTS_OFF = 0, 192, 384, 1248, 1440, 1488
WALL_COLS = 1776
NPK = 12               # kv points per head (0:4 k_pts, 4:12 v_pts)
FEAT = 64              # per-head feature stride in kf/qf
FS = 42                # live features per head
OCH = 68               # va per-head stride: v16 | pts48 | ones | pad3
FOUT = H * (C + 7 * V)  # 864
KCH = 7                # output-proj contraction chunks


def _host_prep(inputs):
    wq = np.asarray(inputs["wq"], np.float32)
    bq = np.asarray(inputs["bq"], np.float32)
    wkv = np.asarray(inputs["wkv"], np.float32)
    bkv = np.asarray(inputs["bkv"], np.float32)
    wqp = np.asarray(inputs["wqp"], np.float32)
    bqp = np.asarray(inputs["bqp"], np.float32)
    wkvp = np.asarray(inputs["wkvp"], np.float32)
    bkvp = np.asarray(inputs["bkvp"], np.float32)
    wg = np.asarray(inputs["wg"], np.float32)
    bg = np.asarray(inputs["bg"], np.float32)
    gw = np.asarray(inputs["geom_weight"], np.float32)
    hw = np.asarray(inputs["head_weights"], np.float32)
    sh = 1.0 / (1.0 + np.exp(-hw))
    gw0, gw1 = float(gw[0]), float(gw[1])

    wall = np.zeros((CS + 2, WALL_COLS), np.float32)
    wall[:CS, K_OFF:K_OFF + 192] = wkv[:, :192]
    wall[CS, K_OFF:K_OFF + 192] = bkv[:192]
    wall[:CS, V_OFF:V_OFF + 192] = wkv[:, 192:]
    wall[CS, V_OFF:V_OFF + 192] = bkv[192:]
    # kv pts pair-planar: block 2j+t (j = input col 0..2, t: 0=coord 1=dir)
    # dst col = PTS_OFF + block*144 + h*12 + p  <-  src h*72 + p*6 + cc
    cc, h, p = np.meshgrid(np.arange(6), np.arange(H), np.arange(12),
                           indexing="ij")
    blk = np.where(cc < 3, 2 * cc, 2 * (cc - 3) + 1)
    dst = (PTS_OFF + blk * 144 + h * 12 + p).ravel()
    src = (h * 72 + p * 6 + cc).ravel()
    wall[:CS, dst] = wkvp[:, src]
    wall[CS, dst] = bkvp[src]
    # q scaled by sh/sqrt(C)
    qs = np.repeat(sh / np.sqrt(C), 16)
    wall[:CS, Q_OFF:Q_OFF + 192] = wq * qs[None, :]
    wall[CS, Q_OFF:Q_OFF + 192] = bq * qs
    wall[:CS, G_OFF:G_OFF + 48] = wg
    wall[CS, G_OFF:G_OFF + 48] = bg
    # q pts pair-planar: dst col = QPTS_OFF + blk*48 + h*4 + p
    cc, h, p = np.meshgrid(np.arange(6), np.arange(H), np.arange(P),
                           indexing="ij")
    blk = np.where(cc < 3, 2 * cc, 2 * (cc - 3) + 1)
    dst = (QPTS_OFF + blk * 48 + h * 4 + p).ravel()
    src = (h * 24 + p * 6 + cc).ravel()
    wall[:CS, dst] = wqp[:, src]
    wall[CS, dst] = bqp[src]
    has_bias = bool(np.any(wall[CS] != 0.0))
    wall[CS + 1] = wall[CS] * 0.5
    wall[CS] = wall[CS + 1]

    bout_half = np.asarray(inputs["bout"], np.float32)[None, :] * 0.5
    wout_b = np.concatenate(
        [np.asarray(inputs["wout"], np.float32), bout_half, bout_half], axis=0)

    # on-chip constants (broadcast to 128 partitions by a rank-1 matmul)
    qconst = np.zeros((1, 144), np.float32)
    SC = gw0 * 0.5 * sh            # coord feature scale (with gate)
    DC = gw1 * sh                  # dir feature scale (with gate)
    qconst[0, 0:48] = np.repeat(SC, P)
    qconst[0, 48:96] = np.repeat(DC, P)
    qconst[0, 96:108] = sh * gw0 / P                      # qf[40]
    c2 = np.where(np.abs(gw0 * sh) > 1e-12, -1.0 / (gw0 * sh + 1e-30), 0.0)
    qconst[0, 108:120] = c2                               # q2 coefficient
    qconst[0, 120:132] = -sh * gw1                        # curvature coeff

    rot9 = np.asarray(inputs["rot"], np.float32).reshape(B, N, 9)
    trans = np.asarray(inputs["trans"], np.float32)
    rt_all = np.ascontiguousarray(np.concatenate([rot9, trans], axis=2))
    s = np.asarray(inputs["s"], np.float32)
    return s, rt_all, wall, wout_b, qconst, has_bias


_PROGRAM_CACHE = {}


def _build_program(has_bias):
    key = (bool(has_bias),)
    if key in _PROGRAM_CACHE:
        return _PROGRAM_CACHE[key]
    nc = bacc.Bacc("TRN2", target_bir_lowering=False, debug=False,
                   num_devices=8)
    s_all = nc.dram_tensor("s_all", [N, CS], F32, kind="ExternalInput")
    rt_d = nc.dram_tensor("rt_all", [N, 12], F32, kind="ExternalInput")
    wall_d = nc.dram_tensor("wall", [CS + 2, WALL_COLS], F32,
                            kind="ExternalInput")
    wout_d = nc.dram_tensor("wout_b", [FOUT + 2, CS], F32,
                            kind="ExternalInput")
    qconst_d = nc.dram_tensor("qconst", [1, 144], F32, kind="ExternalInput")
    out_loc = nc.dram_tensor("out_loc", [NB, CS], F32, kind="ExternalOutput")

    with tile.TileContext(nc) as tc:
        with ExitStack() as ctx:
            _emit(ctx, tc, nc, s_all, rt_d, wall_d, wout_d,
                  qconst_d, out_loc, has_bias)
    nc.compile()
    _PROGRAM_CACHE[key] = nc
    return nc


def _emit(ctx, tc, nc, s_all, rt_d, wall_d, wout_d, qconst_d,
          out_loc, has_bias):
    PS = bass.MemorySpace.PSUM

    const = ctx.enter_context(tc.tile_pool(name="const", bufs=1))
    work = ctx.enter_context(tc.tile_pool(name="work", bufs=1))
    tmp = ctx.enter_context(tc.tile_pool(name="tmp", bufs=2))
    pre_ctx = ExitStack()
    pA = pre_ctx.enter_context(tc.tile_pool(name="pA", bufs=1))
    kio = pre_ctx.enter_context(tc.tile_pool(name="kio", bufs=2))
    ppsum = pre_ctx.enter_context(tc.tile_pool(name="ppsum", bufs=2, space=PS))
    tpsum = pre_ctx.enter_context(tc.tile_pool(name="tpsum", bufs=2, space=PS))

    # ---- constants -------------------------------------------------------
    ident = const.tile([128, 128], F32)
    make_identity(nc, ident[:])
    ident_r = const.tile([OCH, OCH], F32R)
    nc.gpsimd.tensor_copy(ident_r[:], ident[0:OCH, 0:OCH])
    ones2_f32 = const.tile([2, NB], F32)
    nc.gpsimd.memset(ones2_f32[:], 1.0)
    ones_r = const.tile([2, 128], F32R)
    nc.gpsimd.tensor_copy(ones_r[:], ones2_f32[:, 0:128])

    # ---- DMAs (s first: the transposes+projections are the critical path;
    # issue on multiple engine queues to parallelize descriptor setup)
    s_sb = []
    for kt in range(NKT):
        t = pA.tile([128, CS], F32, name=f"s{kt}")
        nc.sync.dma_start(t[:], s_all[kt * 128:(kt + 1) * 128, :])
        s_sb.append(t)
    # all rot+trans rows in one DMA: rt_sb[:, kt*12+c] = rt_all[kt*128+p, c]
    rt_sb = const.tile([128, 96], F32, name="rt_sb")
    nc.gpsimd.dma_start(rt_sb[:],
                        rt_d[:, :].rearrange("(t p) c -> p t c", p=128))
    rtb_sb = const.tile([128, 96], BF16, name="rtb_sb")
    nc.gpsimd.tensor_copy(rtb_sb[:], rt_sb[:])

    def Rc(kt, j, b=False):
        t = rtb_sb if b else rt_sb
        return t[:, kt * 12 + j:kt * 12 + j + 1]

    def Tc(kt, j, b=False):
        t = rtb_sb if b else rt_sb
        return t[:, kt * 12 + 9 + j:kt * 12 + 9 + j + 1]

    wall_sb = []
    for kc in range(3):
        t = pA.tile([128, WALL_COLS], F32R, name=f"wall{kc}")
        nc.sync.dma_start(t[:], wall_d[kc * 128:(kc + 1) * 128, :].bitcast(F32R))
        wall_sb.append(t)
    wall_bias = pA.tile([2, WALL_COLS], F32R)
    if has_bias:
        nc.sync.dma_start(wall_bias[:], wall_d[CS:CS + 2, :].bitcast(F32R))

    qconst_sb = const.tile([1, 144], F32R)
    nc.gpsimd.dma_start(qconst_sb[:], qconst_d[:, :].bitcast(F32R))

    wout_sb = []
    for kc in range(KCH):
        r0 = kc * 128
        r1 = min(FOUT + 2, r0 + 128)
        t = const.tile([r1 - r0, CS], F32R, name=f"wout{kc}")
        wout_sb.append(t)

    def emit_wout_dmas():
        for kc in range(KCH):
            r0 = kc * 128
            r1 = min(FOUT + 2, r0 + 128)
            nc.sync.dma_start(wout_sb[kc][:], wout_d[r0:r1, :].bitcast(F32R))

    # ---- sT (transpose all of s) ----------------------------------------
    sT = pA.tile([128, 3 * N], F32R, name="sT")   # [:, kc*1024 + key]
    sT3 = sT[:].rearrange("p (c k) -> p c k", k=N)
    for kt in range(NKT):
        tps = tpsum.tile([128, 384], F32, tag="tps")
        for kc in range(3):
            nc.tensor.transpose(tps[:, kc * 128:(kc + 1) * 128],
                                s_sb[kt][:, kc * 128:(kc + 1) * 128], ident[:])
        dst = sT3[:, :, kt * 128:(kt + 1) * 128]
        src = tps[:].rearrange("p (c k) -> p c k", k=128)
        if kt % 2:
            nc.scalar.copy(dst, src)
        else:
            nc.vector.tensor_copy(dst, src)

    # ---- broadcast qconst row to 128 partitions --------------------------
    tps = tpsum.tile([128, 384], F32, tag="tps")
    nc.tensor.matmul(tps[:, 0:144], ones_r[0:1, :], qconst_sb[:, :],
                     start=True, stop=True)
    qcst = const.tile([128, 144], F32)
    nc.vector.tensor_copy(qcst[:], tps[:, 0:144])
    # slices: SC48 0:48 | DC48 48:96 | A12 96:108 | c2 108:120 | c3 120:132

    # ---- K/V side: all 8 key tiles --------------------------------------
    kfT = work.tile([128, 6 * N], BF16, name="kfT")   # [:, t*1024 + key]
    kfT3 = kfT[:].rearrange("p (t k) -> p t k", k=N)
    vaG = [work.tile([128, H * OCH], BF16, name=f"vaG{kb}")
           for kb in range(NKT)]
    kds = [work.tile([128, 144], F32, name=f"kds{qt}") for qt in range(2)]

    GROUPS_K = [(0, 384), (384, 896), (896, 1248)]

    def proj_mm(ps, c0, c1, kt):
        pv = ps[:, 0:c1 - c0]
        for kc in range(3):
            last = (kc == 2) and not has_bias
            nc.tensor.matmul(pv, sT3[:, kc, kt * 128:(kt + 1) * 128],
                             wall_sb[kc][:, c0:c1], start=(kc == 0), stop=last)
        if has_bias:
            nc.tensor.matmul(pv, ones_r[:, :], wall_bias[:, c0:c1],
                             start=False, stop=True)

    def transform(pts, pco, kt, W):
        """Rigid transform, pair-planar [dc_j|dd_j] blocks of 2W.

        The coord and dir chains for output comp i share the same rotation
        column, so each chain step runs once on the fused [128, 2W] pair.
        """
        W2 = 2 * W
        for i in range(3):
            dco = pco[:, i * W2:(i + 1) * W2]
            nc.scalar.activation(dco, pts[:, 0:W2], AF.Copy,
                                 scale=Rc(kt, 3 * i))
            nc.vector.scalar_tensor_tensor(dco, pts[:, W2:2 * W2],
                                           Rc(kt, 3 * i + 1, True), dco,
                                           OP.mult, OP.add)
            nc.vector.scalar_tensor_tensor(dco, pts[:, 2 * W2:3 * W2],
                                           Rc(kt, 3 * i + 2, True), dco,
                                           OP.mult, OP.add)
            # + translation on the coord half only
            nc.scalar.activation(pco[:, i * W2:i * W2 + W],
                                 pco[:, i * W2:i * W2 + W], AF.Identity,
                                 bias=Tc(kt, i))

    for kt in range(NKT):
        # projections: K+V | pts-a | pts-b
        ps_kv = ppsum.tile([128, 384], F32, tag="pg384", name="pskv")
        proj_mm(ps_kv, 0, 384, kt)
        ps_p1 = ppsum.tile([128, 512], F32, tag="pg512", name="psp1")
        proj_mm(ps_p1, 384, 896, kt)
        ps_p2 = ppsum.tile([128, 352], F32, tag="pg352", name="psp2")
        proj_mm(ps_p2, 896, 1248, kt)

        kf = kio.tile([128, H * FEAT], F32, tag="kf", name="kf", bufs=3)
        kfv = kf[:].rearrange("p (h f) -> p h f", f=FEAT)
        if kt < 2:
            # zero the pad cols 42:64 of this physical buffer once
            nc.gpsimd.memset(kfv[:, :, 42:64], 0.0)
        va = vaG[kt]
        vav = va[:].rearrange("p (h f) -> p h f", f=OCH)
        pts = kio.tile([128, 864], BF16, tag="pts", name="pts", bufs=3)

        # evacuations
        nc.scalar.copy(kfv[:, :, 0:16],
                       ps_kv[:, 0:192].rearrange("p (h c) -> p h c", c=16))
        nc.vector.tensor_copy(vav[:, :, 0:16],
                              ps_kv[:, 192:384].rearrange("p (h c) -> p h c", c=16))
        nc.vector.tensor_scalar_max(pts[:, 0:512], ps_p1[:], 0.0)
        nc.scalar.activation(pts[:, 512:864], ps_p2[:], AF.Relu)

        # rigid transform (planar, bf16)
        pco = kio.tile([128, 864], BF16, tag="pco", name="pco", bufs=3)
        transform(pts[:], pco[:], kt, 144)
        pco5 = pco[:].rearrange("p (j t h x) -> p j t h x", j=3, t=2, x=NPK)
        pco3 = pco[:].rearrange("p (c h x) -> p c h x", c=6, x=NPK)

        # kf coord/dir features ([cc*4+p] per head) + va pts (fused copies)
        nc.gpsimd.tensor_copy(
            kfv[:, :, 16:28].rearrange("p h (c x) -> p c h x", c=3),
            pco5[:, :, 0, :, 0:4])
        nc.gpsimd.tensor_copy(
            kfv[:, :, 28:40].rearrange("p h (c x) -> p c h x", c=3),
            pco5[:, :, 1, :, 0:4])
        nc.vector.tensor_copy(
            vav[:, :, 16:64].rearrange("p h (c x) -> p c h x", c=6),
            pco3[:, :, :, 4:12])
        nc.gpsimd.memset(vav[:, :, 64:65], 1.0)
        nc.gpsimd.memset(vav[:, :, 65:68], 0.0)

        # k2 (negated sum of squared coord features)
        sqs = tmp.tile([128, 144], F32, tag="sqs", name="sqs")
        nc.vector.tensor_tensor(
            sqs[:].rearrange("p (h x) -> p h x", x=12),
            kfv[:, :, 16:28], kfv[:, :, 16:28], OP.mult)
        k2 = tmp.tile([128, 12], F32, tag="k2", name="k2")
        nc.vector.tensor_reduce(
            k2[:], sqs[:].rearrange("p (h c x) -> p h c x", c=3, x=4),
            AX.XY, OP.add, negate=True)
        nc.vector.tensor_copy(kfv[:, :, 40], k2[:])
        nc.gpsimd.memset(kfv[:, :, 41], 1.0)
        if kt < 2:
            nc.gpsimd.tensor_copy(kds[kt][:].rearrange("p (h x) -> p h x", x=12),
                                  kfv[:, :, 28:40])

        # transpose kf -> kfT (2 head-pairs per psum tile)
        for t0 in range(0, 6, 2):
            tps = tpsum.tile([128, 384], F32, tag="tps")
            nc.tensor.transpose(tps[:, 0:128],
                                kf[:, t0 * 128:(t0 + 1) * 128], ident[:])
            nc.tensor.transpose(tps[:, 128:256],
                                kf[:, (t0 + 1) * 128:(t0 + 2) * 128], ident[:])
            dst = kfT3[:, t0:t0 + 2, kt * 128:(kt + 1) * 128]
            src = tps[:, 0:256].rearrange("p (t k) -> p t k", k=128)
            if t0 == 2:
                nc.scalar.copy(dst, src)
            else:
                nc.vector.tensor_copy(dst, src)

    # ---- Q side (own rows = tiles 0..1) ---------------------------------
    qf_sb = [work.tile([128, H * FEAT], F32, name=f"qf{qt}") for qt in range(2)]
    for qt in range(2):
        qf = qf_sb[qt]
        qfv = qf[:].rearrange("p (h f) -> p h f", f=FEAT)
        ps_a = ppsum.tile([128, 384], F32, tag="pg384", name="psqa")
        proj_mm(ps_a, Q_OFF, Q_OFF + 384, qt)
        ps_b = ppsum.tile([128, 352], F32, tag="pg352", name="psqb")
        proj_mm(ps_b, Q_OFF + 384, WALL_COLS, qt)

        nc.scalar.copy(qfv[:, :, 0:16],
                       ps_a[:, 0:192].rearrange("p (h c) -> p h c", c=16))
        g_sb = tmp.tile([128, 48], F32, tag="gsb", name="gsb")
        nc.scalar.activation(g_sb[:], ps_a[:, 192:240], AF.Sigmoid)
        qpts = tmp.tile([128, 288], BF16, tag="qpts", name="qpts")
        nc.vector.tensor_scalar_max(qpts[:, 0:144], ps_a[:, 240:384], 0.0)
        nc.vector.tensor_scalar_max(qpts[:, 144:288], ps_b[:, 0:144], 0.0)

        qpco = tmp.tile([128, 288], F32, tag="qpco", name="qpco")
        transform(qpts[:], qpco[:], qt, 48)
        qpco5 = qpco[:].rearrange("p (j t h x) -> p j t h x", j=3, t=2, x=4)

        gc = tmp.tile([128, 48], F32, tag="gc", name="gc")
        gd = tmp.tile([128, 48], F32, tag="gd", name="gd")
        nc.vector.tensor_tensor(gc[:], g_sb[:], qcst[:, 0:48], OP.mult)
        nc.vector.tensor_tensor(gd[:], g_sb[:], qcst[:, 48:96], OP.mult)
        gc3 = gc[:].rearrange("p (h x) -> p h x", x=4)
        gd3 = gd[:].rearrange("p (h x) -> p h x", x=4)
        for cc in range(3):
            nc.vector.tensor_tensor(qfv[:, :, 16 + cc * 4:20 + cc * 4],
                                    qpco5[:, cc, 0], gc3, OP.mult)
            nc.gpsimd.tensor_tensor(qfv[:, :, 28 + cc * 4:32 + cc * 4],
                                    qpco5[:, cc, 1], gd3, OP.mult)
        nc.vector.tensor_copy(qfv[:, :, 40], qcst[:, 96:108])

        # q2 from coord features
        sqs = tmp.tile([128, 144], F32, tag="sqs", name="sqs")
        nc.vector.tensor_tensor(
            sqs[:].rearrange("p (h x) -> p h x", x=12),
            qfv[:, :, 16:28], qfv[:, :, 16:28], OP.mult)
        q2s = tmp.tile([128, 12], F32, tag="q2s", name="q2s")
        nc.vector.tensor_reduce(
            q2s[:], sqs[:].rearrange("p (h c x) -> p h c x", c=3, x=4),
            AX.XY, OP.add)

        # curvature from dir features vs raw kd features of same rows
        qdv = qfv[:, :, 28:40]
        kdv = kds[qt][:].rearrange("p (h x) -> p h x", x=12)
        crs = tmp.tile([128, 144], F32, tag="crs", name="crs")
        t1 = tmp.tile([128, 48], F32, tag="t1", name="t1")
        t2 = tmp.tile([128, 48], F32, tag="t2", name="t2")
        t13 = t1[:].rearrange("p (h x) -> p h x", x=4)
        t23 = t2[:].rearrange("p (h x) -> p h x", x=4)
        for c, (a, b2) in enumerate(((1, 2), (2, 0), (0, 1))):
            nc.vector.tensor_tensor(t13, qdv[:, :, a * 4:a * 4 + 4],
                                    kdv[:, :, b2 * 4:b2 * 4 + 4], OP.mult)
            nc.gpsimd.tensor_tensor(t23, qdv[:, :, b2 * 4:b2 * 4 + 4],
                                    kdv[:, :, a * 4:a * 4 + 4], OP.mult)
            nc.vector.tensor_tensor(crs[:, c * 48:(c + 1) * 48], t1[:], t2[:],
                                    OP.subtract)
        nc.vector.tensor_tensor(crs[:], crs[:], crs[:], OP.mult)
        csum = tmp.tile([128, 48], F32, tag="csum", name="csum")
        nc.vector.tensor_reduce(
            csum[:], crs[:].rearrange("p (c x) -> p x c", c=3), AX.X, OP.add)
        # |qfd|^2, |kd|^2 per (h,p)
        sqd = tmp.tile([128, 144], F32, tag="sqd", name="sqd")
        nq2 = tmp.tile([128, 48], F32, tag="nq2", name="nq2")
        nk2 = tmp.tile([128, 48], F32, tag="nk2", name="nk2")
        nc.gpsimd.tensor_tensor(sqd[:].rearrange("p (h x) -> p h x", x=12),
                                qdv, qdv, OP.mult)
        nc.vector.tensor_reduce(
            nq2[:].rearrange("p (h x) -> p h x", x=4),
            sqd[:].rearrange("p (h c x) -> p h x c", c=3, x=4), AX.X, OP.add)
        nc.gpsimd.tensor_tensor(sqd[:].rearrange("p (h x) -> p h x", x=12),
                                kdv, kdv, OP.mult)
        nc.vector.tensor_reduce(
            nk2[:].rearrange("p (h x) -> p h x", x=4),
            sqd[:].rearrange("p (h c x) -> p h x c", c=3, x=4), AX.X, OP.add)
        nc.vector.tensor_tensor(nq2[:], nq2[:], nk2[:], OP.mult)
        nc.scalar.activation(nq2[:], nq2[:], AF.Sqrt)
        nc.vector.tensor_scalar_add(nq2[:], nq2[:], EPS)
        nc.vector.reciprocal(nq2[:], nq2[:])
        nc.scalar.activation(csum[:], csum[:], AF.Sqrt)
        nc.vector.tensor_tensor(csum[:], csum[:], nq2[:], OP.mult)
        curv = tmp.tile([128, 12], F32, tag="curv", name="curv")
        nc.vector.tensor_reduce(
            curv[:], csum[:].rearrange("p (h x) -> p h x", x=4), AX.X, OP.add)
        # qf[41] = c2*q2s + c3*curv
        nc.vector.tensor_tensor(q2s[:], q2s[:], qcst[:, 108:120], OP.mult)
        nc.vector.tensor_tensor(curv[:], curv[:], qcst[:, 120:132], OP.mult)
        nc.vector.tensor_tensor(qfv[:, :, 41], q2s[:], curv[:], OP.add)

    # ---- qfT: masked transposes (even head | odd head halves) -----------
    qfT = [work.tile([128, 2 * NB], BF16, name=f"qfT{t}") for t in range(6)]
    for t in range(6):
        nc.gpsimd.memset(qfT[t][:], 0.0)
    for t in range(6):
        for qt in range(2):
            tps = tpsum.tile([128, 384], F32, tag="tps")
            nc.tensor.transpose(tps[:, 0:128],
                                qf_sb[qt][:, t * 128:(t + 1) * 128], ident[:])
            eng = nc.scalar if (t + qt) % 2 else nc.vector
            eng_copy = eng.copy if eng is nc.scalar else eng.tensor_copy
            eng_copy(qfT[t][0:FS, qt * 128:(qt + 1) * 128], tps[0:FS, 0:128])
            eng2 = nc.vector if (t + qt) % 2 else nc.scalar
            eng2_copy = eng2.copy if eng2 is nc.scalar else eng2.tensor_copy
            eng2_copy(qfT[t][64:64 + FS, NB + qt * 128:NB + (qt + 1) * 128],
                      tps[64:64 + FS, 0:128])

    # ---- attention -------------------------------------------------------
    emit_wout_dmas()
    pre_ctx.close()
    att_ctx = ExitStack()
    apsum = att_ctx.enter_context(tc.tile_pool(name="apsum", bufs=2, space=PS))
    opsum = att_ctx.enter_context(tc.tile_pool(name="opsum", bufs=2, space=PS))
    otp = att_ctx.enter_context(tc.tile_pool(name="otp", bufs=2, space=PS))
    expT_tiles = [work.tile([128, 4096], BF16, name=f"expT{i}")
                  for i in range(3)]
    o_all = [work.tile([128, FEAT * H], F32, name=f"oall{qt}")
             for qt in range(2)]
    feats = [work.tile([128, FOUT], F32, name=f"feats{qt}") for qt in range(2)]
    ld_sb = [work.tile([128, 288], F32, name=f"ld{qt}") for qt in range(2)]
    RUNP = 2

    def emit_qk_exp(t):
        expT = expT_tiles[t % 3]
        for p4 in range(4):
            aps = apsum.tile([128, 1024], F32, tag="aps", name="aps")
            for j in range(2):
                kb = p4 * 2 + j
                nc.tensor.matmul(aps[:, j * 512:(j + 1) * 512],
                                 kfT3[:, t, kb * 128:(kb + 1) * 128],
                                 qfT[t][:, :], start=True, stop=True)
            nc.scalar.activation(expT[:, p4 * 1024:(p4 + 1) * 1024], aps[:],
                                 AF.Exp)

    def emit_av(h):
        t, e = h // 2, h % 2
        expT = expT_tiles[t % 3]
        ot_ps = opsum.tile([OCH, NB], F32, tag="ot", name="ot_ps")
        for kb in range(NKT):
            nc.tensor.matmul(
                ot_ps[:], vaG[kb][:, h * OCH:(h + 1) * OCH],
                expT[:, kb * 512 + e * NB:kb * 512 + (e + 1) * NB],
                start=(kb == 0), stop=(kb == NKT - 1))
        ot_sb = tmp.tile([OCH, NB], F32R, tag="otsb", name="otsb", bufs=2)
        nc.vector.tensor_copy(ot_sb[:], ot_ps[:])
        for qt in range(2):
            tp = otp.tile([128, OCH], F32R, tag="tp", name="tp")
            nc.tensor.transpose(tp[:], ot_sb[:, qt * 128:(qt + 1) * 128],
                                ident_r[:, :])
            rec = tmp.tile([128, 1], F32, tag="rec", name="rec", bufs=2)
            nc.vector.reciprocal(rec[:], tp[:, 64:65].bitcast(F32))
            nc.vector.tensor_scalar_mul(
                o_all[qt][:, h * FEAT:h * FEAT + 64], tp[:, 0:64].bitcast(F32),
                rec[:])

    def emit_inv_rot(qt, hh):
        """Rotate o_geom back to local frame for heads hh*6..hh*6+5."""
        hs = slice(hh * 6, hh * 6 + 6)
        ov = o_all[qt][:].rearrange("p (h f) -> p h f", f=FEAT)[:, hs]
        gv = feats[qt][:, 192:FOUT].rearrange(
            "p (h x c) -> p h x c", h=H, c=7)[:, hs]

        def og(j):
            return ov[:, :, 16 + 16 * j:24 + 16 * j]

        ogs = tmp.tile([128, 144], F32, tag="ogs", name="ogs", bufs=2)
        ogs3 = ogs[:].rearrange("p (c x) -> p c x", c=3)
        for j in range(3):
            nc.vector.tensor_scalar(
                ogs3[:, j].rearrange("p (h x) -> p h x", x=V), og(j),
                Tc(qt, j), None, OP.subtract)
        lci = tmp.tile([128, 48], F32, tag="lci", name="lci", bufs=2)
        for i in range(3):
            nc.vector.tensor_scalar_mul(lci[:], ogs3[:, 0], Rc(qt, i))
            nc.vector.scalar_tensor_tensor(lci[:], ogs3[:, 1],
                                           Rc(qt, 3 + i), lci[:],
                                           OP.mult, OP.add)
            nc.vector.scalar_tensor_tensor(
                gv[:, :, :, i], ogs3[:, 2].rearrange("p (h x) -> p h x", x=V),
                Rc(qt, 6 + i),
                lci[:].rearrange("p (h x) -> p h x", x=V), OP.mult, OP.add)

    def emit_inv_norm(qt):
        gv = feats[qt][:, 192:FOUT].rearrange("p (h x c) -> p h x c", h=H, c=7)
        # ld chains, full width (all 12 heads)
        ovd = o_all[qt][:].rearrange("p (h f) -> p h f", f=FEAT)
        for i in range(3):
            ldd = ld_sb[qt][:, i * 96:(i + 1) * 96]
            ldd3 = ldd.rearrange("p (h x) -> p h x", x=V)
            nc.vector.tensor_scalar_mul(ldd3, ovd[:, :, 24:32], Rc(qt, i))
            nc.vector.scalar_tensor_tensor(ldd3, ovd[:, :, 40:48],
                                           Rc(qt, 3 + i), ldd3,
                                           OP.mult, OP.add)
            nc.vector.scalar_tensor_tensor(ldd3, ovd[:, :, 56:64],
                                           Rc(qt, 6 + i), ldd3,
                                           OP.mult, OP.add)
        lsq = tmp.tile([128, 288], F32, tag="lsq", name="lsq")
        lsq4 = lsq[:].rearrange("p (h x c) -> p h x c", c=3, x=V)
        nc.vector.tensor_tensor(lsq4, gv[:, :, :, 0:3], gv[:, :, :, 0:3],
                                OP.mult)
        ncs = tmp.tile([128, 96], F32, tag="ncs", name="ncs")
        nc.vector.tensor_reduce(
            ncs[:], lsq[:].rearrange("p (x c) -> p x c", c=3), AX.X, OP.add)
        nc.scalar.activation(gv[:, :, :, 6],
                             ncs[:].rearrange("p (h x) -> p h x", x=V), AF.Sqrt)
        # ld normalization
        ldq = ld_sb[qt]
        nc.gpsimd.tensor_tensor(lsq[:], ldq[:], ldq[:], OP.mult)
        nds = tmp.tile([128, 96], F32, tag="nds", name="nds")
        nc.vector.tensor_reduce(
            nds[:], lsq[:].rearrange("p (c x) -> p x c", c=3), AX.X, OP.add)
        nc.scalar.activation(nds[:], nds[:], AF.Sqrt)
        nc.vector.tensor_scalar_max(nds[:], nds[:], EPS)
        nc.vector.reciprocal(nds[:], nds[:])
        nds3 = nds[:].rearrange("p (h x) -> p h x", x=V)
        for i in range(3):
            nc.gpsimd.tensor_tensor(
                gv[:, :, :, 3 + i],
                ldq[:, i * 96:(i + 1) * 96].rearrange("p (h x) -> p h x", x=V),
                nds3, OP.mult)
        nc.gpsimd.tensor_copy(
            feats[qt][:, 0:192].rearrange("p (h c) -> p h c", c=16),
            o_all[qt][:].rearrange("p (h f) -> p h f", f=FEAT)[:, :, 0:16])

    for t in range(6 + RUNP):
        if t < 6:
            emit_qk_exp(t)
        if t >= RUNP:
            emit_av(2 * (t - RUNP))
            emit_av(2 * (t - RUNP) + 1)
            if t - RUNP == 2:
                emit_inv_rot(0, 0)
                emit_inv_rot(1, 0)
    # ---- inverse norms + output projection, pipelined per query tile -----
    att_ctx.close()
    tpsum2 = ctx.enter_context(tc.tile_pool(name="tpsum2", bufs=2, space=PS))
    opsum2 = ctx.enter_context(tc.tile_pool(name="opsum2", bufs=2, space=PS))
    fT = []
    for kc in range(KCH):
        r0 = kc * 128
        rw = min(FOUT, r0 + 128) - r0
        pw = rw + 2 if kc == KCH - 1 else rw
        fT.append(work.tile([pw, NB], F32R, name=f"fT{kc}"))
    lastr = FOUT - (KCH - 1) * 128
    nc.gpsimd.tensor_copy(fT[KCH - 1][lastr:lastr + 2, :], ones2_f32[:])
    for qt in range(2):
        emit_inv_rot(qt, 1)
        emit_inv_norm(qt)
        for kc in range(KCH):
            r0 = kc * 128
            rw = min(FOUT, r0 + 128) - r0
            ps = tpsum2.tile([128, 128], F32, tag="tps2")
            nc.tensor.transpose(ps[:rw, :], feats[qt][:, r0:r0 + rw], ident[:])
            if kc % 2:
                nc.scalar.copy(fT[kc][:rw, qt * 128:(qt + 1) * 128], ps[:rw, :])
            else:
                nc.vector.tensor_copy(fT[kc][:rw, qt * 128:(qt + 1) * 128],
                                      ps[:rw, :])
        ps = opsum2.tile([128, CS], F32, tag="oproj")
        for kc in range(KCH):
            nc.tensor.matmul(ps[:], fT[kc][:, qt * 128:(qt + 1) * 128],
                             wout_sb[kc][:], start=(kc == 0),
                             stop=(kc == KCH - 1))
        osb = tmp.tile([128, CS], F32, tag="osb", name="osb")
        nc.scalar.copy(osb[:], ps[:])
        nc.sync.dma_start(out_loc[qt * 128:(qt + 1) * 128, :], osb[:])


def _run(inputs, trace=False):
    s, rt_all, wall, wout_b, qconst, has_bias = _host_prep(inputs)
    nc = _build_program(has_bias)
    in_maps = []
    for c in range(8):
        b, qb = c // 4, c % 4
        # rotate key rows so this core's queries are rows 0:256
        idx = np.r_[qb * NB:N, 0:qb * NB]
        in_maps.append({
            "s_all": np.ascontiguousarray(s[b][idx]),
            "rt_all": np.ascontiguousarray(rt_all[b][idx]),
            "wall": wall, "wout_b": wout_b, "qconst": qconst,
        })
    res = run_bass_kernel_spmd(nc, in_maps, list(range(8)), trace=trace)
    out = np.empty((B, N, CS), np.float32)
    for c in range(8):
        b, qb = c // 4, c % 4
        out[b, qb * NB:(qb + 1) * NB] = res.results[c]["out_loc"]
    return out, res


def kernel(**inputs):
    out, _ = _run(inputs, trace=False)
    return out


def kernel_traced(**inputs):
    return _run(inputs, trace=True)


# revision 61
# speedup vs baseline: 2.3407x; 1.0117x over previous
"""Bass/Tile TRN2 kernel for EnhancedIPA3 — collective-free redesign.

8 cores = batch(2) x query-block(4).  Each core redundantly computes the
K/V-side features for ALL 1024 keys of its batch (projections + rigid
frame transforms), then runs attention for its own 256 queries only.  No
inter-core communication: the collective-bootstrap barrier and the two
serialized AllGathers of the previous design are gone, and the cores are
fully independent, so cross-core launch skew no longer costs anything.

Key rows are rotated per core so the core's own query rows are always
tiles 0..1 (softmax over keys is permutation invariant) — one SPMD
program serves all 8 cores.

Self-contained: hardcodes all shapes; only depends on numpy + concourse.
"""

import numpy as np
from contextlib import ExitStack

import concourse.bass as bass
import concourse.bacc as bacc
import concourse.mybir as mybir
import concourse.tile as tile
from concourse.bass_utils import run_bass_kernel_spmd
from concourse.masks import make_identity

F32 = mybir.dt.float32
F32R = mybir.dt.float32r
BF16 = mybir.dt.bfloat16
AF = mybir.ActivationFunctionType
OP = mybir.AluOpType
AX = mybir.AxisListType

B, N, CS, H, C, P, V = 2, 1024, 384, 12, 16, 4, 8
EPS = 1e-8
NB = 256               # query rows per core (2 tiles)
NKT = 8                # key tiles of 128
# wall column map
K_OFF, V_OFF, PTS_OFF, Q_OFF, G_OFF, QPTS_OFF = 0, 192, 384, 1248, 1440, 1488
WALL_COLS = 1776
NPK = 12               # kv points per head (0:4 k_pts, 4:12 v_pts)
FEAT = 64              # per-head feature stride in kf/qf
FS = 42                # live features per head
OCH = 68               # va per-head stride: v16 | pts48 | ones | pad3
FOUT = H * (C + 7 * V)  # 864
KCH = 7                # output-proj contraction chunks


def _host_prep(inputs):
    wq = np.asarray(inputs["wq"], np.float32)
    bq = np.asarray(inputs["bq"], np.float32)
    wkv = np.asarray(inputs["wkv"], np.float32)
    bkv = np.asarray(inputs["bkv"], np.float32)
    wqp = np.asarray(inputs["wqp"], np.float32)
    bqp = np.asarray(inputs["bqp"], np.float32)
    wkvp = np.asarray(inputs["wkvp"], np.float32)
    bkvp = np.asarray(inputs["bkvp"], np.float32)
    wg = np.asarray(inputs["wg"], np.float32)
    bg = np.asarray(inputs["bg"], np.float32)
    gw = np.asarray(inputs["geom_weight"], np.float32)
    hw = np.asarray(inputs["head_weights"], np.float32)
    sh = 1.0 / (1.0 + np.exp(-hw))
    gw0, gw1 = float(gw[0]), float(gw[1])

    wall = np.zeros((CS + 2, WALL_COLS), np.float32)
    wall[:CS, K_OFF:K_OFF + 192] = wkv[:, :192]
    wall[CS, K_OFF:K_OFF + 192] = bkv[:192]
    wall[:CS, V_OFF:V_OFF + 192] = wkv[:, 192:]
    wall[CS, V_OFF:V_OFF + 192] = bkv[192:]
    # kv pts pair-planar: block 2j+t (j = input col 0..2, t: 0=coord 1=dir)
    # dst col = PTS_OFF + block*144 + h*12 + p  <-  src h*72 + p*6 + cc
    cc, h, p = np.meshgrid(np.arange(6), np.arange(H), np.arange(12),
                           indexing="ij")
    blk = np.where(cc < 3, 2 * cc, 2 * (cc - 3) + 1)
    dst = (PTS_OFF + blk * 144 + h * 12 + p).ravel()
    src = (h * 72 + p * 6 + cc).ravel()
    wall[:CS, dst] = wkvp[:, src]
    wall[CS, dst] = bkvp[src]
    # q scaled by sh/sqrt(C)
    qs = np.repeat(sh / np.sqrt(C), 16)
    wall[:CS, Q_OFF:Q_OFF + 192] = wq * qs[None, :]
    wall[CS, Q_OFF:Q_OFF + 192] = bq * qs
    wall[:CS, G_OFF:G_OFF + 48] = wg
    wall[CS, G_OFF:G_OFF + 48] = bg
    # q pts pair-planar: dst col = QPTS_OFF + blk*48 + h*4 + p
    cc, h, p = np.meshgrid(np.arange(6), np.arange(H), np.arange(P),
                           indexing="ij")
    blk = np.where(cc < 3, 2 * cc, 2 * (cc - 3) + 1)
    dst = (QPTS_OFF + blk * 48 + h * 4 + p).ravel()
    src = (h * 24 + p * 6 + cc).ravel()
    wall[:CS, dst] = wqp[:, src]
    wall[CS, dst] = bqp[src]
    has_bias = bool(np.any(wall[CS] != 0.0))
    wall[CS + 1] = wall[CS] * 0.5
    wall[CS] = wall[CS + 1]

    bout_half = np.asarray(inputs["bout"], np.float32)[None, :] * 0.5
    wout_b = np.concatenate(
        [np.asarray(inputs["wout"], np.float32), bout_half, bout_half], axis=0)

    # on-chip constants (broadcast to 128 partitions by a rank-1 matmul)
    qconst = np.zeros((1, 144), np.float32)
    SC = gw0 * 0.5 * sh            # coord feature scale (with gate)
    DC = gw1 * sh                  # dir feature scale (with gate)
    qconst[0, 0:48] = np.repeat(SC, P)
    qconst[0, 48:96] = np.repeat(DC, P)
    qconst[0, 96:108] = sh * gw0 / P                      # qf[40]
    c2 = np.where(np.abs(gw0 * sh) > 1e-12, -1.0 / (gw0 * sh + 1e-30), 0.0)
    qconst[0, 108:120] = c2                               # q2 coefficient
    qconst[0, 120:132] = -sh * gw1                        # curvature coeff

    rot9 = np.asarray(inputs["rot"], np.float32).reshape(B, N, 9)
    trans = np.asarray(inputs["trans"], np.float32)
    rt_all = np.ascontiguousarray(np.concatenate([rot9, trans], axis=2))
    s = np.asarray(inputs["s"], np.float32)
    return s, rt_all, wall, wout_b, qconst, has_bias


_PROGRAM_CACHE = {}


def _build_program(has_bias):
    key = (bool(has_bias),)
    if key in _PROGRAM_CACHE:
        return _PROGRAM_CACHE[key]
    nc = bacc.Bacc("TRN2", target_bir_lowering=False, debug=False,
                   num_devices=8)
    s_all = nc.dram_tensor("s_all", [N, CS], F32, kind="ExternalInput")
    rt_d = nc.dram_tensor("rt_all", [N, 12], F32, kind="ExternalInput")
    wall_d = nc.dram_tensor("wall", [CS + 2, WALL_COLS], F32,
                            kind="ExternalInput")
    wout_d = nc.dram_tensor("wout_b", [FOUT + 2, CS], F32,
                            kind="ExternalInput")
    qconst_d = nc.dram_tensor("qconst", [1, 144], F32, kind="ExternalInput")
    out_loc = nc.dram_tensor("out_loc", [NB, CS], F32, kind="ExternalOutput")

    with tile.TileContext(nc) as tc:
        with ExitStack() as ctx:
            _emit(ctx, tc, nc, s_all, rt_d, wall_d, wout_d,
                  qconst_d, out_loc, has_bias)
    nc.compile()
    _PROGRAM_CACHE[key] = nc
    return nc


def _emit(ctx, tc, nc, s_all, rt_d, wall_d, wout_d, qconst_d,
          out_loc, has_bias):
    PS = bass.MemorySpace.PSUM

    const = ctx.enter_context(tc.tile_pool(name="const", bufs=1))
    work = ctx.enter_context(tc.tile_pool(name="work", bufs=1))
    tmp = ctx.enter_context(tc.tile_pool(name="tmp", bufs=2))
    pre_ctx = ExitStack()
    pA = pre_ctx.enter_context(tc.tile_pool(name="pA", bufs=1))
    kio = pre_ctx.enter_context(tc.tile_pool(name="kio", bufs=2))
    ppsum = pre_ctx.enter_context(tc.tile_pool(name="ppsum", bufs=2, space=PS))
    tpsum = pre_ctx.enter_context(tc.tile_pool(name="tpsum", bufs=2, space=PS))

    # ---- constants -------------------------------------------------------
    ident = const.tile([128, 128], F32)
    make_identity(nc, ident[:])
    ident_r = const.tile([OCH, OCH], F32R)
    nc.gpsimd.tensor_copy(ident_r[:], ident[0:OCH, 0:OCH])
    ones2_f32 = const.tile([2, NB], F32)
    nc.gpsimd.memset(ones2_f32[:], 1.0)
    ones_r = const.tile([2, 128], F32R)
    nc.gpsimd.tensor_copy(ones_r[:], ones2_f32[:, 0:128])

    # ---- DMAs (s first: the transposes+projections are the critical path;
    # issue on multiple engine queues to parallelize descriptor setup)
    s_sb = []
    for kt in range(NKT):
        t = pA.tile([128, CS], F32, name=f"s{kt}")
        nc.sync.dma_start(t[:], s_all[kt * 128:(kt + 1) * 128, :])
        s_sb.append(t)
    # all rot+trans rows in one DMA: rt_sb[:, kt*12+c] = rt_all[kt*128+p, c]
    rt_sb = const.tile([128, 96], F32, name="rt_sb")
    nc.gpsimd.dma_start(rt_sb[:],
                        rt_d[:, :].rearrange("(t p) c -> p t c", p=128))
    rtb_sb = const.tile([128, 96], BF16, name="rtb_sb")
    nc.gpsimd.tensor_copy(rtb_sb[:], rt_sb[:])

    def Rc(kt, j, b=False):
        t = rtb_sb if b else rt_sb
        return t[:, kt * 12 + j:kt * 12 + j + 1]

    def Tc(kt, j, b=False):
        t = rtb_sb if b else rt_sb
        return t[:, kt * 12 + 9 + j:kt * 12 + 9 + j + 1]

    wall_sb = []
    for kc in range(3):
        t = pA.tile([128, WALL_COLS], F32R, name=f"wall{kc}")
        nc.sync.dma_start(t[:], wall_d[kc * 128:(kc + 1) * 128, :].bitcast(F32R))
        wall_sb.append(t)
    wall_bias = pA.tile([2, WALL_COLS], F32R)
    if has_bias:
        nc.sync.dma_start(wall_bias[:], wall_d[CS:CS + 2, :].bitcast(F32R))

    qconst_sb = const.tile([1, 144], F32R)
    nc.gpsimd.dma_start(qconst_sb[:], qconst_d[:, :].bitcast(F32R))

    wout_sb = []
    for kc in range(KCH):
        r0 = kc * 128
        r1 = min(FOUT + 2, r0 + 128)
        t = const.tile([r1 - r0, CS], F32R, name=f"wout{kc}")
        wout_sb.append(t)

    def emit_wout_dmas():
        for kc in range(KCH):
            r0 = kc * 128
            r1 = min(FOUT + 2, r0 + 128)
            nc.sync.dma_start(wout_sb[kc][:], wout_d[r0:r1, :].bitcast(F32R))

    # ---- sT (transpose all of s) ----------------------------------------
    sT = pA.tile([128, 3 * N], F32R, name="sT")   # [:, kc*1024 + key]
    sT3 = sT[:].rearrange("p (c k) -> p c k", k=N)
    for kt in range(NKT):
        tps = tpsum.tile([128, 384], F32, tag="tps")
        for kc in range(3):
            nc.tensor.transpose(tps[:, kc * 128:(kc + 1) * 128],
                                s_sb[kt][:, kc * 128:(kc + 1) * 128], ident[:])
        dst = sT3[:, :, kt * 128:(kt + 1) * 128]
        src = tps[:].rearrange("p (c k) -> p c k", k=128)
        if kt % 2:
            nc.scalar.copy(dst, src)
        else:
            nc.vector.tensor_copy(dst, src)

    # ---- broadcast qconst row to 128 partitions --------------------------
    tps = tpsum.tile([128, 384], F32, tag="tps")
    nc.tensor.matmul(tps[:, 0:144], ones_r[0:1, :], qconst_sb[:, :],
                     start=True, stop=True)
    qcst = const.tile([128, 144], F32)
    nc.vector.tensor_copy(qcst[:], tps[:, 0:144])
    # slices: SC48 0:48 | DC48 48:96 | A12 96:108 | c2 108:120 | c3 120:132

    # ---- K/V side: all 8 key tiles --------------------------------------
    kfT = work.tile([128, 6 * N], BF16, name="kfT")   # [:, t*1024 + key]
    kfT3 = kfT[:].rearrange("p (t k) -> p t k", k=N)
    vaG = [work.tile([128, H * OCH], BF16, name=f"vaG{kb}")
           for kb in range(NKT)]
    kds = [work.tile([128, 144], F32, name=f"kds{qt}") for qt in range(2)]

    GROUPS_K = [(0, 384), (384, 896), (896, 1248)]

    def proj_mm(ps, c0, c1, kt):
        pv = ps[:, 0:c1 - c0]
        for kc in range(3):
            last = (kc == 2) and not has_bias
            nc.tensor.matmul(pv, sT3[:, kc, kt * 128:(kt + 1) * 128],
                             wall_sb[kc][:, c0:c1], start=(kc == 0), stop=last)
        if has_bias:
            nc.tensor.matmul(pv, ones_r[:, :], wall_bias[:, c0:c1],
                             start=False, stop=True)

    def transform(pts, pco, kt, W):
        """Rigid transform, pair-planar [dc_j|dd_j] blocks of 2W.

        The coord and dir chains for output comp i share the same rotation
        column, so each chain step runs once on the fused [128, 2W] pair.
        """
        W2 = 2 * W
        for i in range(3):
            dco = pco[:, i * W2:(i + 1) * W2]
            nc.scalar.activation(dco, pts[:, 0:W2], AF.Copy,
                                 scale=Rc(kt, 3 * i))
            nc.vector.scalar_tensor_tensor(dco, pts[:, W2:2 * W2],
                                           Rc(kt, 3 * i + 1, True), dco,
                                           OP.mult, OP.add)
            nc.vector.scalar_tensor_tensor(dco, pts[:, 2 * W2:3 * W2],
                                           Rc(kt, 3 * i + 2, True), dco,
                                           OP.mult, OP.add)
            # + translation on the coord half only
            nc.scalar.activation(pco[:, i * W2:i * W2 + W],
                                 pco[:, i * W2:i * W2 + W], AF.Identity,
                                 bias=Tc(kt, i))

    for kt in range(NKT):
        # projections: K+V | pts-a | pts-b
        ps_kv = ppsum.tile([128, 384], F32, tag="pg384", name="pskv")
        proj_mm(ps_kv, 0, 384, kt)
        ps_p1 = ppsum.tile([128, 512], F32, tag="pg512", name="psp1")
        proj_mm(ps_p1, 384, 896, kt)
        ps_p2 = ppsum.tile([128, 352], F32, tag="pg352", name="psp2")
        proj_mm(ps_p2, 896, 1248, kt)

        kf = kio.tile([128, H * FEAT], F32, tag="kf", name="kf", bufs=3)
        kfv = kf[:].rearrange("p (h f) -> p h f", f=FEAT)
        if kt < 2:
            # zero the pad cols 42:64 of this physical buffer once
            nc.gpsimd.memset(kfv[:, :, 42:64], 0.0)
        va = vaG[kt]
        vav = va[:].rearrange("p (h f) -> p h f", f=OCH)
        pts = kio.tile([128, 864], BF16, tag="pts", name="pts", bufs=3)

        # evacuations
        nc.scalar.copy(kfv[:, :, 0:16],
                       ps_kv[:, 0:192].rearrange("p (h c) -> p h c", c=16))
        nc.vector.tensor_copy(vav[:, :, 0:16],
                              ps_kv[:, 192:384].rearrange("p (h c) -> p h c", c=16))
        nc.scalar.activation(pts[:, 0:512], ps_p1[:], AF.Relu)
        nc.scalar.activation(pts[:, 512:864], ps_p2[:], AF.Relu)

        # rigid transform (planar, bf16)
        pco = kio.tile([128, 864], BF16, tag="pco", name="pco", bufs=3)
        transform(pts[:], pco[:], kt, 144)
        pco5 = pco[:].rearrange("p (j t h x) -> p j t h x", j=3, t=2, x=NPK)
        pco3 = pco[:].rearrange("p (c h x) -> p c h x", c=6, x=NPK)

        # kf coord/dir features ([cc*4+p] per head) + va pts (fused copies)
        nc.gpsimd.tensor_copy(
            kfv[:, :, 16:28].rearrange("p h (c x) -> p c h x", c=3),
            pco5[:, :, 0, :, 0:4])
        nc.gpsimd.tensor_copy(
            kfv[:, :, 28:40].rearrange("p h (c x) -> p c h x", c=3),
            pco5[:, :, 1, :, 0:4])
        nc.vector.tensor_copy(
            vav[:, :, 16:64].rearrange("p h (c x) -> p c h x", c=6),
            pco3[:, :, :, 4:12])
        nc.gpsimd.memset(vav[:, :, 64:65], 1.0)
        nc.gpsimd.memset(vav[:, :, 65:68], 0.0)

        # k2 (negated sum of squared coord features)
        sqs = tmp.tile([128, 144], F32, tag="sqs", name="sqs")
        nc.vector.tensor_tensor(
            sqs[:].rearrange("p (h x) -> p h x", x=12),
            kfv[:, :, 16:28], kfv[:, :, 16:28], OP.mult)
        nc.vector.tensor_reduce(
            kfv[:, :, 40], sqs[:].rearrange("p (h c x) -> p h c x", c=3, x=4),
            AX.XY, OP.add, negate=True)
        nc.gpsimd.memset(kfv[:, :, 41], 1.0)
        if kt < 2:
            nc.gpsimd.tensor_copy(kds[kt][:].rearrange("p (h x) -> p h x", x=12),
                                  kfv[:, :, 28:40])

        # transpose kf -> kfT (2 head-pairs per psum tile)
        for t0 in range(0, 6, 2):
            tps = tpsum.tile([128, 384], F32, tag="tps")
            nc.tensor.transpose(tps[:, 0:128],
                                kf[:, t0 * 128:(t0 + 1) * 128], ident[:])
            nc.tensor.transpose(tps[:, 128:256],
                                kf[:, (t0 + 1) * 128:(t0 + 2) * 128], ident[:])
            dst = kfT3[:, t0:t0 + 2, kt * 128:(kt + 1) * 128]
            src = tps[:, 0:256].rearrange("p (t k) -> p t k", k=128)
            if t0 == 2:
                nc.scalar.copy(dst, src)
            else:
                nc.vector.tensor_copy(dst, src)

    # ---- Q side (own rows = tiles 0..1) ---------------------------------
    qf_sb = [work.tile([128, H * FEAT], F32, name=f"qf{qt}") for qt in range(2)]
    for qt in range(2):
        qf = qf_sb[qt]
        qfv = qf[:].rearrange("p (h f) -> p h f", f=FEAT)
        ps_a = ppsum.tile([128, 384], F32, tag="pg384", name="psqa")
        proj_mm(ps_a, Q_OFF, Q_OFF + 384, qt)
        ps_b = ppsum.tile([128, 352], F32, tag="pg352", name="psqb")
        proj_mm(ps_b, Q_OFF + 384, WALL_COLS, qt)

        nc.scalar.copy(qfv[:, :, 0:16],
                       ps_a[:, 0:192].rearrange("p (h c) -> p h c", c=16))
        g_sb = tmp.tile([128, 48], F32, tag="gsb", name="gsb")
        nc.scalar.activation(g_sb[:], ps_a[:, 192:240], AF.Sigmoid)
        qpts = tmp.tile([128, 288], BF16, tag="qpts", name="qpts")
        nc.vector.tensor_scalar_max(qpts[:, 0:144], ps_a[:, 240:384], 0.0)
        nc.vector.tensor_scalar_max(qpts[:, 144:288], ps_b[:, 0:144], 0.0)

        qpco = tmp.tile([128, 288], F32, tag="qpco", name="qpco")
        transform(qpts[:], qpco[:], qt, 48)
        qpco5 = qpco[:].rearrange("p (j t h x) -> p j t h x", j=3, t=2, x=4)

        gc = tmp.tile([128, 48], F32, tag="gc", name="gc")
        gd = tmp.tile([128, 48], F32, tag="gd", name="gd")
        nc.vector.tensor_tensor(gc[:], g_sb[:], qcst[:, 0:48], OP.mult)
        nc.vector.tensor_tensor(gd[:], g_sb[:], qcst[:, 48:96], OP.mult)
        gc3 = gc[:].rearrange("p (h x) -> p h x", x=4)
        gd3 = gd[:].rearrange("p (h x) -> p h x", x=4)
        for cc in range(3):
            nc.vector.tensor_tensor(qfv[:, :, 16 + cc * 4:20 + cc * 4],
                                    qpco5[:, cc, 0], gc3, OP.mult)
            nc.gpsimd.tensor_tensor(qfv[:, :, 28 + cc * 4:32 + cc * 4],
                                    qpco5[:, cc, 1], gd3, OP.mult)
        nc.vector.tensor_copy(qfv[:, :, 40], qcst[:, 96:108])

        # q2 from coord features
        sqs = tmp.tile([128, 144], F32, tag="sqs", name="sqs")
        nc.vector.tensor_tensor(
            sqs[:].rearrange("p (h x) -> p h x", x=12),
            qfv[:, :, 16:28], qfv[:, :, 16:28], OP.mult)
        q2s = tmp.tile([128, 12], F32, tag="q2s", name="q2s")
        nc.vector.tensor_reduce(
            q2s[:], sqs[:].rearrange("p (h c x) -> p h c x", c=3, x=4),
            AX.XY, OP.add)

        # curvature from dir features vs raw kd features of same rows
        qdv = qfv[:, :, 28:40]
        kdv = kds[qt][:].rearrange("p (h x) -> p h x", x=12)
        crs = tmp.tile([128, 144], F32, tag="crs", name="crs")
        t1 = tmp.tile([128, 48], F32, tag="t1", name="t1")
        t2 = tmp.tile([128, 48], F32, tag="t2", name="t2")
        t13 = t1[:].rearrange("p (h x) -> p h x", x=4)
        t23 = t2[:].rearrange("p (h x) -> p h x", x=4)
        for c, (a, b2) in enumerate(((1, 2), (2, 0), (0, 1))):
            nc.vector.tensor_tensor(t13, qdv[:, :, a * 4:a * 4 + 4],
                                    kdv[:, :, b2 * 4:b2 * 4 + 4], OP.mult)
            nc.gpsimd.tensor_tensor(t23, qdv[:, :, b2 * 4:b2 * 4 + 4],
                                    kdv[:, :, a * 4:a * 4 + 4], OP.mult)
            nc.vector.tensor_tensor(crs[:, c * 48:(c + 1) * 48], t1[:], t2[:],
                                    OP.subtract)
        nc.vector.tensor_tensor(crs[:], crs[:], crs[:], OP.mult)
        csum = tmp.tile([128, 48], F32, tag="csum", name="csum")
        nc.vector.tensor_reduce(
            csum[:], crs[:].rearrange("p (c x) -> p x c", c=3), AX.X, OP.add)
        # |qfd|^2, |kd|^2 per (h,p)
        sqd = tmp.tile([128, 144], F32, tag="sqd", name="sqd")
        nq2 = tmp.tile([128, 48], F32, tag="nq2", name="nq2")
        nk2 = tmp.tile([128, 48], F32, tag="nk2", name="nk2")
        nc.gpsimd.tensor_tensor(sqd[:].rearrange("p (h x) -> p h x", x=12),
                                qdv, qdv, OP.mult)
        nc.vector.tensor_reduce(
            nq2[:].rearrange("p (h x) -> p h x", x=4),
            sqd[:].rearrange("p (h c x) -> p h x c", c=3, x=4), AX.X, OP.add)
        nc.gpsimd.tensor_tensor(sqd[:].rearrange("p (h x) -> p h x", x=12),
                                kdv, kdv, OP.mult)
        nc.vector.tensor_reduce(
            nk2[:].rearrange("p (h x) -> p h x", x=4),
            sqd[:].rearrange("p (h c x) -> p h x c", c=3, x=4), AX.X, OP.add)
        nc.vector.tensor_tensor(nq2[:], nq2[:], nk2[:], OP.mult)
        nc.scalar.activation(nq2[:], nq2[:], AF.Sqrt)
        nc.vector.tensor_scalar_add(nq2[:], nq2[:], EPS)
        nc.vector.reciprocal(nq2[:], nq2[:])
        nc.scalar.activation(csum[:], csum[:], AF.Sqrt)
        nc.vector.tensor_tensor(csum[:], csum[:], nq2[:], OP.mult)
        curv = tmp.tile([128, 12], F32, tag="curv", name="curv")
        nc.vector.tensor_reduce(
            curv[:], csum[:].rearrange("p (h x) -> p h x", x=4), AX.X, OP.add)
        # qf[41] = c2*q2s + c3*curv
        nc.vector.tensor_tensor(q2s[:], q2s[:], qcst[:, 108:120], OP.mult)
        nc.vector.tensor_tensor(curv[:], curv[:], qcst[:, 120:132], OP.mult)
        nc.vector.tensor_tensor(qfv[:, :, 41], q2s[:], curv[:], OP.add)

    # ---- qfT: masked transposes (even head | odd head halves) -----------
    qfT = [work.tile([128, 2 * NB], BF16, name=f"qfT{t}") for t in range(6)]
    for t in range(6):
        nc.gpsimd.memset(qfT[t][:], 0.0)
    for t in range(6):
        for qt in range(2):
            tps = tpsum.tile([128, 384], F32, tag="tps")
            nc.tensor.transpose(tps[:, 0:128],
                                qf_sb[qt][:, t * 128:(t + 1) * 128], ident[:])
            eng = nc.scalar if (t + qt) % 2 else nc.vector
            (eng.copy if eng is nc.scalar else eng.tensor_copy)(
                qfT[t][0:FS, qt * 128:(qt + 1) * 128], tps[0:FS, 0:128])
            eng2 = nc.vector if (t + qt) % 2 else nc.scalar
            (eng2.copy if eng2 is nc.scalar else eng2.tensor_copy)(
                qfT[t][64:64 + FS, NB + qt * 128:NB + (qt + 1) * 128],
                tps[64:64 + FS, 0:128])

    # ---- attention -------------------------------------------------------
    emit_wout_dmas()
    pre_ctx.close()
    att_ctx = ExitStack()
    apsum = att_ctx.enter_context(tc.tile_pool(name="apsum", bufs=2, space=PS))
    opsum = att_ctx.enter_context(tc.tile_pool(name="opsum", bufs=2, space=PS))
    otp = att_ctx.enter_context(tc.tile_pool(name="otp", bufs=2, space=PS))
    expT_tiles = [work.tile([128, 4096], BF16, name=f"expT{i}")
                  for i in range(3)]
    o_all = [work.tile([128, FEAT * H], F32, name=f"oall{qt}")
             for qt in range(2)]
    feats = [work.tile([128, FOUT], F32, name=f"feats{qt}") for qt in range(2)]
    ld_sb = [work.tile([128, 288], F32, name=f"ld{qt}") for qt in range(2)]
    RUNP = 2

    def emit_qk_exp(t):
        expT = expT_tiles[t % 3]
        for p4 in range(4):
            aps = apsum.tile([128, 1024], F32, tag="aps", name="aps")
            for j in range(2):
                kb = p4 * 2 + j
                nc.tensor.matmul(aps[:, j * 512:(j + 1) * 512],
                                 kfT3[:, t, kb * 128:(kb + 1) * 128],
                                 qfT[t][:, :], start=True, stop=True)
            nc.scalar.activation(expT[:, p4 * 1024:(p4 + 1) * 1024], aps[:],
                                 AF.Exp)

    def emit_av(h):
        t, e = h // 2, h % 2
        expT = expT_tiles[t % 3]
        ot_ps = opsum.tile([OCH, NB], F32, tag="ot", name="ot_ps")
        for kb in range(NKT):
            nc.tensor.matmul(
                ot_ps[:], vaG[kb][:, h * OCH:(h + 1) * OCH],
                expT[:, kb * 512 + e * NB:kb * 512 + (e + 1) * NB],
                start=(kb == 0), stop=(kb == NKT - 1))
        ot_sb = tmp.tile([OCH, NB], F32R, tag="otsb", name="otsb", bufs=2)
        nc.vector.tensor_copy(ot_sb[:], ot_ps[:])
        for qt in range(2):
            tp = otp.tile([128, OCH], F32R, tag="tp", name="tp")
            nc.tensor.transpose(tp[:], ot_sb[:, qt * 128:(qt + 1) * 128],
                                ident_r[:, :])
            rec = tmp.tile([128, 1], F32, tag="rec", name="rec", bufs=2)
            nc.vector.reciprocal(rec[:], tp[:, 64:65].bitcast(F32))
            nc.vector.tensor_scalar_mul(
                o_all[qt][:, h * FEAT:h * FEAT + 64], tp[:, 0:64].bitcast(F32),
                rec[:])

    def emit_inv_rot(qt, hh):
        """Rotate o_geom back to local frame for heads hh*6..hh*6+5."""
        hs = slice(hh * 6, hh * 6 + 6)
        ov = o_all[qt][:].rearrange("p (h f) -> p h f", f=FEAT)[:, hs]
        gv = feats[qt][:, 192:FOUT].rearrange(
            "p (h x c) -> p h x c", h=H, c=7)[:, hs]

        def og(j):
            return ov[:, :, 16 + 16 * j:24 + 16 * j]

        ogs = tmp.tile([128, 144], F32, tag="ogs", name="ogs", bufs=2)
        ogs3 = ogs[:].rearrange("p (c x) -> p c x", c=3)
        for j in range(3):
            nc.vector.tensor_scalar(
                ogs3[:, j].rearrange("p (h x) -> p h x", x=V), og(j),
                Tc(qt, j), None, OP.subtract)
        lci = tmp.tile([128, 48], F32, tag="lci", name="lci", bufs=2)
        for i in range(3):
            nc.vector.tensor_scalar_mul(lci[:], ogs3[:, 0], Rc(qt, i))
            nc.vector.scalar_tensor_tensor(lci[:], ogs3[:, 1],
                                           Rc(qt, 3 + i), lci[:],
                                           OP.mult, OP.add)
            nc.vector.scalar_tensor_tensor(
                gv[:, :, :, i], ogs3[:, 2].rearrange("p (h x) -> p h x", x=V),
                Rc(qt, 6 + i),
                lci[:].rearrange("p (h x) -> p h x", x=V), OP.mult, OP.add)
            ldd = ld_sb[qt][:, i * 96 + hh * 48:i * 96 + (hh + 1) * 48]
            ldd3 = ldd.rearrange("p (h x) -> p h x", x=V)
            nc.vector.tensor_scalar_mul(ldd3, ov[:, :, 24:32], Rc(qt, i))
            nc.vector.scalar_tensor_tensor(ldd3, ov[:, :, 40:48],
                                           Rc(qt, 3 + i), ldd3,
                                           OP.mult, OP.add)
            nc.vector.scalar_tensor_tensor(ldd3, ov[:, :, 56:64],
                                           Rc(qt, 6 + i), ldd3,
                                           OP.mult, OP.add)

    def emit_inv_norm(qt):
        gv = feats[qt][:, 192:FOUT].rearrange("p (h x c) -> p h x c", h=H, c=7)
        lsq = tmp.tile([128, 288], F32, tag="lsq", name="lsq")
        lsq4 = lsq[:].rearrange("p (h x c) -> p h x c", c=3, x=V)
        nc.vector.tensor_tensor(lsq4, gv[:, :, :, 0:3], gv[:, :, :, 0:3],
                                OP.mult)
        ncs = tmp.tile([128, 96], F32, tag="ncs", name="ncs")
        nc.vector.tensor_reduce(
            ncs[:], lsq[:].rearrange("p (x c) -> p x c", c=3), AX.X, OP.add)
        nc.scalar.activation(gv[:, :, :, 6],
                             ncs[:].rearrange("p (h x) -> p h x", x=V), AF.Sqrt)
        # ld normalization
        ldq = ld_sb[qt]
        nc.gpsimd.tensor_tensor(lsq[:], ldq[:], ldq[:], OP.mult)
        nds = tmp.tile([128, 96], F32, tag="nds", name="nds")
        nc.vector.tensor_reduce(
            nds[:], lsq[:].rearrange("p (c x) -> p x c", c=3), AX.X, OP.add)
        nc.scalar.activation(nds[:], nds[:], AF.Sqrt)
        nc.vector.tensor_scalar_max(nds[:], nds[:], EPS)
        nc.vector.reciprocal(nds[:], nds[:])
        nds3 = nds[:].rearrange("p (h x) -> p h x", x=V)
        for i in range(3):
            nc.gpsimd.tensor_tensor(
                gv[:, :, :, 3 + i],
                ldq[:, i * 96:(i + 1) * 96].rearrange("p (h x) -> p h x", x=V),
                nds3, OP.mult)
        nc.gpsimd.tensor_copy(
            feats[qt][:, 0:192].rearrange("p (h c) -> p h c", c=16),
            o_all[qt][:].rearrange("p (h f) -> p h f", f=FEAT)[:, :, 0:16])

    for t in range(6 + RUNP):
        if t < 6:
            emit_qk_exp(t)
        if t >= RUNP:
            emit_av(2 * (t - RUNP))
            emit_av(2 * (t - RUNP) + 1)
            if t - RUNP == 2:
                emit_inv_rot(0, 0)
                emit_inv_rot(1, 0)
    # ---- inverse norms + output projection, pipelined per query tile -----
    att_ctx.close()
    tpsum2 = ctx.enter_context(tc.tile_pool(name="tpsum2", bufs=2, space=PS))
    opsum2 = ctx.enter_context(tc.tile_pool(name="opsum2", bufs=2, space=PS))
    fT = []
    for kc in range(KCH):
        r0 = kc * 128
        rw = min(FOUT, r0 + 128) - r0
        pw = rw + 2 if kc == KCH - 1 else rw
        fT.append(work.tile([pw, NB], F32R, name=f"fT{kc}"))
    lastr = FOUT - (KCH - 1) * 128
    nc.gpsimd.tensor_copy(fT[KCH - 1][lastr:lastr + 2, :], ones2_f32[:])
    for qt in range(2):
        emit_inv_rot(qt, 1)
        emit_inv_norm(qt)
        for kc in range(KCH):
            r0 = kc * 128
            rw = min(FOUT, r0 + 128) - r0
            ps = tpsum2.tile([128, 128], F32, tag="tps2")
            nc.tensor.transpose(ps[:rw, :], feats[qt][:, r0:r0 + rw], ident[:])
            if kc % 2:
                nc.scalar.copy(fT[kc][:rw, qt * 128:(qt + 1) * 128], ps[:rw, :])
            else:
                nc.vector.tensor_copy(fT[kc][:rw, qt * 128:(qt + 1) * 128],
                                      ps[:rw, :])
        ps = opsum2.tile([128, CS], F32, tag="oproj")
        for kc in range(KCH):
            nc.tensor.matmul(ps[:], fT[kc][:, qt * 128:(qt + 1) * 128],
                             wout_sb[kc][:], start=(kc == 0),
                             stop=(kc == KCH - 1))
        osb = tmp.tile([128, CS], F32, tag="osb", name="osb")
        nc.scalar.copy(osb[:], ps[:])
        nc.sync.dma_start(out_loc[qt * 128:(qt + 1) * 128, :], osb[:])


def _run(inputs, trace=False):
    s, rt_all, wall, wout_b, qconst, has_bias = _host_prep(inputs)
    nc = _build_program(has_bias)
    in_maps = []
    for c in range(8):
        b, qb = c // 4, c % 4
        # rotate key rows so this core's queries are rows 0:256
        idx = np.r_[qb * NB:N, 0:qb * NB]
        in_maps.append({
            "s_all": np.ascontiguousarray(s[b][idx]),
            "rt_all": np.ascontiguousarray(rt_all[b][idx]),
            "wall": wall, "wout_b": wout_b, "qconst": qconst,
        })
    res = run_bass_kernel_spmd(nc, in_maps, list(range(8)), trace=trace)
    out = np.empty((B, N, CS), np.float32)
    for c in range(8):
        b, qb = c // 4, c % 4
        out[b, qb * NB:(qb + 1) * NB] = res.results[c]["out_loc"]
    return out, res


def kernel(**inputs):
    out, _ = _run(inputs, trace=False)
    return out


def kernel_traced(**inputs):
    return _run(inputs, trace=True)
